# revision 25
# baseline (speedup 1.0000x reference)
"""Trainium2 Bass kernel for nn_DocREModel (DocRE: gather -> RGCN -> SE -> 5x5 convs).

Sharding: 4 documents x 2 cores each. Each pair replicates the cheap upstream
(mention/link/ea gathers -> RGCN -> fmap/SE) and splits the dominant 5x5 conv
stack by output channels, with intra-pair AllGathers; output halves are
assembled on host. All index-driven gathers happen on host (pure data
movement; one SPMD program serves all 8 cores), all dense math on device.

Perf notes (v3):
- DMA is a serialized ~360B/ns resource in the cost model; all input loads
  go on ONE queue (sync/SP) in exact consumption order, in <=~800KB pieces,
  so arrival order is deterministic and matches the compute schedule.
- The PE p-state ramp resets on >~2us idle gaps. Tiny "touch" matmuls that
  read one column of each arriving DMA piece pulse the PE at piece cadence,
  holding the fast clock through DMA-bound phases.
- bf16 for W_trans, gathered activations, RGCN + conv weights (f32 PSUM
  accumulation everywhere); f32r for the remaining f32 path.
- Convs are 25 shift-tap matmuls over zero-padded 26x26 images via strided
  APs. conv1/conv2 outputs are produced in two 11-row halves so each half's
  AllGather (+ readback + mask-combine) overlaps the next half's matmuls;
  conv3 output is relu'd + stored per (out-chunk, row-half) to shrink the
  kernel tail. RGCN folds the self-loop in as a 4th identity relation and
  pipelines each layer by y column halves.
"""

import numpy as np
import ml_dtypes

import concourse.bacc as bacc
import concourse.tile as tile
from concourse import mybir
from concourse.bass_utils import run_bass_kernel_spmd

F32 = mybir.dt.float32
F32R = mybir.dt.float32r
BF16 = mybir.dt.bfloat16
AF = mybir.ActivationFunctionType
ALU = mybir.AluOpType

NB, H, C, HID, EMB = 4, 12, 1024, 768, 512
E, M, L, SPAN = 22, 4, 16, 32
TD, INTER = 20, 256
NN = E + E * M + L
NREL, NLAYERS = 3, 4
EM, EMH, HS, LS = E * M, E * M * H, H * SPAN, L * SPAN
D0 = EMB + TD           # 532
EE = E * E              # 484
PADW = 26 * 26          # 676 padded 26x26 image
N_CORES = 8


def _build_adj():
    A = np.zeros((NREL, NN, NN), np.float32)
    for e in range(E):
        for m in range(M):
            mi = E + e * M + m
            A[0, e, mi] = A[0, mi, e] = 1.0
            for m2 in range(M):
                if m2 != m:
                    A[1, mi, E + e * M + m2] = 1.0
            li = E + E * M + ((e * M + m) % L)
            A[2, mi, li] = A[2, li, mi] = 1.0
    A = A / (A.sum(-1, keepdims=True) + 1e-5)
    return A


_TYPES = np.concatenate([np.zeros(E, np.int32), np.ones(EM, np.int32),
                         np.full(L, 2, np.int32)])

_KC0 = [(0, 128), (128, 128), (256, 128), (384, 128), (512, 20)]   # 532 rows
_KC1 = [(0, 128), (128, 128), (256, 128), (384, 128)]              # 512 rows


def _const_layout():
    """f32r constants [128, CR]: stage-1 smalls + SE weights."""
    lay = {}
    c = 0

    def add(nm, cols):
        nonlocal c
        lay[nm] = (c, cols)
        c += cols
    add("g2T", E)
    for kc in range(4):
        add(f"sumT{kc}", L)
    for kc in range(4):
        add(f"fsw1T{kc}", INTER)
    for kc in range(4):
        add(f"fcw1T{kc}", INTER)
    for kc in range(2):
        add(f"fsw2T{kc}", EMB)
    for kc in range(2):
        add(f"fcw2T{kc}", EMB)
    return lay, c


def _constf_layout():
    lay = {}
    c = 0

    def add(nm, cols):
        nonlocal c
        lay[nm] = (c, cols)
        c += cols
    for nm, nch in (("ses1", 2), ("seb1", 2), ("fcs1", 2), ("fcb1", 2),
                    ("ses2", 4), ("seb2", 4), ("fcs2", 4), ("fcb2", 4)):
        for kc in range(nch):
            add(f"{nm}{kc}", 1)
    add("b1h", 1)
    add("b2h", 1)
    add("b3h0", 1)
    add("b3h1", 1)
    add("mtop", 1)
    add("mbot", 1)
    add("identf", 128)
    return lay, c


def _actr_layout():
    """bf16 gathered activations [128, CA]."""
    lay = {}
    c = 0

    def add(nm, cols):
        nonlocal c
        lay[nm] = (c, cols)
        c += cols
    for kc in range(6):
        add(f"xmT{kc}", EM)
    for kc in range(3):
        add(f"attl{kc}", LS)
    for kc in range(6):
        add(f"xspT{kc}", LS)
    return lay, c


_LAY_R, _CR = _const_layout()
_LAY_F, _CF = _constf_layout()
_LAY_A, _CA = _actr_layout()

_CRB = _LAY_R["fsw1T0"][0]          # stage-1 smalls | SE weights split
_CAA = _LAY_A["xspT0"][0]           # xmT+attl | xspT split
_CW = 7 * EMB                       # constrw (bf16): wtr chunks + brow block


def build_program(solo=False, stages=4):
    nc = bacc.Bacc("TRN2", target_bir_lowering=False, debug=False)

    def din(name, shape, dt=F32R):
        return nc.dram_tensor(name, list(shape), dt, kind="ExternalInput").ap()

    constr_d = din("constr", [128, _CR])
    constf_d = din("constf", [128, _CF], F32)
    constw_d = din("constw", [128, _CW], BF16)
    actr_d = din("actr", [128, _CA], BF16)
    xp_d = din("xp", [128, 8 * HID], BF16)
    amp_d = din("amp", [128, 9 * C], BF16)
    gTb_d = din("gTb", [128, 9 * E], BF16)
    wstp_d = [din("wstp0", [128, 20 * EMB], BF16)] + \
             [din(f"wstp{i}", [128, 16 * EMB], BF16) for i in (1, 2)] + \
             [din("wstp3", [128, 8 * EMB], BF16)]
    w1sb_d = din("w1sb", [4, 128, 25 * 128], BF16)
    w2sb_d = din("w2sb", [2, 128, 25 * 128], BF16)
    w3sb_d = din("w3sb", [2, 128, 25 * 256], BF16)
    aallTb_d = din("aallTb", [128, 4 * (NREL + 1) * NN + 2 * E], BF16)
    tfb_d = din("tfb", [NN, TD], BF16)
    identb_d = din("identb", [128, 128], BF16)

    out_d = nc.dram_tensor("out", [256, EE], F32, kind="ExternalOutput").ap()

    groups = [[0, 1], [2, 3], [4, 5], [6, 7]]

    with tile.TileContext(nc) as tc:
      with tc.tile_pool(name="pconst", bufs=1) as pconst, \
           tc.tile_pool(name="pwork", bufs=1) as pwork, \
           tc.tile_pool(name="pwarm", bufs=1, space="PSUM") as pwarm, \
           tc.tile_pool(name="pdram", bufs=1, space="DRAM") as pdram:
        prgw_cm = tc.tile_pool(name="prgw", bufs=1)
        prgw = prgw_cm.__enter__()

        constr = pconst.tile([128, _CR], F32R)
        constf = pconst.tile([128, _CF], F32)
        constw = pconst.tile([128, _CW], BF16)
        identb = pconst.tile([128, 128], BF16)
        aallTb = pconst.tile([128, 4 * (NREL + 1) * NN + 2 * E], BF16)
        onesb = pconst.tile([128, 128], BF16)
        nc.vector.memset(onesb[:], 1.0)
        # scratch for the sigmoid-table swap dummy (see ep below)
        actwarm = pconst.tile([1, 1], F32)

        def cr(nm, rows=128):
            c0, cols = _LAY_R[nm]
            return constr[0:rows, c0:c0 + cols]

        def cf(nm, rows=128):
            c0, cols = _LAY_F[nm]
            return constf[0:rows, c0:c0 + cols]

        wtr = [constw[:, kc * EMB:(kc + 1) * EMB] for kc in range(6)]
        brow = constw[0:1, 6 * EMB:7 * EMB]
        onescol = onesb[:, 0:1]
        onesrow = onesb[0:1, :]
        g2T = cr("g2T", rows=EM)
        sumT = [cr(f"sumT{kc}") for kc in range(4)]
        sew = {nm: [cr(f"{nm}{kc}") for kc in range(n)]
               for nm, n in (("fsw1T", 4), ("fcw1T", 4), ("fsw2T", 2),
                             ("fcw2T", 2))}
        sev = {nm: [cf(f"{nm}{kc}") for kc in range(n)]
               for nm, n in (("ses1", 2), ("seb1", 2), ("fcs1", 2), ("fcb1", 2),
                             ("ses2", 4), ("seb2", 4), ("fcs2", 4),
                             ("fcb2", 4))}
        b1h = cf("b1h")
        b2h = cf("b2h")
        b3h = [cf("b3h0"), cf("b3h1")]
        ident = cf("identf")

        # conv pad images: memset early while the DVE is idle
        own1 = pwork.tile([128, PADW], BF16)
        oth1 = pwork.tile([128, PADW], BF16)
        own2 = pwork.tile([128, PADW], BF16)
        oth2 = pwork.tile([128, PADW], BF16)
        for t_ in (own1, oth1, own2, oth2):
            nc.vector.memset(t_[:], 0.0)

        # persistent intermediates: h0 kept as three partition-0 node-type
        # parts (entities / mentions / links); RGCN layer 0 contracts over
        # the parts with a 3-part adjacency tensor.
        eln_b = pwork.tile([E, D0], BF16)
        mrep_b = pwork.tile([EM, D0], BF16)
        link_b = pwork.tile([L, D0], BF16)
        h0p = [(mrep_b, EM), (link_b, L), (eln_b, E)]
        ectxT_sb = [pwork.tile([128, E], F32, tag=f"ectxT{i}", name=f"ectxT{i}")
                    for i in range(4)]
        z_sb = [pwork.tile([128, E], BF16, tag=f"z{i}", name=f"z{i}")
                for i in range(6)]
        easumT = pwork.tile([1, E], BF16)
        zt_sb = pwork.tile([E, HID], F32)
        ec2_sb = pwork.tile([E, EMB], F32)

        # ---------------- ordered DMA supply script (sync/SP queue) ---------
        # Exact consumption order, <=~800KB pieces. "touch" matmuls (fired at
        # chosen program points) read one column of an arriving DMA piece and
        # pulse the PE, so the p-state ramp never resets during DMA-bound
        # stretches.
        warm = pwarm.tile([1, 1], F32)
        tch = {}

        def fire(*names):
            for nm in names:
                nc.tensor.matmul(warm[:], tch[nm], tch[nm],
                                 start=True, stop=True)

        gTbt = prgw.tile([128, 9 * E], BF16)
        nc.scalar.dma_start(gTbt[:], gTb_d[:])
        nc.sync.dma_start(constf[:], constf_d[:])
        amp = prgw.tile([128, 9 * C], BF16)
        for (p0, pn) in ((0, 1), (1, 2), (3, 2), (5, 2), (7, 2)):
            nc.sync.dma_start(amp[:, p0 * C:(p0 + pn) * C],
                              amp_d[:, p0 * C:(p0 + pn) * C])
        xp = prgw.tile([128, 8 * HID], BF16)
        for i, p0 in enumerate(range(0, 8, 2)):
            nc.sync.dma_start(xp[:, p0 * HID:(p0 + 2) * HID],
                              xp_d[:, p0 * HID:(p0 + 2) * HID])
            tch[f"xp{i}"] = xp[:, p0 * HID:p0 * HID + 1]
        for i, (p0, pn) in enumerate(((0, 3), (3, 4))):   # wtr + brow block
            nc.sync.dma_start(constw[:, p0 * EMB:(p0 + pn) * EMB],
                              constw_d[:, p0 * EMB:(p0 + pn) * EMB])
            tch[f"wtr{i}"] = constw[:, p0 * EMB:p0 * EMB + 1]
        nc.sync.dma_start(constr[:, 0:_CRB], constr_d[:, 0:_CRB])
        actr = prgw.tile([128, _CA], BF16)
        nc.sync.dma_start(actr[:, 0:_CAA], actr_d[:, 0:_CAA])
        tch["actr0"] = actr[:, 0:1]
        nc.sync.dma_start(actr[:, _CAA:_CA], actr_d[:, _CAA:_CA])
        tch["actr1"] = actr[:, _CAA:_CAA + 1]
        # RGCN weights
        wstp_t = [prgw.tile([128, 20 * EMB], BF16, tag="wstp0",
                            name="wstp0")] + \
                 [prgw.tile([128, 16 * EMB], BF16, tag=f"wstp{i}",
                            name=f"wstp{i}") for i in (1, 2)] + \
                 [prgw.tile([128, 8 * EMB], BF16, tag="wstp3", name="wstp3")]
        for layer in range(NLAYERS):
            ncols = (20 * EMB if layer == 0 else
                     16 * EMB if layer < 3 else 8 * EMB)
            hh = ncols // 2
            for i, (p0, pn) in enumerate(((0, hh), (hh, ncols - hh))):
                nc.sync.dma_start(wstp_t[layer][:, p0:p0 + pn],
                                  wstp_d[layer][:, p0:p0 + pn])
                tch[f"wstp{layer}{i}"] = wstp_t[layer][:, p0:p0 + 1]
        # SE weights
        nc.sync.dma_start(constr[:, _CRB:_CR], constr_d[:, _CRB:_CR])
        tch["sew"] = constr[:, _CRB:_CRB + 1].bitcast(F32)


        def ca(nm, rows=128):
            c0, cols = _LAY_A[nm]
            return actr[0:rows, c0:c0 + cols]

        xmT = [ca(f"xmT{kc}") for kc in range(6)]
        xspT = [ca(f"xspT{kc}") for kc in range(6)]
        attl = [ca(f"attl{kc}") for kc in range(3)]

        def fire_touches():
            for ap_sl in touches:
                nc.tensor.matmul(warm[:], ap_sl, ap_sl, start=True, stop=True)

        # ================= stage 1: gathered-row transforms =================
        with tc.tile_pool(name="pbig", bufs=1) as pbig:
            expm = pbig.tile([EM, EMB], F32R)
            sp_ps = []
            wsb = [pbig.tile([128, 1], F32, tag=f"wsb{i}", name=f"wsb{i}")
                   for i in range(4)]
            wsp = [pbig.tile([128, EMB], F32R, tag=f"wsp{i}", name=f"wsp{i}")
                   for i in range(4)]
            ea_sb = pbig.tile([E, C], F32R)
            eaT = [pbig.tile([128, E], BF16, tag=f"eaT{i}", name=f"eaT{i}")
                   for i in range(8)]


            with tc.tile_pool(name="ps1b", bufs=1, space="PSUM") as ps1b:
                # ea = G^T @ attm ; normalize rows
                ea_p0 = ps1b.tile([E, 512], F32, tag="ea0", name="ea0")
                ea_p1 = ps1b.tile([E, 512], F32, tag="ea1", name="ea1")
                for kc in range(9):
                    rows = 128 if kc < 8 else 32
                    at = amp[0:rows, kc * C:kc * C + C]
                    gt = gTbt[0:rows, kc * E:(kc + 1) * E]
                    nc.tensor.matmul(ea_p0[:], gt, at[:, 0:512],
                                     start=(kc == 0), stop=(kc == 8))
                    nc.tensor.matmul(ea_p1[:], gt, at[:, 512:1024],
                                     start=(kc == 0), stop=(kc == 8))
                fire("xp0", "xp1")
                r0 = pbig.tile([E, 1], F32)
                r1 = pbig.tile([E, 1], F32)
                nc.vector.tensor_reduce(r0[:], ea_p0[:], mybir.AxisListType.X,
                                        ALU.add)
                nc.vector.tensor_reduce(r1[:], ea_p1[:], mybir.AxisListType.X,
                                        ALU.add)
                rsum = pbig.tile([E, 1], F32)
                nc.vector.tensor_tensor(out=rsum[:], in0=r0[:], in1=r1[:],
                                        op=ALU.add)
                rsum2 = pbig.tile([E, 1], F32)
                nc.vector.tensor_scalar(out=rsum2[:], in0=rsum[:], scalar1=1e-5,
                                        scalar2=None, op0=ALU.add)
                rinv = pbig.tile([E, 1], F32)
                nc.vector.reciprocal(rinv[:], rsum2[:])
                # ea left unnormalized; rinv is applied as a row scale on the
                # zt copy below (zt is linear in ea), shortening the critical
                # chain ea -> eaT -> zt.
                nc.scalar.activation(ea_sb[:, 0:512], ea_p0[:], AF.Copy)
                nc.scalar.activation(ea_sb[:, 512:1024], ea_p1[:], AF.Copy)
                easum = pbig.tile([E, 1], F32)
                nc.vector.tensor_tensor(out=easum[:], in0=rsum[:], in1=rinv[:],
                                        op=ALU.mult)
                # small late-use constants; placed here so their HWDGE slots
                # don't delay the early amp pieces
                nc.scalar.dma_start(eln_b[0:E, EMB:D0], tfb_d[0:E, :])
                nc.scalar.dma_start(mrep_b[0:EM, EMB:D0], tfb_d[E:E + EM, :])
                nc.scalar.dma_start(link_b[0:L, EMB:D0], tfb_d[E + EM:NN, :])
                nc.scalar.dma_start(aallTb[:], aallTb_d[:])
                nc.scalar.dma_start(identb[:], identb_d[:])
                for kc in range(8):
                    if kc == 2:
                        fire("xp2")
                    elif kc == 4:
                        fire("xp3")
                    tp = ps1b.tile([128, E], F32, tag="eaTt", name="eaTt", bufs=2)
                    nc.tensor.transpose(tp[:],
                                        ea_sb[:, kc * 128:(kc + 1) * 128]
                                        .bitcast(F32), ident[0:E, 0:E])
                    if kc % 2 == 0:
                        nc.scalar.copy(eaT[kc][:], tp[:])
                    else:
                        nc.vector.tensor_copy(out=eaT[kc][:], in_=tp[:])
                tp = ps1b.tile([1, E], F32, tag="easumt", name="easumt")
                nc.tensor.transpose(tp[:], easum[:], ident[0:E, 0:E])
                nc.scalar.copy(easumT[:], tp[:])

            with tc.tile_pool(name="ps1c", bufs=1, space="PSUM") as ps1c:
                # zT = ea @ x  [22, 768] (two 384-wide halves); rinv applied
                # on the copy out of PSUM
                zt_ps = [ps1c.tile([E, 384], F32, tag=f"zt_p{i}",
                                   name=f"zt_p{i}") for i in range(2)]
                for kc in range(8):
                    if kc == 3:
                        fire("wtr0")
                    elif kc == 6:
                        fire("wtr1")
                    xt = xp[:, kc * HID:(kc + 1) * HID]
                    for hh in range(2):
                        nc.tensor.matmul(zt_ps[hh][:], eaT[kc][:],
                                         xt[:, hh * 384:(hh + 1) * 384],
                                         start=(kc == 0), stop=(kc == 7))
                fire("actr0")
                nc.scalar.activation(zt_sb[:, 0:384], zt_ps[0][:], AF.Copy,
                                     scale=rinv[:])
                nc.scalar.activation(zt_sb[:, 384:768], zt_ps[1][:], AF.Copy,
                                     scale=rinv[:])

            with tc.tile_pool(name="ps1a", bufs=1, space="PSUM") as ps1a:
                # mentions: mrep = x_m @ Wtr + b -> straight into h0b (bf16)
                mrep_p = ps1a.tile([EM, EMB], F32, tag="mrep", name="mrep")
                for kc in range(6):
                    nc.tensor.matmul(mrep_p[:], xmT[kc][:, 0:EM], wtr[kc][:],
                                     start=(kc == 0), stop=False)
                nc.tensor.matmul(mrep_p[:], onesrow[0:1, 0:EM], brow[:],
                                 start=False, stop=True)
                nc.scalar.activation(mrep_b[0:EM, 0:EMB], mrep_p[:], AF.Copy)
                nc.scalar.activation(expm[:], mrep_p[:], AF.Exp)
                # e_rep = ln(G2 @ exp(mrep))
                ep_p = ps1a.tile([E, EMB], F32, tag="ep", name="ep")
                nc.tensor.matmul(ep_p[:], g2T[:], expm[:], start=True, stop=True)
                nc.scalar.activation(eln_b[0:E, 0:EMB], ep_p[:], AF.Ln)
                # swap to the sigmoid table now (every later act is in it);
                # reading ep_p pins this after the Ln in the schedule
                nc.scalar.activation(actwarm[:], ep_p[0:1, 0:1], AF.Sigmoid)

                # w = colsum(attl) / 384
                for mc in range(4):
                    w_p = ps1a.tile([128, 1], F32, tag="w_p", name="w_p", bufs=1)
                    for kc in range(3):
                        nc.tensor.matmul(w_p[:],
                                         attl[kc][:, mc * 128:(mc + 1) * 128],
                                         onescol[:],
                                         start=(kc == 0), stop=(kc == 2))
                    nc.scalar.activation(wsb[mc][:], w_p[:], AF.Copy,
                                         scale=1.0 / (H * SPAN))
                fire("actr1")
                # spans: sp = x_span @ Wtr + b
                for mc in range(4):
                    if mc > 0:
                        fire(f"wstp0{mc - 1}" if mc < 3 else "wstp10")
                    sp_p = ps1a.tile([128, EMB], F32, tag="sp_p", name="sp_p",
                                     bufs=3)
                    for kc in range(6):
                        nc.tensor.matmul(sp_p[:],
                                         xspT[kc][:, mc * 128:(mc + 1) * 128],
                                         wtr[kc][:], start=(kc == 0), stop=False)
                    nc.tensor.matmul(sp_p[:], onesrow[:], brow[:],
                                     start=False, stop=True)
                    spc = pbig.tile([128, EMB], F32, tag="spc", name="spc",
                                    bufs=4)
                    nc.scalar.copy(spc[:], sp_p[:])
                    sp_ps.append(spc)
                # wsp = psum(sp) * w ; link = SUM^T @ wsp
                for mc in range(4):
                    nc.vector.tensor_scalar(out=wsp[mc][:], in0=sp_ps[mc][:],
                                            scalar1=wsb[mc][:], scalar2=None,
                                            op0=ALU.mult)
                link_p = ps1a.tile([L, EMB], F32, tag="link", name="link")
                for kc in range(4):
                    nc.tensor.matmul(link_p[:], sumT[kc][:], wsp[kc][:],
                                     start=(kc == 0), stop=(kc == 3))
                nc.scalar.activation(link_b[0:L, 0:EMB], link_p[:], AF.Copy)
                fire("wstp11")


        if stages >= 2:
          # ================= stage 2: RGCN (4 layers, y col-half pipelined) ==
          ecT = [pwork.tile([128, E], F32R, tag=f"ecT{i}", name=f"ecT{i}")
                 for i in range(4)]
          with tc.tile_pool(name="prg", bufs=2) as prg, \
               tc.tile_pool(name="psr", bufs=1, space="PSUM") as psr:
              h = None
              UW = (NREL + 1) * NN
              for layer in range(NLAYERS):
                  kcs = _KC0 if layer == 0 else _KC1
                  nk = len(kcs)
                  wstp = wstp_t[layer]
                  nrel_l = 2 if layer == 3 else NREL + 1
                  uw_l = 2 * E if layer == 3 else UW
                  wst_t = [wstp[:, (r * nk + si) * EMB:(r * nk + si + 1) * EMB]
                           for r in range(nrel_l) for si in range(nk)]
                  # u = h^T @ A_allT per d-chunk. Layer 3 only needs entity
                  # output rows, which only rel-0 and the self-loop feed.
                  u_sb = []
                  for si, (s0, sl) in enumerate(kcs):
                      u_p = psr.tile([128, (NREL + 1) * NN], F32, tag="u_p",
                                     name="u_p", bufs=2)
                      if layer == 0:
                          for pi, (pt, rows) in enumerate(h0p):
                              nc.tensor.matmul(u_p[0:sl, 0:UW],
                                               pt[0:rows, s0:s0 + sl],
                                               aallTb[0:rows,
                                                      (pi + 1) * UW:
                                                      (pi + 2) * UW],
                                               start=(pi == 0), stop=(pi == 2))
                      elif layer == 3:
                          nc.tensor.matmul(u_p[0:sl, 0:2 * E],
                                           h[0:NN, s0:s0 + sl],
                                           aallTb[0:NN, 4 * UW:4 * UW + 2 * E],
                                           start=True, stop=True)
                      else:
                          nc.tensor.matmul(u_p[0:sl, 0:UW],
                                           h[0:NN, s0:s0 + sl],
                                           aallTb[0:NN, 0:UW],
                                           start=True, stop=True)
                      u = prg.tile([128, (NREL + 1) * NN], BF16, tag=f"u{si}",
                                   name=f"u{si}")
                      if si % 2 == 0:
                          nc.scalar.copy(u[0:sl, 0:uw_l], u_p[0:sl, 0:uw_l])
                      else:
                          nc.vector.tensor_copy(out=u[0:sl, 0:uw_l],
                                                in_=u_p[0:sl, 0:uw_l])
                      u_sb.append(u)
                  # y = sum_r (u_r)^T @ Wst_r, in two column halves so the
                  # relu of half 0 overlaps the matmuls of half 1
                  if layer < 2:
                      fire(f"wstp{layer + 2}0", f"wstp{layer + 2}1")
                  elif layer == 2:
                      fire("sew")
                  # deferred e_ctx chain, one piece per inter-layer window
                  if layer == 0:
                      for kc in range(6):
                          ztp = psr.tile([128, E], F32, tag="tp22", name="ztp",
                                         bufs=2)
                          nc.tensor.transpose(ztp[:],
                                              zt_sb[:, kc * 128:(kc + 1) * 128],
                                              ident[0:E, 0:E])
                          if kc % 2 == 0:
                              nc.scalar.copy(z_sb[kc][:], ztp[:])
                          else:
                              nc.vector.tensor_copy(out=z_sb[kc][:], in_=ztp[:])
                  elif layer == 1:
                      ec2_p = psr.tile([E, EMB], F32, tag="ec2", name="ec2")
                      for kc in range(6):
                          nc.tensor.matmul(ec2_p[:], z_sb[kc][:], wtr[kc][:],
                                           start=(kc == 0), stop=False)
                      nc.tensor.matmul(ec2_p[:], easumT[:], brow[:],
                                       start=False, stop=True)
                      nc.scalar.copy(ec2_sb[:], ec2_p[:])
                  elif layer == 2:
                      for mc in range(4):
                          ecp = psr.tile([128, E], F32, tag="tp22", name="ecp",
                                         bufs=2)
                          nc.tensor.transpose(ecp[:],
                                              ec2_sb[:, mc * 128:(mc + 1) * 128],
                                              ident[0:E, 0:E])
                          if mc % 2 == 0:
                              nc.scalar.copy(ectxT_sb[mc][:], ecp[:])
                          else:
                              nc.vector.tensor_copy(out=ectxT_sb[mc][:],
                                                    in_=ecp[:])
                  nrows = E if layer == 3 else NN
                  rw = E if layer == 3 else NN
                  hn = prg.tile([NN, EMB], BF16, tag="h_next", name="h_next")
                  for yh in range(2):
                      y_p = psr.tile([NN, 256], F32, tag=f"y_p{yh}",
                                     name=f"y_p{yh}")
                      n_mm = nrel_l * nk
                      k_mm = 0
                      for si, (s0, sl) in enumerate(kcs):
                          for r in range(nrel_l):
                              nc.tensor.matmul(
                                  y_p[0:nrows, :],
                                  u_sb[si][0:sl, r * rw:r * rw + nrows],
                                  wst_t[r * nk + si][0:sl,
                                                     yh * 256:(yh + 1) * 256],
                                  start=(k_mm == 0), stop=(k_mm == n_mm - 1))
                              k_mm += 1
                      nc.scalar.activation(hn[0:nrows, yh * 256:(yh + 1) * 256],
                                           y_p[0:nrows, :], AF.Relu)
                  h = hn

              # entity_struT + e_ctxT -> ecT
              for mc in range(4):
                  tp = psr.tile([128, E], F32, tag="tp22", name="est", bufs=2)
                  nc.tensor.matmul(tp[:], h[0:E, mc * 128:(mc + 1) * 128],
                                   identb[0:E, 0:E], start=True, stop=True)
                  nc.vector.tensor_tensor(out=ecT[mc][:], in0=tp[:],
                                          in1=ectxT_sb[mc][:], op=ALU.add)

        prgw_cm.__exit__(None, None, None)

        if stages >= 3:
          # ================= stage 3: fmap + SE =================
          fmap = [pwork.tile([128, EE], F32R, tag=f"fmap{i}", name=f"fmap{i}")
                  for i in range(4)]
          pooled = [pwork.tile([128, 1], F32R, tag=f"pool{i}", name=f"pool{i}")
                    for i in range(4)]
          fusedp = [pwork.tile([128, PADW], BF16, tag=f"fusedp{i}",
                               name=f"fusedp{i}") for i in range(4)]
          for mc in range(4):
              o6v = fmap[mc][:].rearrange("p (i j) -> p i j", i=E)
              in0 = ecT[mc][:].rearrange("p (i j) -> p i j", j=1) \
                  .to_broadcast([128, E, E])
              in1 = ecT[mc][:].rearrange("p (o j) -> p o j", o=1) \
                  .to_broadcast([128, E, E])
              nc.vector.tensor_tensor(out=o6v, in0=in0, in1=in1, op=ALU.mult)
              rs = pwork.tile([128, 1], F32, tag=f"rs{mc}", name=f"rs{mc}")
              nc.vector.tensor_reduce(rs[:], ecT[mc][:], mybir.AxisListType.X,
                                      ALU.add)
              nc.scalar.activation(pooled[mc][:], rs[:], AF.Square, scale=1.0 / E)

          with tc.tile_pool(name="pse", bufs=1, space="PSUM") as pse:
              # c-path first (tiny serial chain, hides under fmap/s1p);
              # seb2 is folded into the fcb2 host constant.
              c1_sb = [pwork.tile([128, 1], F32R, tag=f"c1_{i}", name=f"c1_{i}")
                       for i in range(2)]
              for oc in range(2):
                  c1_p = pse.tile([128, 1], F32, tag="cp", name="c1p", bufs=2)
                  for mc in range(4):
                      nc.tensor.matmul(c1_p[:],
                                       sew["fcw1T"][mc][:, oc * 128:(oc + 1) * 128]
                                       .bitcast(F32),
                                       pooled[mc][:].bitcast(F32),
                                       start=(mc == 0), stop=(mc == 3))
                  nc.scalar.activation(c1_sb[oc][:], c1_p[:], AF.Relu,
                                       bias=sev["fcb1"][oc][:],
                                       scale=sev["fcs1"][oc][:])
              cbb = [pwork.tile([128, 1], F32, tag=f"cbb{i}", name=f"cbb{i}")
                     for i in range(4)]
              # s-path; s2's first half starts as soon as s1_sb[0] is ready
              s1_sb = [pwork.tile([128, EE], F32R, tag=f"s1_{i}", name=f"s1_{i}")
                       for i in range(2)]
              for oc in range(2):
                  s1_p = pse.tile([128, EE], F32, tag="s1p", name="s1p", bufs=2)
                  for mc in range(4):
                      nc.tensor.matmul(s1_p[:],
                                       sew["fsw1T"][mc][:, oc * 128:(oc + 1) * 128],
                                       fmap[mc][:], start=(mc == 0), stop=(mc == 3))
                  nc.scalar.activation(s1_sb[oc][:], s1_p[:], AF.Relu,
                                       bias=sev["seb1"][oc][:],
                                       scale=sev["ses1"][oc][:])
              for mc in range(4):
                  c2_p = pse.tile([128, 1], F32, tag="cp", name="c2p", bufs=2)
                  for kc in range(2):
                      nc.tensor.matmul(c2_p[:],
                                       sew["fcw2T"][kc][:, mc * 128:(mc + 1) * 128]
                                       .bitcast(F32),
                                       c1_sb[kc][:].bitcast(F32),
                                       start=(kc == 0), stop=(kc == 1))
                  nc.scalar.activation(cbb[mc][:], c2_p[:], AF.Identity,
                                       bias=sev["fcb2"][mc][:],
                                       scale=sev["fcs2"][mc][:])
              for mc in range(4):
                  nc.vector.memset(fusedp[mc][:], 0.0)
              s2_ps = [pse.tile([128, EE], F32, tag="s2p", name=f"s2p{mc}",
                                bufs=3) for mc in range(4)]
              for mc in range(4):
                  nc.tensor.matmul(s2_ps[mc][:],
                                   sew["fsw2T"][0][:, mc * 128:(mc + 1) * 128],
                                   s1_sb[0][:], start=True, stop=False)
              for mc in range(4):
                  nc.tensor.matmul(s2_ps[mc][:],
                                   sew["fsw2T"][1][:, mc * 128:(mc + 1) * 128],
                                   s1_sb[1][:], start=False, stop=True)
                  sig = pwork.tile([128, EE], F32, tag="sig", name="sig", bufs=2)
                  nc.scalar.activation(sig[:], s2_ps[mc][:], AF.Sigmoid,
                                       bias=cbb[mc][:], scale=sev["ses2"][mc][:])
                  outv = fusedp[mc][:].rearrange("p (i j) -> p i j", j=26)[:, 2:24,
                                                                          2:24]
                  nc.vector.tensor_tensor(
                      out=outv,
                      in0=fmap[mc][:].rearrange("p (i j) -> p i j", i=E),
                      in1=sig[:].rearrange("p (i j) -> p i j", i=E),
                      op=ALU.mult)

        if stages >= 4:
          # ================= stage 4: conv stack =================
          def tap_rows(padt, tap, r0, nr):
              dy, dx = tap // 5, tap % 5
              return padt[:].rearrange("p (i j) -> p i j", j=26)[
                  :, dy + r0:dy + r0 + nr, dx:dx + 22]

          def tap_view(padt, tap):
              return tap_rows(padt, tap, 0, 22)

          with tc.tile_pool(name="pcw", bufs=1) as pcw, \
               tc.tile_pool(name="psc", bufs=1, space="PSUM") as psc:
              w1, w2, w3 = [], [], []
              for kc in range(4):
                  t = pcw.tile([128, 25 * 128], BF16, tag=f"w1_{kc}",
                               name=f"w1_{kc}")
                  nc.sync.dma_start(t[:], w1sb_d[kc])
                  w1.append(t)
              for kc in range(2):
                  t = pcw.tile([128, 25 * 128], BF16, tag=f"w2_{kc}",
                               name=f"w2_{kc}")
                  nc.sync.dma_start(t[:], w2sb_d[kc])
                  w2.append(t)
              for kc in range(2):
                  t = pcw.tile([128, 25 * 256], BF16, tag=f"w3_{kc}",
                               name=f"w3_{kc}")
                  nc.sync.dma_start(t[:], w3sb_d[kc])
                  w3.append(t)
              mtop = cf("mtop")
              mbot = cf("mbot")

              def interior_rows(t_, r0, nr):
                  return t_[:].rearrange("p (i j) -> p i j", j=26)[
                      :, 2 + r0:2 + r0 + nr, 2:24]

              def gather_combine(src_pad, dst_pad, gg, rh, nm, rows=None):
                  """relu'd rows rh of src -> allgather -> mask-combine into
                  dst rows rh, issued right after the producing relu."""
                  r0, nr = rows if rows is not None else \
                      ((0, 13) if rh == 0 else (13, 9))
                  rb = pdram.tile([128, nr * 22], BF16, tag=f"{nm}b{rh}",
                                  name=f"{nm}b{rh}")
                  nc.sync.dma_start(rb[:], interior_rows(src_pad, r0, nr))
                  if solo:
                      nc.sync.dma_start(gg[0:128, :], rb[:])
                      nc.sync.dma_start(gg[128:256, :], rb[:])
                  else:
                      nc.gpsimd.collective_compute(
                          "AllGather", ALU.bypass, replica_groups=groups,
                          ins=[rb[:].opt()], outs=[gg[:].opt()])
                  gt = pcw.tile([128, nr * 22], BF16, tag=f"{nm}gt{rh}",
                                name=f"{nm}gt{rh}")
                  gb = pcw.tile([128, nr * 22], BF16, tag=f"{nm}gb{rh}",
                                name=f"{nm}gb{rh}")
                  nc.sync.dma_start(gt[:], gg[0:128, :])
                  nc.sync.dma_start(gb[:], gg[128:256, :])
                  tmp = pcw.tile([128, nr * 22], F32, tag=f"{nm}tmp{rh}",
                                 name=f"{nm}tmp{rh}")
                  nc.vector.tensor_scalar(out=tmp[:], in0=gb[:],
                                          scalar1=mbot[:], scalar2=None,
                                          op0=ALU.mult)
                  nc.vector.scalar_tensor_tensor(
                      out=interior_rows(dst_pad, r0, nr), in0=gt[:],
                      scalar=mtop[:], in1=tmp[:], op0=ALU.mult, op1=ALU.add)

              # row halves 13/9: conv_{n+1}'s own-input taps for out rows
              # 0..10 need rows <=12, i.e. only the first half -> no wait on
              # the second half's relu.
              ROWS = ((0, 13), (13, 9))

              # conv1 (my half of 256 out channels), in two row halves; each
              # half's AllGather + readback starts as soon as the half is done
              r1g = [pdram.tile([256, nr * 22], BF16, tag=f"r1g{i}",
                                name=f"r1g{i}")
                     for i, (r0, nr) in enumerate(ROWS)]
              for rh, (r0, nr) in enumerate(ROWS):
                  r1_p = psc.tile([128, nr * 22], F32, tag="convp",
                                  name="convp", bufs=4)
                  first = True
                  for kc in range(4):
                      for tap in range(25):
                          nc.tensor.matmul(r1_p[:],
                                           w1[kc][:, tap * 128:(tap + 1) * 128],
                                           tap_rows(fusedp[kc], tap, r0, nr),
                                           start=first,
                                           stop=(kc == 3 and tap == 24))
                          first = False
                  nc.scalar.activation(interior_rows(own1, r0, nr), r1_p[:],
                                       AF.Relu, bias=b1h[:])
                  gather_combine(own1, oth1, r1g[rh], rh, "r1")

              # conv2: own-input taps for both row-halves first (no gather
              # dependency), then other-input taps; output in row halves.
              # conv2 out rows (0-8, 9-21): the first half's own AND other
              # taps read only rows <=12 of r1, i.e. conv1-half0 + gather#0 —
              # no wait on conv1-half1's gather.
              ROWS2 = ((0, 9), (9, 13))
              r2g = [pdram.tile([256, nr * 22], BF16, tag=f"r2g{i}",
                                name=f"r2g{i}")
                     for i, (r0, nr) in enumerate(ROWS2)]
              r2_ps = [psc.tile([128, nr * 22], F32, tag=f"convp2_{i}",
                                name=f"convp2_{i}")
                       for i, (r0, nr) in enumerate(ROWS2)]
              for rh, (r0, nr) in enumerate(ROWS2):
                  for tap in range(25):
                      nc.tensor.matmul(r2_ps[rh][:],
                                       w2[0][:, tap * 128:(tap + 1) * 128],
                                       tap_rows(own1, tap, r0, nr),
                                       start=(tap == 0), stop=False)
                  for tap in range(25):
                      nc.tensor.matmul(r2_ps[rh][:],
                                       w2[1][:, tap * 128:(tap + 1) * 128],
                                       tap_rows(oth1, tap, r0, nr),
                                       start=False, stop=(tap == 24))
                  nc.scalar.activation(interior_rows(own2, r0, nr), r2_ps[rh][:],
                                       AF.Relu, bias=b2h[:])
                  gather_combine(own2, oth2, r2g[rh], rh, "r2", (r0, nr))

              # conv3 (my 256 of 512 out channels): own-input taps for both
              # out chunks first, then other-input taps per (oc, row-half)
              # with relu+store pipelined per row half (short tail).
              r3_ps = [psc.tile([128, ROWS[rh][1] * 22], F32, tag="convp",
                                name=f"convp3_{oc}{rh}", bufs=4)
                       for oc in range(2) for rh in range(2)]
              for oc in range(2):
                  for rh, (r0, nr) in enumerate(ROWS):
                      for tap in range(25):
                          nc.tensor.matmul(
                              r3_ps[oc * 2 + rh][:],
                              w3[0][:, tap * 256 + oc * 128:
                                    tap * 256 + (oc + 1) * 128],
                              tap_rows(own2, tap, r0, nr),
                              start=(tap == 0), stop=False)
              for oc in range(2):
                  for rh, (r0, nr) in enumerate(ROWS):
                      for tap in range(25):
                          nc.tensor.matmul(
                              r3_ps[oc * 2 + rh][:],
                              w3[1][:, tap * 256 + oc * 128:
                                    tap * 256 + (oc + 1) * 128],
                              tap_rows(oth2, tap, r0, nr),
                              start=False, stop=(tap == 24))
                      o_sb = pcw.tile([128, nr * 22], F32, tag="osb",
                                      name="osb", bufs=4)
                      nc.scalar.activation(o_sb[:], r3_ps[oc * 2 + rh][:],
                                           AF.Relu, bias=b3h[oc][:])
                      nc.sync.dma_start(
                          out_d[oc * 128:(oc + 1) * 128,
                                r0 * 22:(r0 + nr) * 22], o_sb[:])

    nc.compile()
    return nc


_NC_CACHE = None


def _get_program():
    global _NC_CACHE
    if _NC_CACHE is None:
        _NC_CACHE = build_program()
    return _NC_CACHE


def _prep_shared(w):
    """Packed weights/constants identical on every core."""
    ADJ = _build_adj()
    out = {}
    constr = np.zeros((128, _CR), np.float32)

    def put(nm, arr):
        c0, cols = _LAY_R[nm]
        r, cc = arr.shape
        constr[0:r, c0:c0 + cc] = arr
    g2T = np.zeros((EM, E), np.float32)
    for e in range(E):
        g2T[e * M:(e + 1) * M, e] = 1.0
    put("g2T", g2T)
    sumT = np.kron(np.eye(L, dtype=np.float32), np.ones((SPAN, 1), np.float32))
    for kc in range(4):
        put(f"sumT{kc}", sumT[kc * 128:(kc + 1) * 128])
    for nm, arr, nch in (("fsw1T", w['fs_w1'].T, 4), ("fcw1T", w['fc_w1'].T, 4),
                         ("fsw2T", w['fs_w2'].T, 2), ("fcw2T", w['fc_w2'].T, 2)):
        for kc in range(nch):
            put(f"{nm}{kc}", np.ascontiguousarray(arr[kc * 128:(kc + 1) * 128]))
    out['constr'] = constr

    constw = np.zeros((128, _CW), np.float32)
    wt = w['W_trans']
    for kc in range(6):
        constw[:, kc * EMB:(kc + 1) * EMB] = wt[kc * 128:(kc + 1) * 128]
    constw[0, 6 * EMB:7 * EMB] = w['b_trans']
    out['constw'] = constw.astype(ml_dtypes.bfloat16)

    gTb = np.zeros((128, 9 * E), np.float32)
    gT = np.zeros((EMH, E), np.float32)
    for e in range(E):
        gT[e * M * H:(e + 1) * M * H, e] = 1.0 / (M * H)
    for kc in range(9):
        r = min(128, EMH - kc * 128)
        gTb[0:r, kc * E:(kc + 1) * E] = gT[kc * 128:kc * 128 + r]
    out['gTb'] = gTb.astype(ml_dtypes.bfloat16)
    aall = np.concatenate(
        [ADJ[r].T for r in range(NREL)] + [np.eye(NN, dtype=np.float32)],
        axis=1)
    UW = (NREL + 1) * NN
    aallp = np.zeros((128, 4 * UW + 2 * E), np.float32)
    aallp[0:NN, 0:UW] = aall
    aallp[0:EM, UW:2 * UW] = aall[E:E + EM]
    aallp[0:L, 2 * UW:3 * UW] = aall[E + EM:NN]
    aallp[0:E, 3 * UW:4 * UW] = aall[0:E]
    # layer-3 entity-only columns: [A0^T[:, :E] | I[:, :E]]
    aallp[0:NN, 4 * UW:4 * UW + E] = aall[:, 0:E]
    aallp[0:NN, 4 * UW + E:4 * UW + 2 * E] = aall[:, NREL * NN:NREL * NN + E]
    out['aallTb'] = aallp.astype(ml_dtypes.bfloat16)
    out['tfb'] = np.ascontiguousarray(
        w['type_embed'][_TYPES]).astype(ml_dtypes.bfloat16)
    out['identb'] = np.eye(128, dtype=np.float32).astype(ml_dtypes.bfloat16)

    constf = np.zeros((128, _CF), np.float32)

    def putf(nm, arr):
        c0, cols = _LAY_F[nm]
        constf[0:arr.shape[0], c0:c0 + 1] = arr.reshape(-1, 1)
    vecs = {"ses1": w['fs_g1'], "seb1": w['fs_b1'] * w['fs_g1'] + w['fs_be1'],
            "fcs1": w['fc_g1'], "fcb1": w['fc_b1'] * w['fc_g1'] + w['fc_be1'],
            "ses2": w['fs_g2'], "seb2": w['fs_b2'] * w['fs_g2'] + w['fs_be2'],
            "fcs2": w['fc_g2'],
            "fcb2": (w['fc_b2'] * w['fc_g2'] + w['fc_be2'] +
                     w['fs_b2'] * w['fs_g2'] + w['fs_be2'])}
    for nm, v in vecs.items():
        nch = 2 if v.shape[0] == INTER else 4
        for kc in range(nch):
            putf(f"{nm}{kc}", v[kc * 128:(kc + 1) * 128])
    out['constf_base'] = constf

    for layer in range(NLAYERS):
        din_l = D0 if layer == 0 else EMB
        kcs = _KC0 if layer == 0 else _KC1
        nk = len(kcs)
        Wst = w['rgcn_Wrel0'].reshape(NREL * D0, EMB) if layer == 0 else \
            w['rgcn_Wrel'][layer - 1].reshape(NREL * EMB, EMB)
        Wself = w['rgcn_Wself0'] if layer == 0 else w['rgcn_Wself'][layer - 1]
        if layer == 3:
            # entity rows only need rel-0 and the self-loop
            p = np.zeros((128, 2 * nk * EMB), np.float32)
            for si, (s0, sl) in enumerate(kcs):
                p[0:sl, si * EMB:(si + 1) * EMB] = Wst[s0:s0 + sl]
                p[0:sl, (nk + si) * EMB:(nk + si + 1) * EMB] = \
                    Wself[s0:s0 + sl]
        else:
            p = np.zeros((128, (NREL + 1) * nk * EMB), np.float32)
            for r in range(NREL):
                for si, (s0, sl) in enumerate(kcs):
                    p[0:sl, (r * nk + si) * EMB:(r * nk + si + 1) * EMB] = \
                        Wst[r * din_l + s0:r * din_l + s0 + sl]
            for si, (s0, sl) in enumerate(kcs):
                p[0:sl, (NREL * nk + si) * EMB:(NREL * nk + si + 1) * EMB] = \
                    Wself[s0:s0 + sl]
        out[f'wstp{layer}'] = p.astype(ml_dtypes.bfloat16)
    return out


def _prep_conv_half(w, half, constf_base):
    out = {}
    w1 = w['cr_w1'][half * 128:(half + 1) * 128]
    out['w1sb'] = np.ascontiguousarray(
        w1.transpose(1, 2, 3, 0).reshape(4, 128, 25 * 128)).astype(ml_dtypes.bfloat16)
    # conv2/conv3 weights in (own-input-half, other-input-half) chunk order
    w2 = w['cr_w2'][half * 128:(half + 1) * 128]
    w2p = w2.transpose(1, 2, 3, 0).reshape(2, 128, 25 * 128)
    order = [half, 1 - half]
    out['w2sb'] = np.ascontiguousarray(w2p[order]).astype(ml_dtypes.bfloat16)
    w3 = w['cr_w3'][half * 256:(half + 1) * 256]
    w3p = w3.transpose(1, 2, 3, 0).reshape(2, 128, 25 * 256)
    out['w3sb'] = np.ascontiguousarray(w3p[order]).astype(ml_dtypes.bfloat16)
    constf = constf_base.copy()

    def putf(nm, arr):
        c0, cols = _LAY_F[nm]
        constf[0:arr.shape[0], c0:c0 + 1] = arr.reshape(-1, 1)
    putf("b1h", w['cr_b1'][half * 128:(half + 1) * 128])
    putf("b2h", w['cr_b2'][half * 128:(half + 1) * 128])
    putf("b3h0", w['cr_b3'][half * 256:half * 256 + 128])
    putf("b3h1", w['cr_b3'][half * 256 + 128:half * 256 + 256])
    putf("mtop", np.full(128, float(half), np.float32))
    putf("mbot", np.full(128, float(1 - half), np.float32))
    c0, cols = _LAY_F["identf"]
    constf[:, c0:c0 + 128] = np.eye(128, dtype=np.float32)
    out['constf'] = constf
    return out


def _prep_doc(x, att, mi, ls):
    out = {}
    mif = mi.reshape(EM)
    attm = np.ascontiguousarray(
        att[:, mif, :].transpose(1, 0, 2).reshape(EMH, C))
    amp = np.zeros((128, 9 * C), np.float32)
    for kc in range(9):
        r = min(128, EMH - kc * 128)
        amp[0:r, kc * C:kc * C + C] = attm[kc * 128:kc * 128 + r]
    out['amp'] = amp.astype(ml_dtypes.bfloat16)
    idx = ls[:, None] + np.arange(SPAN)
    idxf = idx.reshape(LS)
    rows = att[:, idxf, :].reshape(H, L, SPAN, C)
    blocks = np.take_along_axis(rows, idx[None, :, None, :], axis=3)
    attl = blocks.transpose(0, 2, 1, 3).reshape(HS, LS)
    xmT = x[mif].T
    xspT = x[idxf].T
    actr = np.zeros((128, _CA), np.float32)

    def put(nm, arr):
        c0, cols = _LAY_A[nm]
        actr[0:arr.shape[0], c0:c0 + arr.shape[1]] = arr
    for kc in range(6):
        put(f"xmT{kc}", xmT[kc * 128:(kc + 1) * 128])
        put(f"xspT{kc}", xspT[kc * 128:(kc + 1) * 128])
    for kc in range(3):
        put(f"attl{kc}", attl[kc * 128:(kc + 1) * 128])
    out['actr'] = actr.astype(ml_dtypes.bfloat16)
    xpk = np.zeros((128, 8 * HID), np.float32)
    for kc in range(8):
        xpk[:, kc * HID:(kc + 1) * HID] = x[kc * 128:(kc + 1) * 128]
    out['xp'] = xpk.astype(ml_dtypes.bfloat16)
    return out


def build_in_maps(inputs):
    w = {}
    for k, v in inputs.items():
        a = np.asarray(v)
        w[k] = a if a.dtype in (np.int32, np.int64) else \
            np.asarray(a, np.float32)
    shared = _prep_shared(w)
    constf_base = shared.pop('constf_base')
    halves = [_prep_conv_half(w, h, constf_base) for h in range(2)]
    seq = np.asarray(inputs['sequence_output'], np.float32)
    att = np.asarray(inputs['attention'], np.float32)
    mi = np.asarray(inputs['mention_idx']).astype(np.int64)
    ls = np.asarray(inputs['link_start']).astype(np.int64)
    docs = [_prep_doc(seq[n], att[n], mi[n], ls[n]) for n in range(NB)]
    in_maps = []
    for core in range(N_CORES):
        n, half = core // 2, core % 2
        m = dict(shared)
        m.update(halves[half])
        m.update(docs[n])
        in_maps.append({k: (np.ascontiguousarray(v) if v.dtype == ml_dtypes.bfloat16
                            else np.ascontiguousarray(v, np.float32))
                        for k, v in m.items()})
    return in_maps


def kernel(**inputs):
    nc = _get_program()
    in_maps = build_in_maps(inputs)
    res = run_bass_kernel_spmd(nc, in_maps, list(range(N_CORES)))
    out = np.zeros((NB, EMB, E, E), np.float32)
    for core in range(N_CORES):
        n, half = core // 2, core % 2
        out[n, half * 256:(half + 1) * 256] = \
            res.results[core]["out"].reshape(256, E, E)
    return out


# revision 28
# speedup vs baseline: 1.0033x; 1.0033x over previous
"""Trainium2 Bass kernel for nn_DocREModel (DocRE: gather -> RGCN -> SE -> 5x5 convs).

Sharding: 4 documents x 2 cores each. Each pair replicates the cheap upstream
(mention/link/ea gathers -> RGCN -> fmap/SE) and splits the dominant 5x5 conv
stack by output channels, with intra-pair AllGathers; output halves are
assembled on host. All index-driven gathers happen on host (pure data
movement; one SPMD program serves all 8 cores), all dense math on device.

Perf notes (v3):
- DMA is a serialized ~360B/ns resource in the cost model; all input loads
  go on ONE queue (sync/SP) in exact consumption order, in <=~800KB pieces,
  so arrival order is deterministic and matches the compute schedule.
- The PE p-state ramp resets on >~2us idle gaps. Tiny "touch" matmuls that
  read one column of each arriving DMA piece pulse the PE at piece cadence,
  holding the fast clock through DMA-bound phases.
- bf16 for W_trans, gathered activations, RGCN + conv weights (f32 PSUM
  accumulation everywhere); f32r for the remaining f32 path.
- Convs are 25 shift-tap matmuls over zero-padded 26x26 images via strided
  APs. conv1/conv2 outputs are produced in two 11-row halves so each half's
  AllGather (+ readback + mask-combine) overlaps the next half's matmuls;
  conv3 output is relu'd + stored per (out-chunk, row-half) to shrink the
  kernel tail. RGCN folds the self-loop in as a 4th identity relation and
  pipelines each layer by y column halves.
"""

import numpy as np
import ml_dtypes

import concourse.bacc as bacc
import concourse.tile as tile
from concourse import mybir
from concourse.bass_utils import run_bass_kernel_spmd

F32 = mybir.dt.float32
F32R = mybir.dt.float32r
BF16 = mybir.dt.bfloat16
AF = mybir.ActivationFunctionType
ALU = mybir.AluOpType

NB, H, C, HID, EMB = 4, 12, 1024, 768, 512
E, M, L, SPAN = 22, 4, 16, 32
TD, INTER = 20, 256
NN = E + E * M + L
NREL, NLAYERS = 3, 4
EM, EMH, HS, LS = E * M, E * M * H, H * SPAN, L * SPAN
D0 = EMB + TD           # 532
EE = E * E              # 484
PADW = 26 * 26          # 676 padded 26x26 image
N_CORES = 8


def _build_adj():
    A = np.zeros((NREL, NN, NN), np.float32)
    for e in range(E):
        for m in range(M):
            mi = E + e * M + m
            A[0, e, mi] = A[0, mi, e] = 1.0
            for m2 in range(M):
                if m2 != m:
                    A[1, mi, E + e * M + m2] = 1.0
            li = E + E * M + ((e * M + m) % L)
            A[2, mi, li] = A[2, li, mi] = 1.0
    A = A / (A.sum(-1, keepdims=True) + 1e-5)
    return A


_TYPES = np.concatenate([np.zeros(E, np.int32), np.ones(EM, np.int32),
                         np.full(L, 2, np.int32)])

_KC0 = [(0, 128), (128, 128), (256, 128), (384, 128), (512, 20)]   # 532 rows
_KC1 = [(0, 128), (128, 128), (256, 128), (384, 128)]              # 512 rows


def _const_layout():
    """f32r constants [128, CR]: stage-1 smalls + SE weights."""
    lay = {}
    c = 0

    def add(nm, cols):
        nonlocal c
        lay[nm] = (c, cols)
        c += cols
    add("g2T", E)
    for kc in range(4):
        add(f"sumT{kc}", L)
    for kc in range(4):
        add(f"fsw1T{kc}", INTER)
    for kc in range(4):
        add(f"fcw1T{kc}", INTER)
    for kc in range(2):
        add(f"fsw2T{kc}", EMB)
    for kc in range(2):
        add(f"fcw2T{kc}", EMB)
    return lay, c


def _constf_layout():
    lay = {}
    c = 0

    def add(nm, cols):
        nonlocal c
        lay[nm] = (c, cols)
        c += cols
    for nm, nch in (("ses1", 2), ("seb1", 2), ("fcs1", 2), ("fcb1", 2),
                    ("ses2", 4), ("seb2", 4), ("fcs2", 4), ("fcb2", 4)):
        for kc in range(nch):
            add(f"{nm}{kc}", 1)
    add("b1h", 1)
    add("b2h", 1)
    add("b3h0", 1)
    add("b3h1", 1)
    add("mtop", 1)
    add("mbot", 1)
    add("identf", 128)
    return lay, c


def _actr_layout():
    """bf16 gathered activations [128, CA]."""
    lay = {}
    c = 0

    def add(nm, cols):
        nonlocal c
        lay[nm] = (c, cols)
        c += cols
    for kc in range(6):
        add(f"xmT{kc}", EM)
    for kc in range(3):
        add(f"attl{kc}", LS)
    for kc in range(6):
        add(f"xspT{kc}", LS)
    return lay, c


_LAY_R, _CR = _const_layout()
_LAY_F, _CF = _constf_layout()
_LAY_A, _CA = _actr_layout()

_CRB = _LAY_R["fsw1T0"][0]          # stage-1 smalls | SE weights split
_CAA = _LAY_A["xspT0"][0]           # xmT+attl | xspT split
_CW = 7 * EMB                       # constrw (bf16): wtr chunks + brow block


def build_program(solo=False, stages=4):
    nc = bacc.Bacc("TRN2", target_bir_lowering=False, debug=False)

    def din(name, shape, dt=F32R):
        return nc.dram_tensor(name, list(shape), dt, kind="ExternalInput").ap()

    constr_d = din("constr", [128, _CR])
    constf_d = din("constf", [128, _CF], F32)
    constw_d = din("constw", [128, _CW], BF16)
    actr_d = din("actr", [128, _CA], BF16)
    xp_d = din("xp", [128, 8 * HID], BF16)
    amp_d = din("amp", [128, 9 * C], BF16)
    gTb_d = din("gTb", [128, 9 * E], BF16)
    wstp_d = [din("wstp0", [128, 20 * EMB], BF16)] + \
             [din(f"wstp{i}", [128, 16 * EMB], BF16) for i in (1, 2)] + \
             [din("wstp3", [128, 8 * EMB], BF16)]
    w1sb_d = din("w1sb", [4, 128, 25 * 128], BF16)
    w2sb_d = din("w2sb", [2, 128, 25 * 128], BF16)
    w3sb_d = din("w3sb", [2, 128, 25 * 256], BF16)
    aallTb_d = din("aallTb", [128, 4 * (NREL + 1) * NN + 2 * E], BF16)
    tfb_d = din("tfb", [NN, TD], BF16)
    identb_d = din("identb", [128, 128], BF16)

    out_d = nc.dram_tensor("out", [256, EE], F32, kind="ExternalOutput").ap()

    groups = [[0, 1], [2, 3], [4, 5], [6, 7]]

    with tile.TileContext(nc) as tc:
      with tc.tile_pool(name="pconst", bufs=1) as pconst, \
           tc.tile_pool(name="pwork", bufs=1) as pwork, \
           tc.tile_pool(name="pwarm", bufs=1, space="PSUM") as pwarm, \
           tc.tile_pool(name="pdram", bufs=1, space="DRAM") as pdram:
        prgw_cm = tc.tile_pool(name="prgw", bufs=1)
        prgw = prgw_cm.__enter__()

        constr = pconst.tile([128, _CR], F32R)
        constf = pconst.tile([128, _CF], F32)
        constw = pconst.tile([128, _CW], BF16)
        identb = pconst.tile([128, 128], BF16)
        aallTb = pconst.tile([128, 4 * (NREL + 1) * NN + 2 * E], BF16)
        onesb = pconst.tile([128, 128], BF16)
        nc.vector.memset(onesb[:], 1.0)
        # scratch for the sigmoid-table swap dummy (see ep below)
        actwarm = pconst.tile([1, 1], F32)

        def cr(nm, rows=128):
            c0, cols = _LAY_R[nm]
            return constr[0:rows, c0:c0 + cols]

        def cf(nm, rows=128):
            c0, cols = _LAY_F[nm]
            return constf[0:rows, c0:c0 + cols]

        wtr = [constw[:, kc * EMB:(kc + 1) * EMB] for kc in range(6)]
        brow = constw[0:1, 6 * EMB:7 * EMB]
        onescol = onesb[:, 0:1]
        onesrow = onesb[0:1, :]
        g2T = cr("g2T", rows=EM)
        sumT = [cr(f"sumT{kc}") for kc in range(4)]
        sew = {nm: [cr(f"{nm}{kc}") for kc in range(n)]
               for nm, n in (("fsw1T", 4), ("fcw1T", 4), ("fsw2T", 2),
                             ("fcw2T", 2))}
        sev = {nm: [cf(f"{nm}{kc}") for kc in range(n)]
               for nm, n in (("ses1", 2), ("seb1", 2), ("fcs1", 2), ("fcb1", 2),
                             ("ses2", 4), ("seb2", 4), ("fcs2", 4),
                             ("fcb2", 4))}
        b1h = cf("b1h")
        b2h = cf("b2h")
        b3h = [cf("b3h0"), cf("b3h1")]
        ident = cf("identf")

        # conv pad images: memset early while the DVE is idle
        fusedp = [pwork.tile([128, PADW], BF16, tag=f"fusedp{i}",
                             name=f"fusedp{i}") for i in range(4)]
        for t_ in fusedp:
            nc.vector.memset(t_[:], 0.0)
        own1 = pwork.tile([128, PADW], BF16)
        oth1 = pwork.tile([128, PADW], BF16)
        own2 = pwork.tile([128, PADW], BF16)
        oth2 = pwork.tile([128, PADW], BF16)
        for t_ in (own1, oth1, own2, oth2):
            nc.vector.memset(t_[:], 0.0)

        # persistent intermediates: h0 kept as three partition-0 node-type
        # parts (entities / mentions / links); RGCN layer 0 contracts over
        # the parts with a 3-part adjacency tensor.
        eln_b = pwork.tile([E, D0], BF16)
        mrep_b = pwork.tile([EM, D0], BF16)
        link_b = pwork.tile([L, D0], BF16)
        h0p = [(mrep_b, EM), (link_b, L), (eln_b, E)]
        ectxT_sb = [pwork.tile([128, E], F32, tag=f"ectxT{i}", name=f"ectxT{i}")
                    for i in range(4)]
        z_sb = [pwork.tile([128, E], BF16, tag=f"z{i}", name=f"z{i}")
                for i in range(6)]
        easumT = pwork.tile([1, E], BF16)
        zt_sb = pwork.tile([E, HID], F32)
        ec2_sb = pwork.tile([E, EMB], F32)

        # ---------------- ordered DMA supply script (sync/SP queue) ---------
        # Exact consumption order, <=~800KB pieces. "touch" matmuls (fired at
        # chosen program points) read one column of an arriving DMA piece and
        # pulse the PE, so the p-state ramp never resets during DMA-bound
        # stretches.
        warm = pwarm.tile([1, 1], F32)
        tch = {}

        def fire(*names):
            for nm in names:
                nc.tensor.matmul(warm[:], tch[nm], tch[nm],
                                 start=True, stop=True)

        gTbt = prgw.tile([128, 9 * E], BF16)
        nc.scalar.dma_start(gTbt[:], gTb_d[:])
        amp = prgw.tile([128, 9 * C], BF16)
        nc.sync.dma_start(amp[:, 0:C], amp_d[:, 0:C])
        nc.sync.dma_start(constf[:], constf_d[:])
        for (p0, pn) in ((1, 2), (3, 2), (5, 2), (7, 2)):
            nc.sync.dma_start(amp[:, p0 * C:(p0 + pn) * C],
                              amp_d[:, p0 * C:(p0 + pn) * C])
        xp = prgw.tile([128, 8 * HID], BF16)
        for i, p0 in enumerate(range(0, 8, 2)):
            nc.sync.dma_start(xp[:, p0 * HID:(p0 + 2) * HID],
                              xp_d[:, p0 * HID:(p0 + 2) * HID])
            tch[f"xp{i}"] = xp[:, p0 * HID:p0 * HID + 1]
        for i, (p0, pn) in enumerate(((0, 3), (3, 4))):   # wtr + brow block
            nc.sync.dma_start(constw[:, p0 * EMB:(p0 + pn) * EMB],
                              constw_d[:, p0 * EMB:(p0 + pn) * EMB])
            tch[f"wtr{i}"] = constw[:, p0 * EMB:p0 * EMB + 1]
        nc.sync.dma_start(constr[:, 0:_CRB], constr_d[:, 0:_CRB])
        actr = prgw.tile([128, _CA], BF16)
        nc.sync.dma_start(actr[:, 0:_CAA], actr_d[:, 0:_CAA])
        tch["actr0"] = actr[:, 0:1]
        nc.sync.dma_start(actr[:, _CAA:_CA], actr_d[:, _CAA:_CA])
        tch["actr1"] = actr[:, _CAA:_CAA + 1]
        # RGCN weights
        wstp_t = [prgw.tile([128, 20 * EMB], BF16, tag="wstp0",
                            name="wstp0")] + \
                 [prgw.tile([128, 16 * EMB], BF16, tag=f"wstp{i}",
                            name=f"wstp{i}") for i in (1, 2)] + \
                 [prgw.tile([128, 8 * EMB], BF16, tag="wstp3", name="wstp3")]
        for layer in range(NLAYERS):
            ncols = (20 * EMB if layer == 0 else
                     16 * EMB if layer < 3 else 8 * EMB)
            hh = ncols // 2
            for i, (p0, pn) in enumerate(((0, hh), (hh, ncols - hh))):
                nc.sync.dma_start(wstp_t[layer][:, p0:p0 + pn],
                                  wstp_d[layer][:, p0:p0 + pn])
                tch[f"wstp{layer}{i}"] = wstp_t[layer][:, p0:p0 + 1]
        # SE weights
        nc.sync.dma_start(constr[:, _CRB:_CR], constr_d[:, _CRB:_CR])
        tch["sew"] = constr[:, _CRB:_CRB + 1].bitcast(F32)


        def ca(nm, rows=128):
            c0, cols = _LAY_A[nm]
            return actr[0:rows, c0:c0 + cols]

        xmT = [ca(f"xmT{kc}") for kc in range(6)]
        xspT = [ca(f"xspT{kc}") for kc in range(6)]
        attl = [ca(f"attl{kc}") for kc in range(3)]

        def fire_touches():
            for ap_sl in touches:
                nc.tensor.matmul(warm[:], ap_sl, ap_sl, start=True, stop=True)

        # ================= stage 1: gathered-row transforms =================
        with tc.tile_pool(name="pbig", bufs=1) as pbig:
            expm = pbig.tile([EM, EMB], F32R)
            sp_ps = []
            wsb = [pbig.tile([128, 1], F32, tag=f"wsb{i}", name=f"wsb{i}")
                   for i in range(4)]
            wsp = [pbig.tile([128, EMB], F32R, tag=f"wsp{i}", name=f"wsp{i}")
                   for i in range(4)]
            ea_sb = pbig.tile([E, C], F32R)
            eaT = [pbig.tile([128, E], BF16, tag=f"eaT{i}", name=f"eaT{i}")
                   for i in range(8)]


            with tc.tile_pool(name="ps1b", bufs=1, space="PSUM") as ps1b:
                # ea = G^T @ attm ; normalize rows
                ea_p0 = ps1b.tile([E, 512], F32, tag="ea0", name="ea0")
                ea_p1 = ps1b.tile([E, 512], F32, tag="ea1", name="ea1")
                for kc in range(9):
                    rows = 128 if kc < 8 else 32
                    at = amp[0:rows, kc * C:kc * C + C]
                    gt = gTbt[0:rows, kc * E:(kc + 1) * E]
                    nc.tensor.matmul(ea_p0[:], gt, at[:, 0:512],
                                     start=(kc == 0), stop=(kc == 8))
                    nc.tensor.matmul(ea_p1[:], gt, at[:, 512:1024],
                                     start=(kc == 0), stop=(kc == 8))
                fire("xp0", "xp1")
                r0 = pbig.tile([E, 1], F32)
                r1 = pbig.tile([E, 1], F32)
                nc.vector.tensor_reduce(r0[:], ea_p0[:], mybir.AxisListType.X,
                                        ALU.add)
                nc.vector.tensor_reduce(r1[:], ea_p1[:], mybir.AxisListType.X,
                                        ALU.add)
                rsum = pbig.tile([E, 1], F32)
                nc.vector.tensor_tensor(out=rsum[:], in0=r0[:], in1=r1[:],
                                        op=ALU.add)
                rsum2 = pbig.tile([E, 1], F32)
                nc.vector.tensor_scalar(out=rsum2[:], in0=rsum[:], scalar1=1e-5,
                                        scalar2=None, op0=ALU.add)
                rinv = pbig.tile([E, 1], F32)
                nc.vector.reciprocal(rinv[:], rsum2[:])
                # ea left unnormalized; rinv is applied as a row scale on the
                # zt copy below (zt is linear in ea), shortening the critical
                # chain ea -> eaT -> zt.
                nc.scalar.activation(ea_sb[:, 0:512], ea_p0[:], AF.Copy)
                nc.scalar.activation(ea_sb[:, 512:1024], ea_p1[:], AF.Copy)
                easum = pbig.tile([E, 1], F32)
                nc.vector.tensor_tensor(out=easum[:], in0=rsum[:], in1=rinv[:],
                                        op=ALU.mult)
                # small late-use constants; placed here so their HWDGE slots
                # don't delay the early amp pieces
                nc.scalar.dma_start(eln_b[0:E, EMB:D0], tfb_d[0:E, :])
                nc.scalar.dma_start(mrep_b[0:EM, EMB:D0], tfb_d[E:E + EM, :])
                nc.scalar.dma_start(link_b[0:L, EMB:D0], tfb_d[E + EM:NN, :])
                nc.scalar.dma_start(aallTb[:], aallTb_d[:])
                nc.scalar.dma_start(identb[:], identb_d[:])
                for kc in range(8):
                    if kc == 2:
                        fire("xp2")
                    elif kc == 4:
                        fire("xp3")
                    tp = ps1b.tile([128, E], F32, tag="eaTt", name="eaTt", bufs=2)
                    nc.tensor.transpose(tp[:],
                                        ea_sb[:, kc * 128:(kc + 1) * 128]
                                        .bitcast(F32), ident[0:E, 0:E])
                    if kc % 2 == 0:
                        nc.scalar.copy(eaT[kc][:], tp[:])
                    else:
                        nc.vector.tensor_copy(out=eaT[kc][:], in_=tp[:])
                tp = ps1b.tile([1, E], F32, tag="easumt", name="easumt")
                nc.tensor.transpose(tp[:], easum[:], ident[0:E, 0:E])
                nc.scalar.copy(easumT[:], tp[:])

            with tc.tile_pool(name="ps1c", bufs=1, space="PSUM") as ps1c:
                # zT = ea @ x  [22, 768] (two 384-wide halves); rinv applied
                # on the copy out of PSUM
                zt_ps = [ps1c.tile([E, 384], F32, tag=f"zt_p{i}",
                                   name=f"zt_p{i}") for i in range(2)]
                for kc in range(8):
                    if kc == 3:
                        fire("wtr0")
                    elif kc == 6:
                        fire("wtr1")
                    xt = xp[:, kc * HID:(kc + 1) * HID]
                    for hh in range(2):
                        nc.tensor.matmul(zt_ps[hh][:], eaT[kc][:],
                                         xt[:, hh * 384:(hh + 1) * 384],
                                         start=(kc == 0), stop=(kc == 7))
                fire("actr0")
                nc.scalar.activation(zt_sb[:, 0:384], zt_ps[0][:], AF.Copy,
                                     scale=rinv[:])
                nc.scalar.activation(zt_sb[:, 384:768], zt_ps[1][:], AF.Copy,
                                     scale=rinv[:])

            with tc.tile_pool(name="ps1a", bufs=1, space="PSUM") as ps1a:
                # mentions: mrep = x_m @ Wtr + b -> straight into h0b (bf16)
                mrep_p = ps1a.tile([EM, EMB], F32, tag="mrep", name="mrep")
                for kc in range(6):
                    nc.tensor.matmul(mrep_p[:], xmT[kc][:, 0:EM], wtr[kc][:],
                                     start=(kc == 0), stop=False)
                nc.tensor.matmul(mrep_p[:], onesrow[0:1, 0:EM], brow[:],
                                 start=False, stop=True)
                nc.scalar.activation(mrep_b[0:EM, 0:EMB], mrep_p[:], AF.Copy)
                nc.scalar.activation(expm[:], mrep_p[:], AF.Exp)
                # e_rep = ln(G2 @ exp(mrep))
                ep_p = ps1a.tile([E, EMB], F32, tag="ep", name="ep")
                nc.tensor.matmul(ep_p[:], g2T[:], expm[:], start=True, stop=True)
                nc.scalar.activation(eln_b[0:E, 0:EMB], ep_p[:], AF.Ln)
                # swap to the sigmoid table now (every later act is in it);
                # reading ep_p pins this after the Ln in the schedule
                nc.scalar.activation(actwarm[:], ep_p[0:1, 0:1], AF.Sigmoid)

                # w = colsum(attl) / 384
                for mc in range(4):
                    w_p = ps1a.tile([128, 1], F32, tag="w_p", name="w_p", bufs=1)
                    for kc in range(3):
                        nc.tensor.matmul(w_p[:],
                                         attl[kc][:, mc * 128:(mc + 1) * 128],
                                         onescol[:],
                                         start=(kc == 0), stop=(kc == 2))
                    nc.scalar.activation(wsb[mc][:], w_p[:], AF.Copy,
                                         scale=1.0 / (H * SPAN))
                fire("actr1")
                # spans: sp = x_span @ Wtr + b
                for mc in range(4):
                    if mc > 0:
                        fire(f"wstp0{mc - 1}" if mc < 3 else "wstp10")
                    sp_p = ps1a.tile([128, EMB], F32, tag="sp_p", name="sp_p",
                                     bufs=3)
                    for kc in range(6):
                        nc.tensor.matmul(sp_p[:],
                                         xspT[kc][:, mc * 128:(mc + 1) * 128],
                                         wtr[kc][:], start=(kc == 0), stop=False)
                    nc.tensor.matmul(sp_p[:], onesrow[:], brow[:],
                                     start=False, stop=True)
                    spc = pbig.tile([128, EMB], F32, tag="spc", name="spc",
                                    bufs=4)
                    nc.scalar.copy(spc[:], sp_p[:])
                    sp_ps.append(spc)
                # wsp = psum(sp) * w ; link = SUM^T @ wsp
                for mc in range(4):
                    nc.vector.tensor_scalar(out=wsp[mc][:], in0=sp_ps[mc][:],
                                            scalar1=wsb[mc][:], scalar2=None,
                                            op0=ALU.mult)
                link_p = ps1a.tile([L, EMB], F32, tag="link", name="link")
                for kc in range(4):
                    nc.tensor.matmul(link_p[:], sumT[kc][:], wsp[kc][:],
                                     start=(kc == 0), stop=(kc == 3))
                nc.scalar.activation(link_b[0:L, 0:EMB], link_p[:], AF.Copy)
                fire("wstp11")


        if stages >= 2:
          # ================= stage 2: RGCN (4 layers, y col-half pipelined) ==
          ecT = [pwork.tile([128, E], F32R, tag=f"ecT{i}", name=f"ecT{i}")
                 for i in range(4)]
          with tc.tile_pool(name="prg", bufs=2) as prg, \
               tc.tile_pool(name="psr", bufs=1, space="PSUM") as psr:
              h = None
              UW = (NREL + 1) * NN
              for layer in range(NLAYERS):
                  kcs = _KC0 if layer == 0 else _KC1
                  nk = len(kcs)
                  wstp = wstp_t[layer]
                  nrel_l = 2 if layer == 3 else NREL + 1
                  uw_l = 2 * E if layer == 3 else UW
                  wst_t = [wstp[:, (r * nk + si) * EMB:(r * nk + si + 1) * EMB]
                           for r in range(nrel_l) for si in range(nk)]
                  # u = h^T @ A_allT per d-chunk. Layer 3 only needs entity
                  # output rows, which only rel-0 and the self-loop feed.
                  u_sb = []
                  for si, (s0, sl) in enumerate(kcs):
                      u_p = psr.tile([128, (NREL + 1) * NN], F32, tag="u_p",
                                     name="u_p", bufs=2)
                      if layer == 0:
                          for pi, (pt, rows) in enumerate(h0p):
                              nc.tensor.matmul(u_p[0:sl, 0:UW],
                                               pt[0:rows, s0:s0 + sl],
                                               aallTb[0:rows,
                                                      (pi + 1) * UW:
                                                      (pi + 2) * UW],
                                               start=(pi == 0), stop=(pi == 2))
                      elif layer == 3:
                          nc.tensor.matmul(u_p[0:sl, 0:2 * E],
                                           h[0:NN, s0:s0 + sl],
                                           aallTb[0:NN, 4 * UW:4 * UW + 2 * E],
                                           start=True, stop=True)
                      else:
                          nc.tensor.matmul(u_p[0:sl, 0:UW],
                                           h[0:NN, s0:s0 + sl],
                                           aallTb[0:NN, 0:UW],
                                           start=True, stop=True)
                      u = prg.tile([128, (NREL + 1) * NN], BF16, tag=f"u{si}",
                                   name=f"u{si}")
                      if si % 2 == 0:
                          nc.scalar.copy(u[0:sl, 0:uw_l], u_p[0:sl, 0:uw_l])
                      else:
                          nc.vector.tensor_copy(out=u[0:sl, 0:uw_l],
                                                in_=u_p[0:sl, 0:uw_l])
                      u_sb.append(u)
                  # y = sum_r (u_r)^T @ Wst_r, in two column halves so the
                  # relu of half 0 overlaps the matmuls of half 1
                  if layer < 2:
                      fire(f"wstp{layer + 2}0", f"wstp{layer + 2}1")
                  elif layer == 2:
                      fire("sew")
                  # deferred e_ctx chain, one piece per inter-layer window
                  if layer == 0:
                      for kc in range(6):
                          ztp = psr.tile([128, E], F32, tag="tp22", name="ztp",
                                         bufs=2)
                          nc.tensor.transpose(ztp[:],
                                              zt_sb[:, kc * 128:(kc + 1) * 128],
                                              ident[0:E, 0:E])
                          if kc % 2 == 0:
                              nc.scalar.copy(z_sb[kc][:], ztp[:])
                          else:
                              nc.vector.tensor_copy(out=z_sb[kc][:], in_=ztp[:])
                  elif layer == 1:
                      ec2_p = psr.tile([E, EMB], F32, tag="ec2", name="ec2")
                      for kc in range(6):
                          nc.tensor.matmul(ec2_p[:], z_sb[kc][:], wtr[kc][:],
                                           start=(kc == 0), stop=False)
                      nc.tensor.matmul(ec2_p[:], easumT[:], brow[:],
                                       start=False, stop=True)
                      nc.scalar.copy(ec2_sb[:], ec2_p[:])
                  elif layer == 2:
                      for mc in range(4):
                          ecp = psr.tile([128, E], F32, tag="tp22", name="ecp",
                                         bufs=2)
                          nc.tensor.transpose(ecp[:],
                                              ec2_sb[:, mc * 128:(mc + 1) * 128],
                                              ident[0:E, 0:E])
                          if mc % 2 == 0:
                              nc.scalar.copy(ectxT_sb[mc][:], ecp[:])
                          else:
                              nc.vector.tensor_copy(out=ectxT_sb[mc][:],
                                                    in_=ecp[:])
                  nrows = E if layer == 3 else NN
                  rw = E if layer == 3 else NN
                  hn = prg.tile([NN, EMB], BF16, tag="h_next", name="h_next")
                  for yh in range(2):
                      y_p = psr.tile([NN, 256], F32, tag=f"y_p{yh}",
                                     name=f"y_p{yh}")
                      n_mm = nrel_l * nk
                      k_mm = 0
                      for si, (s0, sl) in enumerate(kcs):
                          for r in range(nrel_l):
                              nc.tensor.matmul(
                                  y_p[0:nrows, :],
                                  u_sb[si][0:sl, r * rw:r * rw + nrows],
                                  wst_t[r * nk + si][0:sl,
                                                     yh * 256:(yh + 1) * 256],
                                  start=(k_mm == 0), stop=(k_mm == n_mm - 1))
                              k_mm += 1
                      nc.scalar.activation(hn[0:nrows, yh * 256:(yh + 1) * 256],
                                           y_p[0:nrows, :], AF.Relu)
                  h = hn

              # entity_struT + e_ctxT -> ecT
              for mc in range(4):
                  tp = psr.tile([128, E], F32, tag="tp22", name="est", bufs=2)
                  nc.tensor.matmul(tp[:], h[0:E, mc * 128:(mc + 1) * 128],
                                   identb[0:E, 0:E], start=True, stop=True)
                  nc.vector.tensor_tensor(out=ecT[mc][:], in0=tp[:],
                                          in1=ectxT_sb[mc][:], op=ALU.add)

        prgw_cm.__exit__(None, None, None)

        if stages >= 3:
          # ================= stage 3: fmap + SE =================
          fmap = [pwork.tile([128, EE], F32R, tag=f"fmap{i}", name=f"fmap{i}")
                  for i in range(4)]
          pooled = [pwork.tile([128, 1], F32R, tag=f"pool{i}", name=f"pool{i}")
                    for i in range(4)]
          for mc in range(4):
              o6v = fmap[mc][:].rearrange("p (i j) -> p i j", i=E)
              in0 = ecT[mc][:].rearrange("p (i j) -> p i j", j=1) \
                  .to_broadcast([128, E, E])
              in1 = ecT[mc][:].rearrange("p (o j) -> p o j", o=1) \
                  .to_broadcast([128, E, E])
              nc.vector.tensor_tensor(out=o6v, in0=in0, in1=in1, op=ALU.mult)
              rs = pwork.tile([128, 1], F32, tag=f"rs{mc}", name=f"rs{mc}")
              nc.vector.tensor_reduce(rs[:], ecT[mc][:], mybir.AxisListType.X,
                                      ALU.add)
              nc.scalar.activation(pooled[mc][:], rs[:], AF.Square, scale=1.0 / E)

          with tc.tile_pool(name="pse", bufs=1, space="PSUM") as pse:
              # c-path first (tiny serial chain, hides under fmap/s1p);
              # seb2 is folded into the fcb2 host constant.
              c1_sb = [pwork.tile([128, 1], F32R, tag=f"c1_{i}", name=f"c1_{i}")
                       for i in range(2)]
              for oc in range(2):
                  c1_p = pse.tile([128, 1], F32, tag="cp", name="c1p", bufs=2)
                  for mc in range(4):
                      nc.tensor.matmul(c1_p[:],
                                       sew["fcw1T"][mc][:, oc * 128:(oc + 1) * 128]
                                       .bitcast(F32),
                                       pooled[mc][:].bitcast(F32),
                                       start=(mc == 0), stop=(mc == 3))
                  nc.scalar.activation(c1_sb[oc][:], c1_p[:], AF.Relu,
                                       bias=sev["fcb1"][oc][:],
                                       scale=sev["fcs1"][oc][:])
              cbb = [pwork.tile([128, 1], F32, tag=f"cbb{i}", name=f"cbb{i}")
                     for i in range(4)]
              # s-path; s2's first half starts as soon as s1_sb[0] is ready
              s1_sb = [pwork.tile([128, EE], F32R, tag=f"s1_{i}", name=f"s1_{i}")
                       for i in range(2)]
              for oc in range(2):
                  s1_p = pse.tile([128, EE], F32, tag="s1p", name="s1p", bufs=2)
                  for mc in range(4):
                      nc.tensor.matmul(s1_p[:],
                                       sew["fsw1T"][mc][:, oc * 128:(oc + 1) * 128],
                                       fmap[mc][:], start=(mc == 0), stop=(mc == 3))
                  nc.scalar.activation(s1_sb[oc][:], s1_p[:], AF.Relu,
                                       bias=sev["seb1"][oc][:],
                                       scale=sev["ses1"][oc][:])
              for mc in range(4):
                  c2_p = pse.tile([128, 1], F32, tag="cp", name="c2p", bufs=2)
                  for kc in range(2):
                      nc.tensor.matmul(c2_p[:],
                                       sew["fcw2T"][kc][:, mc * 128:(mc + 1) * 128]
                                       .bitcast(F32),
                                       c1_sb[kc][:].bitcast(F32),
                                       start=(kc == 0), stop=(kc == 1))
                  nc.scalar.activation(cbb[mc][:], c2_p[:], AF.Identity,
                                       bias=sev["fcb2"][mc][:],
                                       scale=sev["fcs2"][mc][:])
              s2_ps = [pse.tile([128, EE], F32, tag="s2p", name=f"s2p{mc}",
                                bufs=3) for mc in range(4)]
              for mc in range(4):
                  nc.tensor.matmul(s2_ps[mc][:],
                                   sew["fsw2T"][0][:, mc * 128:(mc + 1) * 128],
                                   s1_sb[0][:], start=True, stop=False)
              for mc in range(4):
                  nc.tensor.matmul(s2_ps[mc][:],
                                   sew["fsw2T"][1][:, mc * 128:(mc + 1) * 128],
                                   s1_sb[1][:], start=False, stop=True)
                  sig = pwork.tile([128, EE], F32, tag="sig", name="sig", bufs=2)
                  nc.scalar.activation(sig[:], s2_ps[mc][:], AF.Sigmoid,
                                       bias=cbb[mc][:], scale=sev["ses2"][mc][:])
                  outv = fusedp[mc][:].rearrange("p (i j) -> p i j", j=26)[:, 2:24,
                                                                          2:24]
                  nc.vector.tensor_tensor(
                      out=outv,
                      in0=fmap[mc][:].rearrange("p (i j) -> p i j", i=E),
                      in1=sig[:].rearrange("p (i j) -> p i j", i=E),
                      op=ALU.mult)

        if stages >= 4:
          # ================= stage 4: conv stack =================
          def tap_rows(padt, tap, r0, nr):
              dy, dx = tap // 5, tap % 5
              return padt[:].rearrange("p (i j) -> p i j", j=26)[
                  :, dy + r0:dy + r0 + nr, dx:dx + 22]

          def tap_view(padt, tap):
              return tap_rows(padt, tap, 0, 22)

          with tc.tile_pool(name="pcw", bufs=1) as pcw, \
               tc.tile_pool(name="psc", bufs=1, space="PSUM") as psc:
              w1, w2, w3 = [], [], []
              for kc in range(4):
                  t = pcw.tile([128, 25 * 128], BF16, tag=f"w1_{kc}",
                               name=f"w1_{kc}")
                  nc.sync.dma_start(t[:], w1sb_d[kc])
                  w1.append(t)
              for kc in range(2):
                  t = pcw.tile([128, 25 * 128], BF16, tag=f"w2_{kc}",
                               name=f"w2_{kc}")
                  nc.sync.dma_start(t[:], w2sb_d[kc])
                  w2.append(t)
              for kc in range(2):
                  t = pcw.tile([128, 25 * 256], BF16, tag=f"w3_{kc}",
                               name=f"w3_{kc}")
                  nc.sync.dma_start(t[:], w3sb_d[kc])
                  w3.append(t)
              mtop = cf("mtop")
              mbot = cf("mbot")

              def interior_rows(t_, r0, nr):
                  return t_[:].rearrange("p (i j) -> p i j", j=26)[
                      :, 2 + r0:2 + r0 + nr, 2:24]

              def gather_combine(src_pad, dst_pad, gg, rh, nm, rows=None):
                  """relu'd rows rh of src -> allgather -> mask-combine into
                  dst rows rh, issued right after the producing relu."""
                  r0, nr = rows
                  rb = pdram.tile([128, nr * 22], BF16, tag=f"{nm}b{rh}",
                                  name=f"{nm}b{rh}")
                  nc.sync.dma_start(rb[:], interior_rows(src_pad, r0, nr))
                  if solo:
                      nc.sync.dma_start(gg[0:128, :], rb[:])
                      nc.gpsimd.dma_start(gg[128:256, :], rb[:])
                  else:
                      nc.gpsimd.collective_compute(
                          "AllGather", ALU.bypass, replica_groups=groups,
                          ins=[rb[:].opt()], outs=[gg[:].opt()])
                  gt = pcw.tile([128, nr * 22], BF16, tag=f"{nm}gt{rh}",
                                name=f"{nm}gt{rh}")
                  gb = pcw.tile([128, nr * 22], BF16, tag=f"{nm}gb{rh}",
                                name=f"{nm}gb{rh}")
                  nc.sync.dma_start(gt[:], gg[0:128, :])
                  nc.gpsimd.dma_start(gb[:], gg[128:256, :])
                  tmp = pcw.tile([128, nr * 22], F32, tag=f"{nm}tmp{rh}",
                                 name=f"{nm}tmp{rh}")
                  nc.vector.tensor_scalar(out=tmp[:], in0=gb[:],
                                          scalar1=mbot[:], scalar2=None,
                                          op0=ALU.mult)
                  nc.vector.scalar_tensor_tensor(
                      out=interior_rows(dst_pad, r0, nr), in0=gt[:],
                      scalar=mtop[:], in1=tmp[:], op0=ALU.mult, op1=ALU.add)

              # conv1 rows (0,18),(18,4): the tiny second half's gather
              # launches right at conv1's end, so its readback chain hides
              # under conv2's first-half taps.
              ROWS = ((0, 18), (18, 4))

              # conv1 (my half of 256 out channels), in two row halves; each
              # half's AllGather + readback starts as soon as the half is done
              r1g = [pdram.tile([256, nr * 22], BF16, tag=f"r1g{i}",
                                name=f"r1g{i}")
                     for i, (r0, nr) in enumerate(ROWS)]
              for rh, (r0, nr) in enumerate(ROWS):
                  r1_p = psc.tile([128, nr * 22], F32, tag="convp",
                                  name="convp", bufs=4)
                  first = True
                  for kc in range(4):
                      for tap in range(25):
                          nc.tensor.matmul(r1_p[:],
                                           w1[kc][:, tap * 128:(tap + 1) * 128],
                                           tap_rows(fusedp[kc], tap, r0, nr),
                                           start=first,
                                           stop=(kc == 3 and tap == 24))
                          first = False
                  nc.scalar.activation(interior_rows(own1, r0, nr), r1_p[:],
                                       AF.Relu, bias=b1h[:])
                  gather_combine(own1, oth1, r1g[rh], rh, "r1", (r0, nr))

              # conv2: own-input taps for both row-halves first (no gather
              # dependency), then other-input taps; output in row halves.
              # conv2 out rows (0-8, 9-21): the first half's own AND other
              # taps read only rows <=12 of r1, i.e. conv1-half0 + gather#0 —
              # no wait on conv1-half1's gather.
              ROWS2 = ((0, 14), (14, 8))
              r2g = [pdram.tile([256, nr * 22], BF16, tag=f"r2g{i}",
                                name=f"r2g{i}")
                     for i, (r0, nr) in enumerate(ROWS2)]
              r2_ps = [psc.tile([128, nr * 22], F32, tag=f"convp2_{i}",
                                name=f"convp2_{i}")
                       for i, (r0, nr) in enumerate(ROWS2)]
              # rh1's own-taps first (ready at conv1 end) to widen the
              # window that hides gather#1's readback chain; rh0 is fully
              # ready (gather#0 landed mid-conv1) and runs next.
              r10, n1 = ROWS2[1]
              for tap in range(25):
                  nc.tensor.matmul(r2_ps[1][:],
                                   w2[0][:, tap * 128:(tap + 1) * 128],
                                   tap_rows(own1, tap, r10, n1),
                                   start=(tap == 0), stop=False)
              r0, nr = ROWS2[0]
              for tap in range(25):
                  nc.tensor.matmul(r2_ps[0][:],
                                   w2[0][:, tap * 128:(tap + 1) * 128],
                                   tap_rows(own1, tap, r0, nr),
                                   start=(tap == 0), stop=False)
              for tap in range(25):
                  nc.tensor.matmul(r2_ps[0][:],
                                   w2[1][:, tap * 128:(tap + 1) * 128],
                                   tap_rows(oth1, tap, r0, nr),
                                   start=False, stop=(tap == 24))
              nc.scalar.activation(interior_rows(own2, r0, nr), r2_ps[0][:],
                                   AF.Relu, bias=b2h[:])
              gather_combine(own2, oth2, r2g[0], 0, "r2", (r0, nr))
              for tap in range(25):
                  nc.tensor.matmul(r2_ps[1][:],
                                   w2[1][:, tap * 128:(tap + 1) * 128],
                                   tap_rows(oth1, tap, r10, n1),
                                   start=False, stop=(tap == 24))
              nc.scalar.activation(interior_rows(own2, r10, n1), r2_ps[1][:],
                                   AF.Relu, bias=b2h[:])
              gather_combine(own2, oth2, r2g[1], 1, "r2", (r10, n1))

              # conv3 (my 256 of 512 out channels): own-input taps for both
              # out chunks first, then other-input taps per (oc, row-half)
              # with relu+store pipelined per row half (short tail).
              r3_ps = [psc.tile([128, ROWS[rh][1] * 22], F32, tag="convp",
                                name=f"convp3_{oc}{rh}", bufs=4)
                       for oc in range(2) for rh in range(2)]
              for oc in range(2):
                  for rh, (r0, nr) in enumerate(ROWS):
                      for tap in range(25):
                          nc.tensor.matmul(
                              r3_ps[oc * 2 + rh][:],
                              w3[0][:, tap * 256 + oc * 128:
                                    tap * 256 + (oc + 1) * 128],
                              tap_rows(own2, tap, r0, nr),
                              start=(tap == 0), stop=False)
              for oc in range(2):
                  for rh, (r0, nr) in enumerate(ROWS):
                      for tap in range(25):
                          nc.tensor.matmul(
                              r3_ps[oc * 2 + rh][:],
                              w3[1][:, tap * 256 + oc * 128:
                                    tap * 256 + (oc + 1) * 128],
                              tap_rows(oth2, tap, r0, nr),
                              start=False, stop=(tap == 24))
                      o_sb = pcw.tile([128, nr * 22], F32, tag="osb",
                                      name="osb", bufs=4)
                      nc.scalar.activation(o_sb[:], r3_ps[oc * 2 + rh][:],
                                           AF.Relu, bias=b3h[oc][:])
                      nc.sync.dma_start(
                          out_d[oc * 128:(oc + 1) * 128,
                                r0 * 22:(r0 + nr) * 22], o_sb[:])

    nc.compile()
    return nc


_NC_CACHE = None


def _get_program():
    global _NC_CACHE
    if _NC_CACHE is None:
        _NC_CACHE = build_program()
    return _NC_CACHE


def _prep_shared(w):
    """Packed weights/constants identical on every core."""
    ADJ = _build_adj()
    out = {}
    constr = np.zeros((128, _CR), np.float32)

    def put(nm, arr):
        c0, cols = _LAY_R[nm]
        r, cc = arr.shape
        constr[0:r, c0:c0 + cc] = arr
    g2T = np.zeros((EM, E), np.float32)
    for e in range(E):
        g2T[e * M:(e + 1) * M, e] = 1.0
    put("g2T", g2T)
    sumT = np.kron(np.eye(L, dtype=np.float32), np.ones((SPAN, 1), np.float32))
    for kc in range(4):
        put(f"sumT{kc}", sumT[kc * 128:(kc + 1) * 128])
    for nm, arr, nch in (("fsw1T", w['fs_w1'].T, 4), ("fcw1T", w['fc_w1'].T, 4),
                         ("fsw2T", w['fs_w2'].T, 2), ("fcw2T", w['fc_w2'].T, 2)):
        for kc in range(nch):
            put(f"{nm}{kc}", np.ascontiguousarray(arr[kc * 128:(kc + 1) * 128]))
    out['constr'] = constr

    constw = np.zeros((128, _CW), np.float32)
    wt = w['W_trans']
    for kc in range(6):
        constw[:, kc * EMB:(kc + 1) * EMB] = wt[kc * 128:(kc + 1) * 128]
    constw[0, 6 * EMB:7 * EMB] = w['b_trans']
    out['constw'] = constw.astype(ml_dtypes.bfloat16)

    gTb = np.zeros((128, 9 * E), np.float32)
    gT = np.zeros((EMH, E), np.float32)
    for e in range(E):
        gT[e * M * H:(e + 1) * M * H, e] = 1.0 / (M * H)
    for kc in range(9):
        r = min(128, EMH - kc * 128)
        gTb[0:r, kc * E:(kc + 1) * E] = gT[kc * 128:kc * 128 + r]
    out['gTb'] = gTb.astype(ml_dtypes.bfloat16)
    aall = np.concatenate(
        [ADJ[r].T for r in range(NREL)] + [np.eye(NN, dtype=np.float32)],
        axis=1)
    UW = (NREL + 1) * NN
    aallp = np.zeros((128, 4 * UW + 2 * E), np.float32)
    aallp[0:NN, 0:UW] = aall
    aallp[0:EM, UW:2 * UW] = aall[E:E + EM]
    aallp[0:L, 2 * UW:3 * UW] = aall[E + EM:NN]
    aallp[0:E, 3 * UW:4 * UW] = aall[0:E]
    # layer-3 entity-only columns: [A0^T[:, :E] | I[:, :E]]
    aallp[0:NN, 4 * UW:4 * UW + E] = aall[:, 0:E]
    aallp[0:NN, 4 * UW + E:4 * UW + 2 * E] = aall[:, NREL * NN:NREL * NN + E]
    out['aallTb'] = aallp.astype(ml_dtypes.bfloat16)
    out['tfb'] = np.ascontiguousarray(
        w['type_embed'][_TYPES]).astype(ml_dtypes.bfloat16)
    out['identb'] = np.eye(128, dtype=np.float32).astype(ml_dtypes.bfloat16)

    constf = np.zeros((128, _CF), np.float32)

    def putf(nm, arr):
        c0, cols = _LAY_F[nm]
        constf[0:arr.shape[0], c0:c0 + 1] = arr.reshape(-1, 1)
    vecs = {"ses1": w['fs_g1'], "seb1": w['fs_b1'] * w['fs_g1'] + w['fs_be1'],
            "fcs1": w['fc_g1'], "fcb1": w['fc_b1'] * w['fc_g1'] + w['fc_be1'],
            "ses2": w['fs_g2'], "seb2": w['fs_b2'] * w['fs_g2'] + w['fs_be2'],
            "fcs2": w['fc_g2'],
            "fcb2": (w['fc_b2'] * w['fc_g2'] + w['fc_be2'] +
                     w['fs_b2'] * w['fs_g2'] + w['fs_be2'])}
    for nm, v in vecs.items():
        nch = 2 if v.shape[0] == INTER else 4
        for kc in range(nch):
            putf(f"{nm}{kc}", v[kc * 128:(kc + 1) * 128])
    out['constf_base'] = constf

    for layer in range(NLAYERS):
        din_l = D0 if layer == 0 else EMB
        kcs = _KC0 if layer == 0 else _KC1
        nk = len(kcs)
        Wst = w['rgcn_Wrel0'].reshape(NREL * D0, EMB) if layer == 0 else \
            w['rgcn_Wrel'][layer - 1].reshape(NREL * EMB, EMB)
        Wself = w['rgcn_Wself0'] if layer == 0 else w['rgcn_Wself'][layer - 1]
        if layer == 3:
            # entity rows only need rel-0 and the self-loop
            p = np.zeros((128, 2 * nk * EMB), np.float32)
            for si, (s0, sl) in enumerate(kcs):
                p[0:sl, si * EMB:(si + 1) * EMB] = Wst[s0:s0 + sl]
                p[0:sl, (nk + si) * EMB:(nk + si + 1) * EMB] = \
                    Wself[s0:s0 + sl]
        else:
            p = np.zeros((128, (NREL + 1) * nk * EMB), np.float32)
            for r in range(NREL):
                for si, (s0, sl) in enumerate(kcs):
                    p[0:sl, (r * nk + si) * EMB:(r * nk + si + 1) * EMB] = \
                        Wst[r * din_l + s0:r * din_l + s0 + sl]
            for si, (s0, sl) in enumerate(kcs):
                p[0:sl, (NREL * nk + si) * EMB:(NREL * nk + si + 1) * EMB] = \
                    Wself[s0:s0 + sl]
        out[f'wstp{layer}'] = p.astype(ml_dtypes.bfloat16)
    return out


def _prep_conv_half(w, half, constf_base):
    out = {}
    w1 = w['cr_w1'][half * 128:(half + 1) * 128]
    out['w1sb'] = np.ascontiguousarray(
        w1.transpose(1, 2, 3, 0).reshape(4, 128, 25 * 128)).astype(ml_dtypes.bfloat16)
    # conv2/conv3 weights in (own-input-half, other-input-half) chunk order
    w2 = w['cr_w2'][half * 128:(half + 1) * 128]
    w2p = w2.transpose(1, 2, 3, 0).reshape(2, 128, 25 * 128)
    order = [half, 1 - half]
    out['w2sb'] = np.ascontiguousarray(w2p[order]).astype(ml_dtypes.bfloat16)
    w3 = w['cr_w3'][half * 256:(half + 1) * 256]
    w3p = w3.transpose(1, 2, 3, 0).reshape(2, 128, 25 * 256)
    out['w3sb'] = np.ascontiguousarray(w3p[order]).astype(ml_dtypes.bfloat16)
    constf = constf_base.copy()

    def putf(nm, arr):
        c0, cols = _LAY_F[nm]
        constf[0:arr.shape[0], c0:c0 + 1] = arr.reshape(-1, 1)
    putf("b1h", w['cr_b1'][half * 128:(half + 1) * 128])
    putf("b2h", w['cr_b2'][half * 128:(half + 1) * 128])
    putf("b3h0", w['cr_b3'][half * 256:half * 256 + 128])
    putf("b3h1", w['cr_b3'][half * 256 + 128:half * 256 + 256])
    putf("mtop", np.full(128, float(half), np.float32))
    putf("mbot", np.full(128, float(1 - half), np.float32))
    c0, cols = _LAY_F["identf"]
    constf[:, c0:c0 + 128] = np.eye(128, dtype=np.float32)
    out['constf'] = constf
    return out


def _prep_doc(x, att, mi, ls):
    out = {}
    mif = mi.reshape(EM)
    attm = np.ascontiguousarray(
        att[:, mif, :].transpose(1, 0, 2).reshape(EMH, C))
    amp = np.zeros((128, 9 * C), np.float32)
    for kc in range(9):
        r = min(128, EMH - kc * 128)
        amp[0:r, kc * C:kc * C + C] = attm[kc * 128:kc * 128 + r]
    out['amp'] = amp.astype(ml_dtypes.bfloat16)
    idx = ls[:, None] + np.arange(SPAN)
    idxf = idx.reshape(LS)
    rows = att[:, idxf, :].reshape(H, L, SPAN, C)
    blocks = np.take_along_axis(rows, idx[None, :, None, :], axis=3)
    attl = blocks.transpose(0, 2, 1, 3).reshape(HS, LS)
    xmT = x[mif].T
    xspT = x[idxf].T
    actr = np.zeros((128, _CA), np.float32)

    def put(nm, arr):
        c0, cols = _LAY_A[nm]
        actr[0:arr.shape[0], c0:c0 + arr.shape[1]] = arr
    for kc in range(6):
        put(f"xmT{kc}", xmT[kc * 128:(kc + 1) * 128])
        put(f"xspT{kc}", xspT[kc * 128:(kc + 1) * 128])
    for kc in range(3):
        put(f"attl{kc}", attl[kc * 128:(kc + 1) * 128])
    out['actr'] = actr.astype(ml_dtypes.bfloat16)
    xpk = np.zeros((128, 8 * HID), np.float32)
    for kc in range(8):
        xpk[:, kc * HID:(kc + 1) * HID] = x[kc * 128:(kc + 1) * 128]
    out['xp'] = xpk.astype(ml_dtypes.bfloat16)
    return out


def build_in_maps(inputs):
    w = {}
    for k, v in inputs.items():
        a = np.asarray(v)
        w[k] = a if a.dtype in (np.int32, np.int64) else \
            np.asarray(a, np.float32)
    shared = _prep_shared(w)
    constf_base = shared.pop('constf_base')
    halves = [_prep_conv_half(w, h, constf_base) for h in range(2)]
    seq = np.asarray(inputs['sequence_output'], np.float32)
    att = np.asarray(inputs['attention'], np.float32)
    mi = np.asarray(inputs['mention_idx']).astype(np.int64)
    ls = np.asarray(inputs['link_start']).astype(np.int64)
    docs = [_prep_doc(seq[n], att[n], mi[n], ls[n]) for n in range(NB)]
    in_maps = []
    for core in range(N_CORES):
        n, half = core // 2, core % 2
        m = dict(shared)
        m.update(halves[half])
        m.update(docs[n])
        in_maps.append({k: (np.ascontiguousarray(v) if v.dtype == ml_dtypes.bfloat16
                            else np.ascontiguousarray(v, np.float32))
                        for k, v in m.items()})
    return in_maps


def kernel(**inputs):
    nc = _get_program()
    in_maps = build_in_maps(inputs)
    res = run_bass_kernel_spmd(nc, in_maps, list(range(N_CORES)))
    out = np.zeros((NB, EMB, E, E), np.float32)
    for core in range(N_CORES):
        n, half = core // 2, core % 2
        out[n, half * 256:(half + 1) * 256] = \
            res.results[core]["out"].reshape(256, E, E)
    return out


# revision 30
# speedup vs baseline: 1.0083x; 1.0050x over previous
"""Trainium2 Bass kernel for nn_DocREModel (DocRE: gather -> RGCN -> SE -> 5x5 convs).

Sharding: 4 documents x 2 cores each. Each pair replicates the cheap upstream
(mention/link/ea gathers -> RGCN -> fmap/SE) and splits the dominant 5x5 conv
stack by output channels, with intra-pair AllGathers; output halves are
assembled on host. All index-driven gathers happen on host (pure data
movement; one SPMD program serves all 8 cores), all dense math on device.

Perf notes (v3):
- DMA is a serialized ~360B/ns resource in the cost model; all input loads
  go on ONE queue (sync/SP) in exact consumption order, in <=~800KB pieces,
  so arrival order is deterministic and matches the compute schedule.
- The PE p-state ramp resets on >~2us idle gaps. Tiny "touch" matmuls that
  read one column of each arriving DMA piece pulse the PE at piece cadence,
  holding the fast clock through DMA-bound phases.
- bf16 for W_trans, gathered activations, RGCN + conv weights (f32 PSUM
  accumulation everywhere); f32r for the remaining f32 path.
- Convs are 25 shift-tap matmuls over zero-padded 26x26 images via strided
  APs. conv1/conv2 outputs are produced in two 11-row halves so each half's
  AllGather (+ readback + mask-combine) overlaps the next half's matmuls;
  conv3 output is relu'd + stored per (out-chunk, row-half) to shrink the
  kernel tail. RGCN folds the self-loop in as a 4th identity relation and
  pipelines each layer by y column halves.
"""

import numpy as np
import ml_dtypes

import concourse.bacc as bacc
import concourse.tile as tile
from concourse import mybir
from concourse.bass_utils import run_bass_kernel_spmd

F32 = mybir.dt.float32
F32R = mybir.dt.float32r
BF16 = mybir.dt.bfloat16
AF = mybir.ActivationFunctionType
ALU = mybir.AluOpType

NB, H, C, HID, EMB = 4, 12, 1024, 768, 512
E, M, L, SPAN = 22, 4, 16, 32
TD, INTER = 20, 256
NN = E + E * M + L
NREL, NLAYERS = 3, 4
EM, EMH, HS, LS = E * M, E * M * H, H * SPAN, L * SPAN
D0 = EMB + TD           # 532
EE = E * E              # 484
PADW = 26 * 26          # 676 padded 26x26 image
N_CORES = 8


def _build_adj():
    A = np.zeros((NREL, NN, NN), np.float32)
    for e in range(E):
        for m in range(M):
            mi = E + e * M + m
            A[0, e, mi] = A[0, mi, e] = 1.0
            for m2 in range(M):
                if m2 != m:
                    A[1, mi, E + e * M + m2] = 1.0
            li = E + E * M + ((e * M + m) % L)
            A[2, mi, li] = A[2, li, mi] = 1.0
    A = A / (A.sum(-1, keepdims=True) + 1e-5)
    return A


_TYPES = np.concatenate([np.zeros(E, np.int32), np.ones(EM, np.int32),
                         np.full(L, 2, np.int32)])

_KC0 = [(0, 128), (128, 128), (256, 128), (384, 128), (512, 20)]   # 532 rows
_KC1 = [(0, 128), (128, 128), (256, 128), (384, 128)]              # 512 rows


def _const_layout():
    """f32r constants [128, CR]: stage-1 smalls + SE weights."""
    lay = {}
    c = 0

    def add(nm, cols):
        nonlocal c
        lay[nm] = (c, cols)
        c += cols
    add("g2T", E)
    for kc in range(4):
        add(f"sumT{kc}", L)
    for kc in range(4):
        add(f"fsw1T{kc}", INTER)
    for kc in range(4):
        add(f"fcw1T{kc}", INTER)
    for kc in range(2):
        add(f"fsw2T{kc}", EMB)
    for kc in range(2):
        add(f"fcw2T{kc}", EMB)
    return lay, c


def _constf_layout():
    lay = {}
    c = 0

    def add(nm, cols):
        nonlocal c
        lay[nm] = (c, cols)
        c += cols
    for nm, nch in (("ses1", 2), ("seb1", 2), ("fcs1", 2), ("fcb1", 2),
                    ("ses2", 4), ("seb2", 4), ("fcs2", 4), ("fcb2", 4)):
        for kc in range(nch):
            add(f"{nm}{kc}", 1)
    add("b1h", 1)
    add("b2h", 1)
    add("b3h0", 1)
    add("b3h1", 1)
    add("mtop", 1)
    add("mbot", 1)
    add("identf", 128)
    return lay, c


def _actr_layout():
    """bf16 gathered activations [128, CA]."""
    lay = {}
    c = 0

    def add(nm, cols):
        nonlocal c
        lay[nm] = (c, cols)
        c += cols
    for kc in range(6):
        add(f"xmT{kc}", EM)
    for kc in range(3):
        add(f"attl{kc}", LS)
    for kc in range(6):
        add(f"xspT{kc}", LS)
    return lay, c


_LAY_R, _CR = _const_layout()
_LAY_F, _CF = _constf_layout()
_LAY_A, _CA = _actr_layout()

_CRB = _LAY_R["fsw1T0"][0]          # stage-1 smalls | SE weights split
_CAA = _LAY_A["xspT0"][0]           # xmT+attl | xspT split
_CW = 7 * EMB                       # constrw (bf16): wtr chunks + brow block


def build_program(solo=False, stages=4):
    nc = bacc.Bacc("TRN2", target_bir_lowering=False, debug=False)

    def din(name, shape, dt=F32R):
        return nc.dram_tensor(name, list(shape), dt, kind="ExternalInput").ap()

    constr_d = din("constr", [128, _CR])
    constf_d = din("constf", [128, _CF], F32)
    constw_d = din("constw", [128, _CW], BF16)
    actr_d = din("actr", [128, _CA], BF16)
    xp_d = din("xp", [128, 8 * HID], BF16)
    amp_d = din("amp", [128, 9 * C], BF16)
    gTb_d = din("gTb", [128, 9 * E], BF16)
    wstp_d = [din("wstp0", [128, 20 * EMB], BF16)] + \
             [din(f"wstp{i}", [128, 16 * EMB], BF16) for i in (1, 2)] + \
             [din("wstp3", [128, 8 * EMB], BF16)]
    w1sb_d = din("w1sb", [4, 128, 25 * 128], BF16)
    w2sb_d = din("w2sb", [2, 128, 25 * 128], BF16)
    w3sb_d = din("w3sb", [2, 128, 25 * 256], BF16)
    aallTb_d = din("aallTb", [128, 4 * (NREL + 1) * NN + 2 * E], BF16)
    tfb_d = din("tfb", [NN, TD], BF16)
    identb_d = din("identb", [128, 128], BF16)

    out_d = nc.dram_tensor("out", [256, EE], F32, kind="ExternalOutput").ap()

    groups = [[0, 1], [2, 3], [4, 5], [6, 7]]

    with tile.TileContext(nc) as tc:
      with tc.tile_pool(name="pconst", bufs=1) as pconst, \
           tc.tile_pool(name="pwork", bufs=1) as pwork, \
           tc.tile_pool(name="pwarm", bufs=1, space="PSUM") as pwarm, \
           tc.tile_pool(name="pdram", bufs=1, space="DRAM") as pdram:
        prgw_cm = tc.tile_pool(name="prgw", bufs=1)
        prgw = prgw_cm.__enter__()

        constr = pconst.tile([128, _CR], F32R)
        constf = pconst.tile([128, _CF], F32)
        constw = pconst.tile([128, _CW], BF16)
        identb = pconst.tile([128, 128], BF16)
        aallTb = pconst.tile([128, 4 * (NREL + 1) * NN + 2 * E], BF16)
        onesb = pconst.tile([128, 128], BF16)
        nc.vector.memset(onesb[:], 1.0)
        # scratch for the sigmoid-table swap dummy (see ep below)
        actwarm = pconst.tile([1, 1], F32)

        def cr(nm, rows=128):
            c0, cols = _LAY_R[nm]
            return constr[0:rows, c0:c0 + cols]

        def cf(nm, rows=128):
            c0, cols = _LAY_F[nm]
            return constf[0:rows, c0:c0 + cols]

        wtr = [constw[:, kc * EMB:(kc + 1) * EMB] for kc in range(6)]
        brow = constw[0:1, 6 * EMB:7 * EMB]
        onescol = onesb[:, 0:1]
        onesrow = onesb[0:1, :]
        g2T = cr("g2T", rows=EM)
        sumT = [cr(f"sumT{kc}") for kc in range(4)]
        sew = {nm: [cr(f"{nm}{kc}") for kc in range(n)]
               for nm, n in (("fsw1T", 4), ("fcw1T", 4), ("fsw2T", 2),
                             ("fcw2T", 2))}
        sev = {nm: [cf(f"{nm}{kc}") for kc in range(n)]
               for nm, n in (("ses1", 2), ("seb1", 2), ("fcs1", 2), ("fcb1", 2),
                             ("ses2", 4), ("seb2", 4), ("fcs2", 4),
                             ("fcb2", 4))}
        b1h = cf("b1h")
        b2h = cf("b2h")
        b3h = [cf("b3h0"), cf("b3h1")]
        ident = cf("identf")

        # conv pad images: memset early while the DVE is idle
        fusedp = [pwork.tile([128, PADW], BF16, tag=f"fusedp{i}",
                             name=f"fusedp{i}") for i in range(4)]
        for t_ in fusedp:
            nc.vector.memset(t_[:], 0.0)
        own1 = pwork.tile([128, PADW], BF16)
        oth1 = pwork.tile([128, PADW], BF16)
        own2 = pwork.tile([128, PADW], BF16)
        oth2 = pwork.tile([128, PADW], BF16)
        for t_ in (own1, oth1, own2, oth2):
            nc.vector.memset(t_[:], 0.0)

        # persistent intermediates: h0 kept as three partition-0 node-type
        # parts (entities / mentions / links); RGCN layer 0 contracts over
        # the parts with a 3-part adjacency tensor.
        eln_b = pwork.tile([E, D0], BF16)
        mrep_b = pwork.tile([EM, D0], BF16)
        link_b = pwork.tile([L, D0], BF16)
        h0p = [(mrep_b, EM), (link_b, L), (eln_b, E)]
        ectxT_sb = [pwork.tile([128, E], F32, tag=f"ectxT{i}", name=f"ectxT{i}")
                    for i in range(4)]
        z_sb = [pwork.tile([128, E], BF16, tag=f"z{i}", name=f"z{i}")
                for i in range(6)]
        easumT = pwork.tile([1, E], BF16)
        zt_sb = pwork.tile([E, HID], F32)
        ec2_sb = pwork.tile([E, EMB], F32)

        # ---------------- ordered DMA supply script (sync/SP queue) ---------
        # Exact consumption order, <=~800KB pieces. "touch" matmuls (fired at
        # chosen program points) read one column of an arriving DMA piece and
        # pulse the PE, so the p-state ramp never resets during DMA-bound
        # stretches.
        warm = pwarm.tile([1, 1], F32)
        tch = {}

        def fire(*names):
            for nm in names:
                nc.tensor.matmul(warm[:], tch[nm], tch[nm],
                                 start=True, stop=True)

        gTbt = prgw.tile([128, 9 * E], BF16)
        nc.scalar.dma_start(gTbt[:], gTb_d[:])
        amp = prgw.tile([128, 9 * C], BF16)
        nc.sync.dma_start(amp[:, 0:C], amp_d[:, 0:C])
        nc.sync.dma_start(constf[:], constf_d[:])
        for (p0, pn) in ((1, 2), (3, 2), (5, 2), (7, 2)):
            nc.sync.dma_start(amp[:, p0 * C:(p0 + pn) * C],
                              amp_d[:, p0 * C:(p0 + pn) * C])
        xp = prgw.tile([128, 8 * HID], BF16)
        for i, p0 in enumerate(range(0, 8, 2)):
            nc.sync.dma_start(xp[:, p0 * HID:(p0 + 2) * HID],
                              xp_d[:, p0 * HID:(p0 + 2) * HID])
            tch[f"xp{i}"] = xp[:, p0 * HID:p0 * HID + 1]
        for i, (p0, pn) in enumerate(((0, 3), (3, 4))):   # wtr + brow block
            nc.sync.dma_start(constw[:, p0 * EMB:(p0 + pn) * EMB],
                              constw_d[:, p0 * EMB:(p0 + pn) * EMB])
            tch[f"wtr{i}"] = constw[:, p0 * EMB:p0 * EMB + 1]
        nc.sync.dma_start(constr[:, 0:_CRB], constr_d[:, 0:_CRB])
        actr = prgw.tile([128, _CA], BF16)
        nc.sync.dma_start(actr[:, 0:_CAA], actr_d[:, 0:_CAA])
        tch["actr0"] = actr[:, 0:1]
        nc.sync.dma_start(actr[:, _CAA:_CA], actr_d[:, _CAA:_CA])
        tch["actr1"] = actr[:, _CAA:_CAA + 1]
        # RGCN weights
        wstp_t = [prgw.tile([128, 20 * EMB], BF16, tag="wstp0",
                            name="wstp0")] + \
                 [prgw.tile([128, 16 * EMB], BF16, tag=f"wstp{i}",
                            name=f"wstp{i}") for i in (1, 2)] + \
                 [prgw.tile([128, 8 * EMB], BF16, tag="wstp3", name="wstp3")]
        for layer in range(NLAYERS):
            ncols = (20 * EMB if layer == 0 else
                     16 * EMB if layer < 3 else 8 * EMB)
            hh = ncols // 2
            for i, (p0, pn) in enumerate(((0, hh), (hh, ncols - hh))):
                nc.sync.dma_start(wstp_t[layer][:, p0:p0 + pn],
                                  wstp_d[layer][:, p0:p0 + pn])
                tch[f"wstp{layer}{i}"] = wstp_t[layer][:, p0:p0 + 1]
        # SE weights
        nc.sync.dma_start(constr[:, _CRB:_CR], constr_d[:, _CRB:_CR])
        tch["sew"] = constr[:, _CRB:_CRB + 1].bitcast(F32)


        def ca(nm, rows=128):
            c0, cols = _LAY_A[nm]
            return actr[0:rows, c0:c0 + cols]

        xmT = [ca(f"xmT{kc}") for kc in range(6)]
        xspT = [ca(f"xspT{kc}") for kc in range(6)]
        attl = [ca(f"attl{kc}") for kc in range(3)]

        def fire_touches():
            for ap_sl in touches:
                nc.tensor.matmul(warm[:], ap_sl, ap_sl, start=True, stop=True)

        # ================= stage 1: gathered-row transforms =================
        with tc.tile_pool(name="pbig", bufs=1) as pbig:
            expm = pbig.tile([EM, EMB], F32R)
            sp_ps = []
            wsb = [pbig.tile([128, 1], F32, tag=f"wsb{i}", name=f"wsb{i}")
                   for i in range(4)]
            wsp = [pbig.tile([128, EMB], F32R, tag=f"wsp{i}", name=f"wsp{i}")
                   for i in range(4)]
            ea_sb = pbig.tile([E, C], F32R)
            eaT = [pbig.tile([128, E], BF16, tag=f"eaT{i}", name=f"eaT{i}")
                   for i in range(8)]


            with tc.tile_pool(name="ps1b", bufs=1, space="PSUM") as ps1b:
                # ea = G^T @ attm ; normalize rows
                ea_p0 = ps1b.tile([E, 512], F32, tag="ea0", name="ea0")
                ea_p1 = ps1b.tile([E, 512], F32, tag="ea1", name="ea1")
                for kc in range(9):
                    rows = 128 if kc < 8 else 32
                    at = amp[0:rows, kc * C:kc * C + C]
                    gt = gTbt[0:rows, kc * E:(kc + 1) * E]
                    nc.tensor.matmul(ea_p0[:], gt, at[:, 0:512],
                                     start=(kc == 0), stop=(kc == 8))
                    nc.tensor.matmul(ea_p1[:], gt, at[:, 512:1024],
                                     start=(kc == 0), stop=(kc == 8))
                fire("xp0", "xp1")
                r0 = pbig.tile([E, 1], F32)
                r1 = pbig.tile([E, 1], F32)
                nc.vector.tensor_reduce(r0[:], ea_p0[:], mybir.AxisListType.X,
                                        ALU.add)
                nc.vector.tensor_reduce(r1[:], ea_p1[:], mybir.AxisListType.X,
                                        ALU.add)
                rsum = pbig.tile([E, 1], F32)
                nc.vector.tensor_tensor(out=rsum[:], in0=r0[:], in1=r1[:],
                                        op=ALU.add)
                rsum2 = pbig.tile([E, 1], F32)
                nc.vector.tensor_scalar(out=rsum2[:], in0=rsum[:], scalar1=1e-5,
                                        scalar2=None, op0=ALU.add)
                rinv = pbig.tile([E, 1], F32)
                nc.vector.reciprocal(rinv[:], rsum2[:])
                # ea left unnormalized; rinv is applied as a row scale on the
                # zt copy below (zt is linear in ea), shortening the critical
                # chain ea -> eaT -> zt.
                nc.scalar.activation(ea_sb[:, 0:512], ea_p0[:], AF.Copy)
                nc.scalar.activation(ea_sb[:, 512:1024], ea_p1[:], AF.Copy)
                easum = pbig.tile([E, 1], F32)
                nc.vector.tensor_tensor(out=easum[:], in0=rsum[:], in1=rinv[:],
                                        op=ALU.mult)
                # small late-use constants; placed here so their HWDGE slots
                # don't delay the early amp pieces
                nc.scalar.dma_start(eln_b[0:E, EMB:D0], tfb_d[0:E, :])
                nc.scalar.dma_start(mrep_b[0:EM, EMB:D0], tfb_d[E:E + EM, :])
                nc.scalar.dma_start(link_b[0:L, EMB:D0], tfb_d[E + EM:NN, :])
                nc.scalar.dma_start(aallTb[:], aallTb_d[:])
                nc.scalar.dma_start(identb[:], identb_d[:])
                for kc in range(8):
                    if kc == 2:
                        fire("xp2")
                    elif kc == 4:
                        fire("xp3")
                    tp = ps1b.tile([128, E], F32, tag="eaTt", name="eaTt", bufs=2)
                    nc.tensor.transpose(tp[:],
                                        ea_sb[:, kc * 128:(kc + 1) * 128]
                                        .bitcast(F32), ident[0:E, 0:E])
                    if kc % 2 == 0:
                        nc.scalar.copy(eaT[kc][:], tp[:])
                    else:
                        nc.vector.tensor_copy(out=eaT[kc][:], in_=tp[:])
                tp = ps1b.tile([1, E], F32, tag="easumt", name="easumt")
                nc.tensor.transpose(tp[:], easum[:], ident[0:E, 0:E])
                nc.scalar.copy(easumT[:], tp[:])

            with tc.tile_pool(name="ps1c", bufs=1, space="PSUM") as ps1c:
                # zT = ea @ x  [22, 768] (two 384-wide halves); rinv applied
                # on the copy out of PSUM
                zt_ps = [ps1c.tile([E, 384], F32, tag=f"zt_p{i}",
                                   name=f"zt_p{i}") for i in range(2)]
                for kc in range(8):
                    if kc == 3:
                        fire("wtr0")
                    elif kc == 6:
                        fire("wtr1")
                    xt = xp[:, kc * HID:(kc + 1) * HID]
                    for hh in range(2):
                        nc.tensor.matmul(zt_ps[hh][:], eaT[kc][:],
                                         xt[:, hh * 384:(hh + 1) * 384],
                                         start=(kc == 0), stop=(kc == 7))
                fire("actr0")
                nc.scalar.activation(zt_sb[:, 0:384], zt_ps[0][:], AF.Copy,
                                     scale=rinv[:])
                nc.scalar.activation(zt_sb[:, 384:768], zt_ps[1][:], AF.Copy,
                                     scale=rinv[:])

            with tc.tile_pool(name="ps1a", bufs=1, space="PSUM") as ps1a:
                # mentions: mrep = x_m @ Wtr + b -> straight into h0b (bf16)
                mrep_p = ps1a.tile([EM, EMB], F32, tag="mrep", name="mrep")
                for kc in range(6):
                    nc.tensor.matmul(mrep_p[:], xmT[kc][:, 0:EM], wtr[kc][:],
                                     start=(kc == 0), stop=False)
                nc.tensor.matmul(mrep_p[:], onesrow[0:1, 0:EM], brow[:],
                                 start=False, stop=True)
                nc.scalar.activation(mrep_b[0:EM, 0:EMB], mrep_p[:], AF.Copy)
                nc.scalar.activation(expm[:], mrep_p[:], AF.Exp)
                # e_rep = ln(G2 @ exp(mrep))
                ep_p = ps1a.tile([E, EMB], F32, tag="ep", name="ep")
                nc.tensor.matmul(ep_p[:], g2T[:], expm[:], start=True, stop=True)
                nc.scalar.activation(eln_b[0:E, 0:EMB], ep_p[:], AF.Ln)
                # swap to the sigmoid table now (every later act is in it);
                # reading ep_p pins this after the Ln in the schedule
                nc.scalar.activation(actwarm[:], ep_p[0:1, 0:1], AF.Sigmoid)

                # w = colsum(attl) / 384
                for mc in range(4):
                    w_p = ps1a.tile([128, 1], F32, tag="w_p", name="w_p", bufs=1)
                    for kc in range(3):
                        nc.tensor.matmul(w_p[:],
                                         attl[kc][:, mc * 128:(mc + 1) * 128],
                                         onescol[:],
                                         start=(kc == 0), stop=(kc == 2))
                    nc.scalar.activation(wsb[mc][:], w_p[:], AF.Copy,
                                         scale=1.0 / (H * SPAN))
                fire("actr1")
                # spans: sp = x_span @ Wtr + b
                for mc in range(4):
                    if mc > 0:
                        fire(f"wstp0{mc - 1}" if mc < 3 else "wstp10")
                    sp_p = ps1a.tile([128, EMB], F32, tag="sp_p", name="sp_p",
                                     bufs=3)
                    for kc in range(6):
                        nc.tensor.matmul(sp_p[:],
                                         xspT[kc][:, mc * 128:(mc + 1) * 128],
                                         wtr[kc][:], start=(kc == 0), stop=False)
                    nc.tensor.matmul(sp_p[:], onesrow[:], brow[:],
                                     start=False, stop=True)
                    spc = pbig.tile([128, EMB], F32, tag="spc", name="spc",
                                    bufs=4)
                    nc.scalar.copy(spc[:], sp_p[:])
                    sp_ps.append(spc)
                # wsp = psum(sp) * w ; link = SUM^T @ wsp
                for mc in range(4):
                    nc.vector.tensor_scalar(out=wsp[mc][:], in0=sp_ps[mc][:],
                                            scalar1=wsb[mc][:], scalar2=None,
                                            op0=ALU.mult)
                link_p = ps1a.tile([L, EMB], F32, tag="link", name="link")
                for kc in range(4):
                    nc.tensor.matmul(link_p[:], sumT[kc][:], wsp[kc][:],
                                     start=(kc == 0), stop=(kc == 3))
                nc.scalar.activation(link_b[0:L, 0:EMB], link_p[:], AF.Copy)
                fire("wstp11")


        if stages >= 2:
          # ================= stage 2: RGCN (4 layers, y col-half pipelined) ==
          ecT = [pwork.tile([128, E], F32R, tag=f"ecT{i}", name=f"ecT{i}")
                 for i in range(4)]
          with tc.tile_pool(name="prg", bufs=2) as prg, \
               tc.tile_pool(name="psr", bufs=1, space="PSUM") as psr:
              h = None
              UW = (NREL + 1) * NN
              for layer in range(NLAYERS):
                  kcs = _KC0 if layer == 0 else _KC1
                  nk = len(kcs)
                  wstp = wstp_t[layer]
                  nrel_l = 2 if layer == 3 else NREL + 1
                  uw_l = 2 * E if layer == 3 else UW
                  wst_t = [wstp[:, (r * nk + si) * EMB:(r * nk + si + 1) * EMB]
                           for r in range(nrel_l) for si in range(nk)]
                  # u = h^T @ A_allT per d-chunk. Layer 3 only needs entity
                  # output rows, which only rel-0 and the self-loop feed.
                  u_sb = []
                  for si, (s0, sl) in enumerate(kcs):
                      u_p = psr.tile([128, (NREL + 1) * NN], F32, tag="u_p",
                                     name="u_p", bufs=2)
                      if layer == 0:
                          for pi, (pt, rows) in enumerate(h0p):
                              nc.tensor.matmul(u_p[0:sl, 0:UW],
                                               pt[0:rows, s0:s0 + sl],
                                               aallTb[0:rows,
                                                      (pi + 1) * UW:
                                                      (pi + 2) * UW],
                                               start=(pi == 0), stop=(pi == 2))
                      elif layer == 3:
                          nc.tensor.matmul(u_p[0:sl, 0:2 * E],
                                           h[0:NN, s0:s0 + sl],
                                           aallTb[0:NN, 4 * UW:4 * UW + 2 * E],
                                           start=True, stop=True)
                      else:
                          nc.tensor.matmul(u_p[0:sl, 0:UW],
                                           h[0:NN, s0:s0 + sl],
                                           aallTb[0:NN, 0:UW],
                                           start=True, stop=True)
                      u = prg.tile([128, (NREL + 1) * NN], BF16, tag=f"u{si}",
                                   name=f"u{si}")
                      if si % 2 == 0:
                          nc.scalar.copy(u[0:sl, 0:uw_l], u_p[0:sl, 0:uw_l])
                      else:
                          nc.vector.tensor_copy(out=u[0:sl, 0:uw_l],
                                                in_=u_p[0:sl, 0:uw_l])
                      u_sb.append(u)
                  # y = sum_r (u_r)^T @ Wst_r, in two column halves so the
                  # relu of half 0 overlaps the matmuls of half 1
                  if layer < 2:
                      fire(f"wstp{layer + 2}0", f"wstp{layer + 2}1")
                  elif layer == 2:
                      fire("sew")
                  # deferred e_ctx chain, one piece per inter-layer window
                  if layer == 0:
                      for kc in range(6):
                          ztp = psr.tile([128, E], F32, tag="tp22", name="ztp",
                                         bufs=2)
                          nc.tensor.transpose(ztp[:],
                                              zt_sb[:, kc * 128:(kc + 1) * 128],
                                              ident[0:E, 0:E])
                          if kc % 2 == 0:
                              nc.scalar.copy(z_sb[kc][:], ztp[:])
                          else:
                              nc.vector.tensor_copy(out=z_sb[kc][:], in_=ztp[:])
                  elif layer == 1:
                      ec2_p = psr.tile([E, EMB], F32, tag="ec2", name="ec2")
                      for kc in range(6):
                          nc.tensor.matmul(ec2_p[:], z_sb[kc][:], wtr[kc][:],
                                           start=(kc == 0), stop=False)
                      nc.tensor.matmul(ec2_p[:], easumT[:], brow[:],
                                       start=False, stop=True)
                      nc.scalar.copy(ec2_sb[:], ec2_p[:])
                  elif layer == 2:
                      for mc in range(4):
                          ecp = psr.tile([128, E], F32, tag="tp22", name="ecp",
                                         bufs=2)
                          nc.tensor.transpose(ecp[:],
                                              ec2_sb[:, mc * 128:(mc + 1) * 128],
                                              ident[0:E, 0:E])
                          if mc % 2 == 0:
                              nc.scalar.copy(ectxT_sb[mc][:], ecp[:])
                          else:
                              nc.vector.tensor_copy(out=ectxT_sb[mc][:],
                                                    in_=ecp[:])
                  nrows = E if layer == 3 else NN
                  rw = E if layer == 3 else NN
                  hn = prg.tile([NN, EMB], BF16, tag="h_next", name="h_next")
                  for yh in range(2):
                      y_p = psr.tile([NN, 256], F32, tag=f"y_p{yh}",
                                     name=f"y_p{yh}")
                      n_mm = nrel_l * nk
                      k_mm = 0
                      for si, (s0, sl) in enumerate(kcs):
                          for r in range(nrel_l):
                              nc.tensor.matmul(
                                  y_p[0:nrows, :],
                                  u_sb[si][0:sl, r * rw:r * rw + nrows],
                                  wst_t[r * nk + si][0:sl,
                                                     yh * 256:(yh + 1) * 256],
                                  start=(k_mm == 0), stop=(k_mm == n_mm - 1))
                              k_mm += 1
                      nc.scalar.activation(hn[0:nrows, yh * 256:(yh + 1) * 256],
                                           y_p[0:nrows, :], AF.Relu)
                  h = hn

              # entity_struT + e_ctxT -> ecT
              for mc in range(4):
                  tp = psr.tile([128, E], F32, tag="tp22", name="est", bufs=2)
                  nc.tensor.matmul(tp[:], h[0:E, mc * 128:(mc + 1) * 128],
                                   identb[0:E, 0:E], start=True, stop=True)
                  nc.vector.tensor_tensor(out=ecT[mc][:], in0=tp[:],
                                          in1=ectxT_sb[mc][:], op=ALU.add)

        prgw_cm.__exit__(None, None, None)

        if stages >= 3:
          # ================= stage 3: fmap + SE =================
          fmap = [pwork.tile([128, EE], F32R, tag=f"fmap{i}", name=f"fmap{i}")
                  for i in range(4)]
          pooled = [pwork.tile([128, 1], F32R, tag=f"pool{i}", name=f"pool{i}")
                    for i in range(4)]
          for mc in range(4):
              o6v = fmap[mc][:].rearrange("p (i j) -> p i j", i=E)
              in0 = ecT[mc][:].rearrange("p (i j) -> p i j", j=1) \
                  .to_broadcast([128, E, E])
              in1 = ecT[mc][:].rearrange("p (o j) -> p o j", o=1) \
                  .to_broadcast([128, E, E])
              nc.vector.tensor_tensor(out=o6v, in0=in0, in1=in1, op=ALU.mult)
              rs = pwork.tile([128, 1], F32, tag=f"rs{mc}", name=f"rs{mc}")
              nc.vector.tensor_reduce(rs[:], ecT[mc][:], mybir.AxisListType.X,
                                      ALU.add)
              nc.scalar.activation(pooled[mc][:], rs[:], AF.Square, scale=1.0 / E)

          with tc.tile_pool(name="pse", bufs=1, space="PSUM") as pse:
              # c-path first (tiny serial chain, hides under fmap/s1p);
              # seb2 is folded into the fcb2 host constant.
              c1_sb = [pwork.tile([128, 1], F32R, tag=f"c1_{i}", name=f"c1_{i}")
                       for i in range(2)]
              for oc in range(2):
                  c1_p = pse.tile([128, 1], F32, tag="cp", name="c1p", bufs=2)
                  for mc in range(4):
                      nc.tensor.matmul(c1_p[:],
                                       sew["fcw1T"][mc][:, oc * 128:(oc + 1) * 128]
                                       .bitcast(F32),
                                       pooled[mc][:].bitcast(F32),
                                       start=(mc == 0), stop=(mc == 3))
                  nc.scalar.activation(c1_sb[oc][:], c1_p[:], AF.Relu,
                                       bias=sev["fcb1"][oc][:],
                                       scale=sev["fcs1"][oc][:])
              cbb = [pwork.tile([128, 1], F32, tag=f"cbb{i}", name=f"cbb{i}")
                     for i in range(4)]
              # s-path; s2's first half starts as soon as s1_sb[0] is ready
              s1_sb = [pwork.tile([128, EE], F32R, tag=f"s1_{i}", name=f"s1_{i}")
                       for i in range(2)]
              for oc in range(2):
                  s1_p = pse.tile([128, EE], F32, tag="s1p", name="s1p", bufs=2)
                  for mc in range(4):
                      nc.tensor.matmul(s1_p[:],
                                       sew["fsw1T"][mc][:, oc * 128:(oc + 1) * 128],
                                       fmap[mc][:], start=(mc == 0), stop=(mc == 3))
                  nc.scalar.activation(s1_sb[oc][:], s1_p[:], AF.Relu,
                                       bias=sev["seb1"][oc][:],
                                       scale=sev["ses1"][oc][:])
              for mc in range(4):
                  c2_p = pse.tile([128, 1], F32, tag="cp", name="c2p", bufs=2)
                  for kc in range(2):
                      nc.tensor.matmul(c2_p[:],
                                       sew["fcw2T"][kc][:, mc * 128:(mc + 1) * 128]
                                       .bitcast(F32),
                                       c1_sb[kc][:].bitcast(F32),
                                       start=(kc == 0), stop=(kc == 1))
                  nc.scalar.activation(cbb[mc][:], c2_p[:], AF.Identity,
                                       bias=sev["fcb2"][mc][:],
                                       scale=sev["fcs2"][mc][:])
              s2_ps = [pse.tile([128, EE], F32, tag="s2p", name=f"s2p{mc}",
                                bufs=3) for mc in range(4)]
              for mc in range(4):
                  nc.tensor.matmul(s2_ps[mc][:],
                                   sew["fsw2T"][0][:, mc * 128:(mc + 1) * 128],
                                   s1_sb[0][:], start=True, stop=False)
              for mc in range(4):
                  nc.tensor.matmul(s2_ps[mc][:],
                                   sew["fsw2T"][1][:, mc * 128:(mc + 1) * 128],
                                   s1_sb[1][:], start=False, stop=True)
                  sig = pwork.tile([128, EE], F32, tag="sig", name="sig", bufs=2)
                  nc.scalar.activation(sig[:], s2_ps[mc][:], AF.Sigmoid,
                                       bias=cbb[mc][:], scale=sev["ses2"][mc][:])
                  outv = fusedp[mc][:].rearrange("p (i j) -> p i j", j=26)[:, 2:24,
                                                                          2:24]
                  nc.vector.tensor_tensor(
                      out=outv,
                      in0=fmap[mc][:].rearrange("p (i j) -> p i j", i=E),
                      in1=sig[:].rearrange("p (i j) -> p i j", i=E),
                      op=ALU.mult)

        if stages >= 4:
          # ================= stage 4: conv stack =================
          def tap_rows(padt, tap, r0, nr):
              dy, dx = tap // 5, tap % 5
              return padt[:].rearrange("p (i j) -> p i j", j=26)[
                  :, dy + r0:dy + r0 + nr, dx:dx + 22]

          def tap_view(padt, tap):
              return tap_rows(padt, tap, 0, 22)

          with tc.tile_pool(name="pcw", bufs=1) as pcw, \
               tc.tile_pool(name="psc", bufs=1, space="PSUM") as psc:
              w1, w2, w3 = [], [], []
              for kc in range(4):
                  t = pcw.tile([128, 25 * 128], BF16, tag=f"w1_{kc}",
                               name=f"w1_{kc}")
                  nc.sync.dma_start(t[:], w1sb_d[kc])
                  w1.append(t)
              for kc in range(2):
                  t = pcw.tile([128, 25 * 128], BF16, tag=f"w2_{kc}",
                               name=f"w2_{kc}")
                  nc.sync.dma_start(t[:], w2sb_d[kc])
                  w2.append(t)
              for kc in range(2):
                  t = pcw.tile([128, 25 * 256], BF16, tag=f"w3_{kc}",
                               name=f"w3_{kc}")
                  nc.sync.dma_start(t[:], w3sb_d[kc])
                  w3.append(t)
              mtop = cf("mtop")
              mbot = cf("mbot")

              def interior_rows(t_, r0, nr):
                  return t_[:].rearrange("p (i j) -> p i j", j=26)[
                      :, 2 + r0:2 + r0 + nr, 2:24]

              def gather_combine(src_flat, dst_pad, gg, rh, nm, rows=None):
                  """relu'd flat rows rh -> allgather -> mask-combine into
                  dst rows rh, issued right after the producing relu."""
                  r0, nr = rows
                  rb = pdram.tile([128, nr * 22], BF16, tag=f"{nm}b{rh}",
                                  name=f"{nm}b{rh}")
                  nc.sync.dma_start(rb[:], src_flat[:])
                  if solo:
                      nc.sync.dma_start(gg[0:128, :], rb[:])
                      nc.gpsimd.dma_start(gg[128:256, :], rb[:])
                  else:
                      nc.gpsimd.collective_compute(
                          "AllGather", ALU.bypass, replica_groups=groups,
                          ins=[rb[:].opt()], outs=[gg[:].opt()])
                  gt = pcw.tile([128, nr * 22], BF16, tag=f"{nm}gt{rh}",
                                name=f"{nm}gt{rh}")
                  gb = pcw.tile([128, nr * 22], BF16, tag=f"{nm}gb{rh}",
                                name=f"{nm}gb{rh}")
                  nc.sync.dma_start(gt[:], gg[0:128, :])
                  nc.gpsimd.dma_start(gb[:], gg[128:256, :])
                  tmp = pcw.tile([128, nr * 22], F32, tag=f"{nm}tmp{rh}",
                                 name=f"{nm}tmp{rh}")
                  nc.vector.tensor_scalar(out=tmp[:], in0=gb[:],
                                          scalar1=mbot[:], scalar2=None,
                                          op0=ALU.mult)
                  nc.vector.scalar_tensor_tensor(
                      out=interior_rows(dst_pad, r0, nr), in0=gt[:],
                      scalar=mtop[:], in1=tmp[:], op0=ALU.mult, op1=ALU.add)

              # conv1 rows (0,18),(18,4): the tiny second half's gather
              # launches right at conv1's end, so its readback chain hides
              # under conv2's first-half taps.
              ROWS = ((0, 16), (16, 6))

              # conv1 (my half of 256 out channels), in two row halves; each
              # half's AllGather + readback starts as soon as the half is done
              r1g = [pdram.tile([256, nr * 22], BF16, tag=f"r1g{i}",
                                name=f"r1g{i}")
                     for i, (r0, nr) in enumerate(ROWS)]
              for rh, (r0, nr) in enumerate(ROWS):
                  r1_p = psc.tile([128, nr * 22], F32, tag="convp",
                                  name="convp", bufs=4)
                  first = True
                  for kc in range(4):
                      for tap in range(25):
                          nc.tensor.matmul(r1_p[:],
                                           w1[kc][:, tap * 128:(tap + 1) * 128],
                                           tap_rows(fusedp[kc], tap, r0, nr),
                                           start=first,
                                           stop=(kc == 3 and tap == 24))
                          first = False
                  o1f = pcw.tile([128, nr * 22], BF16, tag=f"o1f{rh}",
                                 name=f"o1f{rh}")
                  nc.scalar.activation(o1f[:], r1_p[:], AF.Relu, bias=b1h[:])
                  nc.vector.tensor_copy(out=interior_rows(own1, r0, nr),
                                        in_=o1f[:].rearrange(
                                            "p (i j) -> p i j", j=22))
                  gather_combine(o1f, oth1, r1g[rh], rh, "r1", (r0, nr))

              # conv2: own-input taps for both row-halves first (no gather
              # dependency), then other-input taps; output in row halves.
              # conv2 out rows (0-8, 9-21): the first half's own AND other
              # taps read only rows <=12 of r1, i.e. conv1-half0 + gather#0 —
              # no wait on conv1-half1's gather.
              ROWS2 = ((0, 12), (12, 10))
              r2g = [pdram.tile([256, nr * 22], BF16, tag=f"r2g{i}",
                                name=f"r2g{i}")
                     for i, (r0, nr) in enumerate(ROWS2)]
              r2_ps = [psc.tile([128, nr * 22], F32, tag=f"convp2_{i}",
                                name=f"convp2_{i}")
                       for i, (r0, nr) in enumerate(ROWS2)]
              # rh1's own-taps first (ready at conv1 end) to widen the
              # window that hides gather#1's readback chain; rh0 is fully
              # ready (gather#0 landed mid-conv1) and runs next.
              r10, n1 = ROWS2[1]
              for tap in range(25):
                  nc.tensor.matmul(r2_ps[1][:],
                                   w2[0][:, tap * 128:(tap + 1) * 128],
                                   tap_rows(own1, tap, r10, n1),
                                   start=(tap == 0), stop=False)
              r0, nr = ROWS2[0]
              for tap in range(25):
                  nc.tensor.matmul(r2_ps[0][:],
                                   w2[0][:, tap * 128:(tap + 1) * 128],
                                   tap_rows(own1, tap, r0, nr),
                                   start=(tap == 0), stop=False)
              for tap in range(25):
                  nc.tensor.matmul(r2_ps[0][:],
                                   w2[1][:, tap * 128:(tap + 1) * 128],
                                   tap_rows(oth1, tap, r0, nr),
                                   start=False, stop=(tap == 24))
              o2f0 = pcw.tile([128, nr * 22], BF16, tag="o2f0", name="o2f0")
              nc.scalar.activation(o2f0[:], r2_ps[0][:], AF.Relu, bias=b2h[:])
              nc.vector.tensor_copy(out=interior_rows(own2, r0, nr),
                                    in_=o2f0[:].rearrange(
                                        "p (i j) -> p i j", j=22))
              gather_combine(o2f0, oth2, r2g[0], 0, "r2", (r0, nr))
              for tap in range(25):
                  nc.tensor.matmul(r2_ps[1][:],
                                   w2[1][:, tap * 128:(tap + 1) * 128],
                                   tap_rows(oth1, tap, r10, n1),
                                   start=False, stop=(tap == 24))
              o2f1 = pcw.tile([128, n1 * 22], BF16, tag="o2f1", name="o2f1")
              nc.scalar.activation(o2f1[:], r2_ps[1][:], AF.Relu, bias=b2h[:])
              nc.vector.tensor_copy(out=interior_rows(own2, r10, n1),
                                    in_=o2f1[:].rearrange(
                                        "p (i j) -> p i j", j=22))
              gather_combine(o2f1, oth2, r2g[1], 1, "r2", (r10, n1))

              # conv3 (my 256 of 512 out channels): own-input taps for both
              # out chunks first, then other-input taps per (oc, row-half)
              # with relu+store pipelined per row half (short tail).
              r3_ps = [psc.tile([128, ROWS[rh][1] * 22], F32, tag="convp",
                                name=f"convp3_{oc}{rh}", bufs=4)
                       for oc in range(2) for rh in range(2)]
              for oc in range(2):
                  for rh, (r0, nr) in enumerate(ROWS):
                      for tap in range(25):
                          nc.tensor.matmul(
                              r3_ps[oc * 2 + rh][:],
                              w3[0][:, tap * 256 + oc * 128:
                                    tap * 256 + (oc + 1) * 128],
                              tap_rows(own2, tap, r0, nr),
                              start=(tap == 0), stop=False)
              for oc in range(2):
                  for rh, (r0, nr) in enumerate(ROWS):
                      for tap in range(25):
                          nc.tensor.matmul(
                              r3_ps[oc * 2 + rh][:],
                              w3[1][:, tap * 256 + oc * 128:
                                    tap * 256 + (oc + 1) * 128],
                              tap_rows(oth2, tap, r0, nr),
                              start=False, stop=(tap == 24))
                      o_sb = pcw.tile([128, nr * 22], F32, tag="osb",
                                      name="osb", bufs=4)
                      nc.scalar.activation(o_sb[:], r3_ps[oc * 2 + rh][:],
                                           AF.Relu, bias=b3h[oc][:])
                      nc.sync.dma_start(
                          out_d[oc * 128:(oc + 1) * 128,
                                r0 * 22:(r0 + nr) * 22], o_sb[:])

    nc.compile()
    return nc


_NC_CACHE = None


def _get_program():
    global _NC_CACHE
    if _NC_CACHE is None:
        _NC_CACHE = build_program()
    return _NC_CACHE


def _prep_shared(w):
    """Packed weights/constants identical on every core."""
    ADJ = _build_adj()
    out = {}
    constr = np.zeros((128, _CR), np.float32)

    def put(nm, arr):
        c0, cols = _LAY_R[nm]
        r, cc = arr.shape
        constr[0:r, c0:c0 + cc] = arr
    g2T = np.zeros((EM, E), np.float32)
    for e in range(E):
        g2T[e * M:(e + 1) * M, e] = 1.0
    put("g2T", g2T)
    sumT = np.kron(np.eye(L, dtype=np.float32), np.ones((SPAN, 1), np.float32))
    for kc in range(4):
        put(f"sumT{kc}", sumT[kc * 128:(kc + 1) * 128])
    for nm, arr, nch in (("fsw1T", w['fs_w1'].T, 4), ("fcw1T", w['fc_w1'].T, 4),
                         ("fsw2T", w['fs_w2'].T, 2), ("fcw2T", w['fc_w2'].T, 2)):
        for kc in range(nch):
            put(f"{nm}{kc}", np.ascontiguousarray(arr[kc * 128:(kc + 1) * 128]))
    out['constr'] = constr

    constw = np.zeros((128, _CW), np.float32)
    wt = w['W_trans']
    for kc in range(6):
        constw[:, kc * EMB:(kc + 1) * EMB] = wt[kc * 128:(kc + 1) * 128]
    constw[0, 6 * EMB:7 * EMB] = w['b_trans']
    out['constw'] = constw.astype(ml_dtypes.bfloat16)

    gTb = np.zeros((128, 9 * E), np.float32)
    gT = np.zeros((EMH, E), np.float32)
    for e in range(E):
        gT[e * M * H:(e + 1) * M * H, e] = 1.0 / (M * H)
    for kc in range(9):
        r = min(128, EMH - kc * 128)
        gTb[0:r, kc * E:(kc + 1) * E] = gT[kc * 128:kc * 128 + r]
    out['gTb'] = gTb.astype(ml_dtypes.bfloat16)
    aall = np.concatenate(
        [ADJ[r].T for r in range(NREL)] + [np.eye(NN, dtype=np.float32)],
        axis=1)
    UW = (NREL + 1) * NN
    aallp = np.zeros((128, 4 * UW + 2 * E), np.float32)
    aallp[0:NN, 0:UW] = aall
    aallp[0:EM, UW:2 * UW] = aall[E:E + EM]
    aallp[0:L, 2 * UW:3 * UW] = aall[E + EM:NN]
    aallp[0:E, 3 * UW:4 * UW] = aall[0:E]
    # layer-3 entity-only columns: [A0^T[:, :E] | I[:, :E]]
    aallp[0:NN, 4 * UW:4 * UW + E] = aall[:, 0:E]
    aallp[0:NN, 4 * UW + E:4 * UW + 2 * E] = aall[:, NREL * NN:NREL * NN + E]
    out['aallTb'] = aallp.astype(ml_dtypes.bfloat16)
    out['tfb'] = np.ascontiguousarray(
        w['type_embed'][_TYPES]).astype(ml_dtypes.bfloat16)
    out['identb'] = np.eye(128, dtype=np.float32).astype(ml_dtypes.bfloat16)

    constf = np.zeros((128, _CF), np.float32)

    def putf(nm, arr):
        c0, cols = _LAY_F[nm]
        constf[0:arr.shape[0], c0:c0 + 1] = arr.reshape(-1, 1)
    vecs = {"ses1": w['fs_g1'], "seb1": w['fs_b1'] * w['fs_g1'] + w['fs_be1'],
            "fcs1": w['fc_g1'], "fcb1": w['fc_b1'] * w['fc_g1'] + w['fc_be1'],
            "ses2": w['fs_g2'], "seb2": w['fs_b2'] * w['fs_g2'] + w['fs_be2'],
            "fcs2": w['fc_g2'],
            "fcb2": (w['fc_b2'] * w['fc_g2'] + w['fc_be2'] +
                     w['fs_b2'] * w['fs_g2'] + w['fs_be2'])}
    for nm, v in vecs.items():
        nch = 2 if v.shape[0] == INTER else 4
        for kc in range(nch):
            putf(f"{nm}{kc}", v[kc * 128:(kc + 1) * 128])
    out['constf_base'] = constf

    for layer in range(NLAYERS):
        din_l = D0 if layer == 0 else EMB
        kcs = _KC0 if layer == 0 else _KC1
        nk = len(kcs)
        Wst = w['rgcn_Wrel0'].reshape(NREL * D0, EMB) if layer == 0 else \
            w['rgcn_Wrel'][layer - 1].reshape(NREL * EMB, EMB)
        Wself = w['rgcn_Wself0'] if layer == 0 else w['rgcn_Wself'][layer - 1]
        if layer == 3:
            # entity rows only need rel-0 and the self-loop
            p = np.zeros((128, 2 * nk * EMB), np.float32)
            for si, (s0, sl) in enumerate(kcs):
                p[0:sl, si * EMB:(si + 1) * EMB] = Wst[s0:s0 + sl]
                p[0:sl, (nk + si) * EMB:(nk + si + 1) * EMB] = \
                    Wself[s0:s0 + sl]
        else:
            p = np.zeros((128, (NREL + 1) * nk * EMB), np.float32)
            for r in range(NREL):
                for si, (s0, sl) in enumerate(kcs):
                    p[0:sl, (r * nk + si) * EMB:(r * nk + si + 1) * EMB] = \
                        Wst[r * din_l + s0:r * din_l + s0 + sl]
            for si, (s0, sl) in enumerate(kcs):
                p[0:sl, (NREL * nk + si) * EMB:(NREL * nk + si + 1) * EMB] = \
                    Wself[s0:s0 + sl]
        out[f'wstp{layer}'] = p.astype(ml_dtypes.bfloat16)
    return out


def _prep_conv_half(w, half, constf_base):
    out = {}
    w1 = w['cr_w1'][half * 128:(half + 1) * 128]
    out['w1sb'] = np.ascontiguousarray(
        w1.transpose(1, 2, 3, 0).reshape(4, 128, 25 * 128)).astype(ml_dtypes.bfloat16)
    # conv2/conv3 weights in (own-input-half, other-input-half) chunk order
    w2 = w['cr_w2'][half * 128:(half + 1) * 128]
    w2p = w2.transpose(1, 2, 3, 0).reshape(2, 128, 25 * 128)
    order = [half, 1 - half]
    out['w2sb'] = np.ascontiguousarray(w2p[order]).astype(ml_dtypes.bfloat16)
    w3 = w['cr_w3'][half * 256:(half + 1) * 256]
    w3p = w3.transpose(1, 2, 3, 0).reshape(2, 128, 25 * 256)
    out['w3sb'] = np.ascontiguousarray(w3p[order]).astype(ml_dtypes.bfloat16)
    constf = constf_base.copy()

    def putf(nm, arr):
        c0, cols = _LAY_F[nm]
        constf[0:arr.shape[0], c0:c0 + 1] = arr.reshape(-1, 1)
    putf("b1h", w['cr_b1'][half * 128:(half + 1) * 128])
    putf("b2h", w['cr_b2'][half * 128:(half + 1) * 128])
    putf("b3h0", w['cr_b3'][half * 256:half * 256 + 128])
    putf("b3h1", w['cr_b3'][half * 256 + 128:half * 256 + 256])
    putf("mtop", np.full(128, float(half), np.float32))
    putf("mbot", np.full(128, float(1 - half), np.float32))
    c0, cols = _LAY_F["identf"]
    constf[:, c0:c0 + 128] = np.eye(128, dtype=np.float32)
    out['constf'] = constf
    return out


def _prep_doc(x, att, mi, ls):
    out = {}
    mif = mi.reshape(EM)
    attm = np.ascontiguousarray(
        att[:, mif, :].transpose(1, 0, 2).reshape(EMH, C))
    amp = np.zeros((128, 9 * C), np.float32)
    for kc in range(9):
        r = min(128, EMH - kc * 128)
        amp[0:r, kc * C:kc * C + C] = attm[kc * 128:kc * 128 + r]
    out['amp'] = amp.astype(ml_dtypes.bfloat16)
    idx = ls[:, None] + np.arange(SPAN)
    idxf = idx.reshape(LS)
    rows = att[:, idxf, :].reshape(H, L, SPAN, C)
    blocks = np.take_along_axis(rows, idx[None, :, None, :], axis=3)
    attl = blocks.transpose(0, 2, 1, 3).reshape(HS, LS)
    xmT = x[mif].T
    xspT = x[idxf].T
    actr = np.zeros((128, _CA), np.float32)

    def put(nm, arr):
        c0, cols = _LAY_A[nm]
        actr[0:arr.shape[0], c0:c0 + arr.shape[1]] = arr
    for kc in range(6):
        put(f"xmT{kc}", xmT[kc * 128:(kc + 1) * 128])
        put(f"xspT{kc}", xspT[kc * 128:(kc + 1) * 128])
    for kc in range(3):
        put(f"attl{kc}", attl[kc * 128:(kc + 1) * 128])
    out['actr'] = actr.astype(ml_dtypes.bfloat16)
    xpk = np.zeros((128, 8 * HID), np.float32)
    for kc in range(8):
        xpk[:, kc * HID:(kc + 1) * HID] = x[kc * 128:(kc + 1) * 128]
    out['xp'] = xpk.astype(ml_dtypes.bfloat16)
    return out


def build_in_maps(inputs):
    w = {}
    for k, v in inputs.items():
        a = np.asarray(v)
        w[k] = a if a.dtype in (np.int32, np.int64) else \
            np.asarray(a, np.float32)
    shared = _prep_shared(w)
    constf_base = shared.pop('constf_base')
    halves = [_prep_conv_half(w, h, constf_base) for h in range(2)]
    seq = np.asarray(inputs['sequence_output'], np.float32)
    att = np.asarray(inputs['attention'], np.float32)
    mi = np.asarray(inputs['mention_idx']).astype(np.int64)
    ls = np.asarray(inputs['link_start']).astype(np.int64)
    docs = [_prep_doc(seq[n], att[n], mi[n], ls[n]) for n in range(NB)]
    in_maps = []
    for core in range(N_CORES):
        n, half = core // 2, core % 2
        m = dict(shared)
        m.update(halves[half])
        m.update(docs[n])
        in_maps.append({k: (np.ascontiguousarray(v) if v.dtype == ml_dtypes.bfloat16
                            else np.ascontiguousarray(v, np.float32))
                        for k, v in m.items()})
    return in_maps


def kernel(**inputs):
    nc = _get_program()
    in_maps = build_in_maps(inputs)
    res = run_bass_kernel_spmd(nc, in_maps, list(range(N_CORES)))
    out = np.zeros((NB, EMB, E, E), np.float32)
    for core in range(N_CORES):
        n, half = core // 2, core % 2
        out[n, half * 256:(half + 1) * 256] = \
            res.results[core]["out"].reshape(256, E, E)
    return out


# revision 35
# speedup vs baseline: 1.0212x; 1.0128x over previous
"""Trainium2 Bass kernel for nn_DocREModel (DocRE: gather -> RGCN -> SE -> 5x5 convs).

Sharding: 4 documents x 2 cores each. Each pair replicates the cheap upstream
(mention/link/ea gathers -> RGCN -> fmap/SE) and splits the dominant 5x5 conv
stack by output channels, with intra-pair AllGathers; output halves are
assembled on host. All index-driven gathers happen on host (pure data
movement; one SPMD program serves all 8 cores), all dense math on device.

Perf notes (v3):
- DMA is a serialized ~360B/ns resource in the cost model; all input loads
  go on ONE queue (sync/SP) in exact consumption order, in <=~800KB pieces,
  so arrival order is deterministic and matches the compute schedule.
- The PE p-state ramp resets on >~2us idle gaps. Tiny "touch" matmuls that
  read one column of each arriving DMA piece pulse the PE at piece cadence,
  holding the fast clock through DMA-bound phases.
- bf16 for W_trans, gathered activations, RGCN + conv weights (f32 PSUM
  accumulation everywhere); f32r for the remaining f32 path.
- Convs are 25 shift-tap matmuls over zero-padded 26x26 images via strided
  APs. conv1/conv2 outputs are produced in two 11-row halves so each half's
  AllGather (+ readback + mask-combine) overlaps the next half's matmuls;
  conv3 output is relu'd + stored per (out-chunk, row-half) to shrink the
  kernel tail. RGCN folds the self-loop in as a 4th identity relation and
  pipelines each layer by y column halves.
"""

import numpy as np
import ml_dtypes

import concourse.bacc as bacc
import concourse.tile as tile
from concourse import mybir
from concourse.bass_utils import run_bass_kernel_spmd

F32 = mybir.dt.float32
F32R = mybir.dt.float32r
BF16 = mybir.dt.bfloat16
AF = mybir.ActivationFunctionType
ALU = mybir.AluOpType

NB, H, C, HID, EMB = 4, 12, 1024, 768, 512
E, M, L, SPAN = 22, 4, 16, 32
TD, INTER = 20, 256
NN = E + E * M + L
NREL, NLAYERS = 3, 4
EM, EMH, HS, LS = E * M, E * M * H, H * SPAN, L * SPAN
D0 = EMB + TD           # 532
EE = E * E              # 484
PADW = 26 * 26          # 676 padded 26x26 image
N_CORES = 8


def _build_adj():
    A = np.zeros((NREL, NN, NN), np.float32)
    for e in range(E):
        for m in range(M):
            mi = E + e * M + m
            A[0, e, mi] = A[0, mi, e] = 1.0
            for m2 in range(M):
                if m2 != m:
                    A[1, mi, E + e * M + m2] = 1.0
            li = E + E * M + ((e * M + m) % L)
            A[2, mi, li] = A[2, li, mi] = 1.0
    A = A / (A.sum(-1, keepdims=True) + 1e-5)
    return A


_TYPES = np.concatenate([np.zeros(E, np.int32), np.ones(EM, np.int32),
                         np.full(L, 2, np.int32)])

_KC0 = [(0, 128), (128, 128), (256, 128), (384, 128), (512, 20)]   # 532 rows
_KC1 = [(0, 128), (128, 128), (256, 128), (384, 128)]              # 512 rows


def _const_layout():
    """f32r constants [128, CR]: stage-1 smalls + SE weights."""
    lay = {}
    c = 0

    def add(nm, cols):
        nonlocal c
        lay[nm] = (c, cols)
        c += cols
    add("g2T", E)
    for kc in range(4):
        add(f"sumT{kc}", L)
    for kc in range(4):
        add(f"fsw1T{kc}", INTER)
    for kc in range(4):
        add(f"fcw1T{kc}", INTER)
    for kc in range(2):
        add(f"fsw2T{kc}", EMB)
    for kc in range(2):
        add(f"fcw2T{kc}", EMB)
    return lay, c


def _constf_layout():
    lay = {}
    c = 0

    def add(nm, cols):
        nonlocal c
        lay[nm] = (c, cols)
        c += cols
    for nm, nch in (("ses1", 2), ("seb1", 2), ("fcs1", 2), ("fcb1", 2),
                    ("ses2", 4), ("seb2", 4), ("fcs2", 4), ("fcb2", 4)):
        for kc in range(nch):
            add(f"{nm}{kc}", 1)
    add("b1h", 1)
    add("b2h", 1)
    add("b3h0", 1)
    add("b3h1", 1)
    add("mtop", 1)
    add("mbot", 1)
    add("identf", 128)
    return lay, c


def _actr_layout():
    """bf16 gathered activations [128, CA]."""
    lay = {}
    c = 0

    def add(nm, cols):
        nonlocal c
        lay[nm] = (c, cols)
        c += cols
    for kc in range(6):
        add(f"xmT{kc}", EM)
    for kc in range(3):
        add(f"attl{kc}", LS)
    for kc in range(6):
        add(f"xspT{kc}", LS)
    return lay, c


_LAY_R, _CR = _const_layout()
_LAY_F, _CF = _constf_layout()
_LAY_A, _CA = _actr_layout()

_CRB = _LAY_R["fsw1T0"][0]          # stage-1 smalls | SE weights split
_CAA = _LAY_A["xspT0"][0]           # xmT+attl | xspT split
_CW = 7 * EMB                       # constrw (bf16): wtr chunks + brow block


def build_program(solo=False, stages=4):
    nc = bacc.Bacc("TRN2", target_bir_lowering=False, debug=False)

    def din(name, shape, dt=F32R):
        return nc.dram_tensor(name, list(shape), dt, kind="ExternalInput").ap()

    constr_d = din("constr", [128, _CR])
    constf_d = din("constf", [128, _CF], F32)
    constw_d = din("constw", [128, _CW], BF16)
    actr_d = din("actr", [128, _CA], BF16)
    xp_d = din("xp", [128, 8 * HID], BF16)
    amp_d = din("amp", [128, 9 * C], BF16)
    gTb_d = din("gTb", [128, 9 * E], BF16)
    wstp_d = [din("wstp0", [128, 20 * EMB], BF16)] + \
             [din(f"wstp{i}", [128, 16 * EMB], BF16) for i in (1, 2)] + \
             [din("wstp3", [128, 8 * EMB], BF16)]
    w1sb_d = din("w1sb", [4, 128, 25 * 128], BF16)
    w2sb_d = din("w2sb", [2, 128, 25 * 128], BF16)
    w3sb_d = din("w3sb", [2, 128, 25 * 256], BF16)
    aallTb_d = din("aallTb", [128, 4 * (NREL + 1) * NN + 2 * E], BF16)
    tfb_d = din("tfb", [NN, TD], BF16)
    identb_d = din("identb", [128, 128], BF16)

    out_d = nc.dram_tensor("out", [256, EE], F32, kind="ExternalOutput").ap()

    groups = [[0, 1], [2, 3], [4, 5], [6, 7]]

    with tile.TileContext(nc) as tc:
      with tc.tile_pool(name="pconst", bufs=1) as pconst, \
           tc.tile_pool(name="pwork", bufs=1) as pwork, \
           tc.tile_pool(name="pwarm", bufs=1, space="PSUM") as pwarm, \
           tc.tile_pool(name="pdram", bufs=1, space="DRAM") as pdram:
        prgw_cm = tc.tile_pool(name="prgw", bufs=1)
        prgw = prgw_cm.__enter__()

        constr = pconst.tile([128, _CR], F32R)
        constf = pconst.tile([128, _CF], F32)
        constw = pconst.tile([128, _CW], BF16)
        identb = pconst.tile([128, 128], BF16)
        aallTb = pconst.tile([128, 4 * (NREL + 1) * NN + 2 * E], BF16)
        onesb = pconst.tile([128, 128], BF16)
        nc.vector.memset(onesb[:], 1.0)
        # scratch for the sigmoid-table swap dummy (see ep below)
        actwarm = pconst.tile([1, 1], F32)

        def cr(nm, rows=128):
            c0, cols = _LAY_R[nm]
            return constr[0:rows, c0:c0 + cols]

        def cf(nm, rows=128):
            c0, cols = _LAY_F[nm]
            return constf[0:rows, c0:c0 + cols]

        wtr = [constw[:, kc * EMB:(kc + 1) * EMB] for kc in range(6)]
        brow = constw[0:1, 6 * EMB:7 * EMB]
        onescol = onesb[:, 0:1]
        onesrow = onesb[0:1, :]
        g2T = cr("g2T", rows=EM)
        sumT = [cr(f"sumT{kc}") for kc in range(4)]
        sew = {nm: [cr(f"{nm}{kc}") for kc in range(n)]
               for nm, n in (("fsw1T", 4), ("fcw1T", 4), ("fsw2T", 2),
                             ("fcw2T", 2))}
        sev = {nm: [cf(f"{nm}{kc}") for kc in range(n)]
               for nm, n in (("ses1", 2), ("seb1", 2), ("fcs1", 2), ("fcb1", 2),
                             ("ses2", 4), ("seb2", 4), ("fcs2", 4),
                             ("fcb2", 4))}
        b1h = cf("b1h")
        b2h = cf("b2h")
        b3h = [cf("b3h0"), cf("b3h1")]
        ident = cf("identf")

        # conv pad images: memset early while the DVE is idle
        fusedp = [pwork.tile([128, PADW], BF16, tag=f"fusedp{i}",
                             name=f"fusedp{i}") for i in range(4)]
        for t_ in fusedp:
            nc.vector.memset(t_[:], 0.0)
        own1 = pwork.tile([128, PADW], BF16)
        oth1 = pwork.tile([128, PADW], BF16)
        own2 = pwork.tile([128, PADW], BF16)
        oth2 = pwork.tile([128, PADW], BF16)
        for t_ in (own1, oth1, own2, oth2):
            nc.vector.memset(t_[:], 0.0)

        # persistent intermediates: h0 kept as three partition-0 node-type
        # parts (entities / mentions / links); RGCN layer 0 contracts over
        # the parts with a 3-part adjacency tensor.
        eln_b = pwork.tile([E, D0], BF16)
        mrep_b = pwork.tile([EM, D0], BF16)
        link_b = pwork.tile([L, D0], BF16)
        h0p = [(mrep_b, EM), (link_b, L), (eln_b, E)]
        ectxT_sb = [pwork.tile([128, E], F32, tag=f"ectxT{i}", name=f"ectxT{i}")
                    for i in range(4)]
        z_sb = [pwork.tile([128, E], BF16, tag=f"z{i}", name=f"z{i}")
                for i in range(6)]
        easumT = pwork.tile([1, E], BF16)
        zt_sb = pwork.tile([E, HID], F32)
        ec2_sb = pwork.tile([E, EMB], F32)

        # ---------------- ordered DMA supply script (sync/SP queue) ---------
        # Exact consumption order, <=~800KB pieces. "touch" matmuls (fired at
        # chosen program points) read one column of an arriving DMA piece and
        # pulse the PE, so the p-state ramp never resets during DMA-bound
        # stretches.
        warm = pwarm.tile([1, 1], F32)
        tch = {}

        def fire(*names):
            for nm in names:
                nc.tensor.matmul(warm[:], tch[nm], tch[nm],
                                 start=True, stop=True)

        gTbt = prgw.tile([128, 9 * E], BF16)
        nc.scalar.dma_start(gTbt[:], gTb_d[:])
        amp = prgw.tile([128, 9 * C], BF16)
        nc.sync.dma_start(amp[:, 0:C], amp_d[:, 0:C])
        nc.sync.dma_start(constf[:], constf_d[:])
        for (p0, pn) in ((1, 2), (3, 2), (5, 2), (7, 2)):
            nc.sync.dma_start(amp[:, p0 * C:(p0 + pn) * C],
                              amp_d[:, p0 * C:(p0 + pn) * C])
        xp = prgw.tile([128, 8 * HID], BF16)
        for i, p0 in enumerate(range(0, 8, 2)):
            nc.sync.dma_start(xp[:, p0 * HID:(p0 + 2) * HID],
                              xp_d[:, p0 * HID:(p0 + 2) * HID])
            tch[f"xp{i}"] = xp[:, p0 * HID:p0 * HID + 1]
        for i, (p0, pn) in enumerate(((0, 3), (3, 4))):   # wtr + brow block
            nc.sync.dma_start(constw[:, p0 * EMB:(p0 + pn) * EMB],
                              constw_d[:, p0 * EMB:(p0 + pn) * EMB])
            tch[f"wtr{i}"] = constw[:, p0 * EMB:p0 * EMB + 1]
        nc.sync.dma_start(constr[:, 0:_CRB], constr_d[:, 0:_CRB])
        actr = prgw.tile([128, _CA], BF16)
        nc.sync.dma_start(actr[:, 0:_CAA], actr_d[:, 0:_CAA])
        tch["actr0"] = actr[:, 0:1]
        nc.sync.dma_start(actr[:, _CAA:_CA], actr_d[:, _CAA:_CA])
        tch["actr1"] = actr[:, _CAA:_CAA + 1]
        # RGCN weights
        wstp_t = [prgw.tile([128, 20 * EMB], BF16, tag="wstp0",
                            name="wstp0")] + \
                 [prgw.tile([128, 16 * EMB], BF16, tag=f"wstp{i}",
                            name=f"wstp{i}") for i in (1, 2)] + \
                 [prgw.tile([128, 8 * EMB], BF16, tag="wstp3", name="wstp3")]
        for layer in range(NLAYERS):
            ncols = (20 * EMB if layer == 0 else
                     16 * EMB if layer < 3 else 8 * EMB)
            hh = ncols // 2
            for i, (p0, pn) in enumerate(((0, hh), (hh, ncols - hh))):
                nc.sync.dma_start(wstp_t[layer][:, p0:p0 + pn],
                                  wstp_d[layer][:, p0:p0 + pn])
                tch[f"wstp{layer}{i}"] = wstp_t[layer][:, p0:p0 + 1]
        # SE weights
        nc.sync.dma_start(constr[:, _CRB:_CR], constr_d[:, _CRB:_CR])
        tch["sew"] = constr[:, _CRB:_CRB + 1].bitcast(F32)


        def ca(nm, rows=128):
            c0, cols = _LAY_A[nm]
            return actr[0:rows, c0:c0 + cols]

        xmT = [ca(f"xmT{kc}") for kc in range(6)]
        xspT = [ca(f"xspT{kc}") for kc in range(6)]
        attl = [ca(f"attl{kc}") for kc in range(3)]

        def fire_touches():
            for ap_sl in touches:
                nc.tensor.matmul(warm[:], ap_sl, ap_sl, start=True, stop=True)

        # ================= stage 1: gathered-row transforms =================
        with tc.tile_pool(name="pbig", bufs=1) as pbig:
            expm = pbig.tile([EM, EMB], F32R)
            sp_ps = []
            wsb = [pbig.tile([128, 1], F32, tag=f"wsb{i}", name=f"wsb{i}")
                   for i in range(4)]
            wsp = [pbig.tile([128, EMB], F32R, tag=f"wsp{i}", name=f"wsp{i}")
                   for i in range(4)]
            ea_sb = pbig.tile([E, C], F32R)
            eaT = [pbig.tile([128, E], BF16, tag=f"eaT{i}", name=f"eaT{i}")
                   for i in range(8)]


            with tc.tile_pool(name="ps1b", bufs=1, space="PSUM") as ps1b:
                # ea = G^T @ attm ; normalize rows
                ea_p0 = ps1b.tile([E, 512], F32, tag="ea0", name="ea0")
                ea_p1 = ps1b.tile([E, 512], F32, tag="ea1", name="ea1")
                for kc in range(9):
                    rows = 128 if kc < 8 else 32
                    at = amp[0:rows, kc * C:kc * C + C]
                    gt = gTbt[0:rows, kc * E:(kc + 1) * E]
                    nc.tensor.matmul(ea_p0[:], gt, at[:, 0:512],
                                     start=(kc == 0), stop=(kc == 8))
                    nc.tensor.matmul(ea_p1[:], gt, at[:, 512:1024],
                                     start=(kc == 0), stop=(kc == 8))
                fire("xp0", "xp1")
                r0 = pbig.tile([E, 1], F32)
                r1 = pbig.tile([E, 1], F32)
                nc.vector.tensor_reduce(r0[:], ea_p0[:], mybir.AxisListType.X,
                                        ALU.add)
                nc.vector.tensor_reduce(r1[:], ea_p1[:], mybir.AxisListType.X,
                                        ALU.add)
                rsum = pbig.tile([E, 1], F32)
                nc.vector.tensor_tensor(out=rsum[:], in0=r0[:], in1=r1[:],
                                        op=ALU.add)
                rsum2 = pbig.tile([E, 1], F32)
                nc.vector.tensor_scalar(out=rsum2[:], in0=rsum[:], scalar1=1e-5,
                                        scalar2=None, op0=ALU.add)
                rinv = pbig.tile([E, 1], F32)
                nc.vector.reciprocal(rinv[:], rsum2[:])
                # ea left unnormalized; rinv is applied as a row scale on the
                # zt copy below (zt is linear in ea), shortening the critical
                # chain ea -> eaT -> zt.
                nc.scalar.activation(ea_sb[:, 0:512], ea_p0[:], AF.Copy)
                nc.scalar.activation(ea_sb[:, 512:1024], ea_p1[:], AF.Copy)
                easum = pbig.tile([E, 1], F32)
                nc.vector.tensor_tensor(out=easum[:], in0=rsum[:], in1=rinv[:],
                                        op=ALU.mult)
                # small late-use constants; placed here so their HWDGE slots
                # don't delay the early amp pieces
                nc.scalar.dma_start(eln_b[0:E, EMB:D0], tfb_d[0:E, :])
                nc.scalar.dma_start(mrep_b[0:EM, EMB:D0], tfb_d[E:E + EM, :])
                nc.scalar.dma_start(link_b[0:L, EMB:D0], tfb_d[E + EM:NN, :])
                nc.scalar.dma_start(aallTb[:], aallTb_d[:])
                nc.scalar.dma_start(identb[:], identb_d[:])
                for kc in range(8):
                    if kc == 2:
                        fire("xp2")
                    elif kc == 4:
                        fire("xp3")
                    tp = ps1b.tile([128, E], F32, tag="eaTt", name="eaTt", bufs=2)
                    nc.tensor.transpose(tp[:],
                                        ea_sb[:, kc * 128:(kc + 1) * 128]
                                        .bitcast(F32), ident[0:E, 0:E])
                    if kc % 2 == 0:
                        nc.scalar.copy(eaT[kc][:], tp[:])
                    else:
                        nc.vector.tensor_copy(out=eaT[kc][:], in_=tp[:])
                tp = ps1b.tile([1, E], F32, tag="easumt", name="easumt")
                nc.tensor.transpose(tp[:], easum[:], ident[0:E, 0:E])
                nc.scalar.copy(easumT[:], tp[:])

            with tc.tile_pool(name="ps1c", bufs=1, space="PSUM") as ps1c:
                # zT = ea @ x  [22, 768] (two 384-wide halves); rinv applied
                # on the copy out of PSUM
                zt_ps = [ps1c.tile([E, 384], F32, tag=f"zt_p{i}",
                                   name=f"zt_p{i}") for i in range(2)]
                for kc in range(8):
                    if kc == 3:
                        fire("wtr0")
                    elif kc == 6:
                        fire("wtr1")
                    xt = xp[:, kc * HID:(kc + 1) * HID]
                    for hh in range(2):
                        nc.tensor.matmul(zt_ps[hh][:], eaT[kc][:],
                                         xt[:, hh * 384:(hh + 1) * 384],
                                         start=(kc == 0), stop=(kc == 7))
                fire("actr0")
                nc.scalar.activation(zt_sb[:, 0:384], zt_ps[0][:], AF.Copy,
                                     scale=rinv[:])
                nc.scalar.activation(zt_sb[:, 384:768], zt_ps[1][:], AF.Copy,
                                     scale=rinv[:])

            with tc.tile_pool(name="ps1a", bufs=1, space="PSUM") as ps1a:
                # mentions: mrep = x_m @ Wtr + b -> straight into h0b (bf16)
                mrep_p = ps1a.tile([EM, EMB], F32, tag="mrep", name="mrep")
                for kc in range(6):
                    nc.tensor.matmul(mrep_p[:], xmT[kc][:, 0:EM], wtr[kc][:],
                                     start=(kc == 0), stop=False)
                nc.tensor.matmul(mrep_p[:], onesrow[0:1, 0:EM], brow[:],
                                 start=False, stop=True)
                nc.scalar.activation(mrep_b[0:EM, 0:EMB], mrep_p[:], AF.Copy)
                nc.scalar.activation(expm[:], mrep_p[:], AF.Exp)
                # e_rep = ln(G2 @ exp(mrep))
                ep_p = ps1a.tile([E, EMB], F32, tag="ep", name="ep")
                nc.tensor.matmul(ep_p[:], g2T[:], expm[:], start=True, stop=True)
                nc.scalar.activation(eln_b[0:E, 0:EMB], ep_p[:], AF.Ln)
                # swap to the sigmoid table now (every later act is in it);
                # reading ep_p pins this after the Ln in the schedule
                nc.scalar.activation(actwarm[:], ep_p[0:1, 0:1], AF.Sigmoid)

                # w = colsum(attl) / 384
                for mc in range(4):
                    w_p = ps1a.tile([128, 1], F32, tag="w_p", name="w_p", bufs=1)
                    for kc in range(3):
                        nc.tensor.matmul(w_p[:],
                                         attl[kc][:, mc * 128:(mc + 1) * 128],
                                         onescol[:],
                                         start=(kc == 0), stop=(kc == 2))
                    nc.scalar.activation(wsb[mc][:], w_p[:], AF.Copy,
                                         scale=1.0 / (H * SPAN))
                fire("actr1")
                # spans: sp = x_span @ Wtr + b
                for mc in range(4):
                    if mc > 0:
                        fire(f"wstp0{mc - 1}" if mc < 3 else "wstp10")
                    sp_p = ps1a.tile([128, EMB], F32, tag="sp_p", name="sp_p",
                                     bufs=3)
                    for kc in range(6):
                        nc.tensor.matmul(sp_p[:],
                                         xspT[kc][:, mc * 128:(mc + 1) * 128],
                                         wtr[kc][:], start=(kc == 0), stop=False)
                    nc.tensor.matmul(sp_p[:], onesrow[:], brow[:],
                                     start=False, stop=True)
                    spc = pbig.tile([128, EMB], F32, tag="spc", name="spc",
                                    bufs=4)
                    nc.scalar.copy(spc[:], sp_p[:])
                    sp_ps.append(spc)
                # wsp = psum(sp) * w ; link = SUM^T @ wsp
                for mc in range(4):
                    nc.vector.tensor_scalar(out=wsp[mc][:], in0=sp_ps[mc][:],
                                            scalar1=wsb[mc][:], scalar2=None,
                                            op0=ALU.mult)
                link_p = ps1a.tile([L, EMB], F32, tag="link", name="link")
                for kc in range(4):
                    nc.tensor.matmul(link_p[:], sumT[kc][:], wsp[kc][:],
                                     start=(kc == 0), stop=(kc == 3))
                nc.scalar.activation(link_b[0:L, 0:EMB], link_p[:], AF.Copy)
                fire("wstp11")


        if stages >= 2:
          # ================= stage 2: RGCN (4 layers, y col-half pipelined) ==
          ecT = [pwork.tile([128, E], F32R, tag=f"ecT{i}", name=f"ecT{i}")
                 for i in range(4)]
          with tc.tile_pool(name="prg", bufs=2) as prg, \
               tc.tile_pool(name="psr", bufs=1, space="PSUM") as psr:
              h = None
              UW = (NREL + 1) * NN
              for layer in range(NLAYERS):
                  kcs = _KC0 if layer == 0 else _KC1
                  nk = len(kcs)
                  wstp = wstp_t[layer]
                  nrel_l = 2 if layer == 3 else NREL + 1
                  uw_l = 2 * E if layer == 3 else UW
                  wst_t = [wstp[:, (r * nk + si) * EMB:(r * nk + si + 1) * EMB]
                           for r in range(nrel_l) for si in range(nk)]
                  # u = h^T @ A_allT per d-chunk. Layer 3 only needs entity
                  # output rows, which only rel-0 and the self-loop feed.
                  u_sb = []
                  for si, (s0, sl) in enumerate(kcs):
                      u_p = psr.tile([128, (NREL + 1) * NN], F32, tag="u_p",
                                     name="u_p", bufs=2)
                      if layer == 0:
                          for pi, (pt, rows) in enumerate(h0p):
                              nc.tensor.matmul(u_p[0:sl, 0:UW],
                                               pt[0:rows, s0:s0 + sl],
                                               aallTb[0:rows,
                                                      (pi + 1) * UW:
                                                      (pi + 2) * UW],
                                               start=(pi == 0), stop=(pi == 2))
                      elif layer == 3:
                          nc.tensor.matmul(u_p[0:sl, 0:2 * E],
                                           h[0:NN, s0:s0 + sl],
                                           aallTb[0:NN, 4 * UW:4 * UW + 2 * E],
                                           start=True, stop=True)
                      else:
                          nc.tensor.matmul(u_p[0:sl, 0:UW],
                                           h[0:NN, s0:s0 + sl],
                                           aallTb[0:NN, 0:UW],
                                           start=True, stop=True)
                      u = prg.tile([128, (NREL + 1) * NN], BF16, tag=f"u{si}",
                                   name=f"u{si}")
                      if si % 2 == 0:
                          nc.scalar.copy(u[0:sl, 0:uw_l], u_p[0:sl, 0:uw_l])
                      else:
                          nc.vector.tensor_copy(out=u[0:sl, 0:uw_l],
                                                in_=u_p[0:sl, 0:uw_l])
                      u_sb.append(u)
                  # y = sum_r (u_r)^T @ Wst_r, in two column halves so the
                  # relu of half 0 overlaps the matmuls of half 1
                  if layer < 2:
                      fire(f"wstp{layer + 2}0", f"wstp{layer + 2}1")
                  elif layer == 2:
                      fire("sew")
                  # deferred e_ctx chain, one piece per inter-layer window
                  if layer == 0:
                      for kc in range(6):
                          ztp = psr.tile([128, E], F32, tag="tp22", name="ztp",
                                         bufs=2)
                          nc.tensor.transpose(ztp[:],
                                              zt_sb[:, kc * 128:(kc + 1) * 128],
                                              ident[0:E, 0:E])
                          if kc % 2 == 0:
                              nc.scalar.copy(z_sb[kc][:], ztp[:])
                          else:
                              nc.vector.tensor_copy(out=z_sb[kc][:], in_=ztp[:])
                  elif layer == 1:
                      ec2_p = psr.tile([E, EMB], F32, tag="ec2", name="ec2")
                      for kc in range(6):
                          nc.tensor.matmul(ec2_p[:], z_sb[kc][:], wtr[kc][:],
                                           start=(kc == 0), stop=False)
                      nc.tensor.matmul(ec2_p[:], easumT[:], brow[:],
                                       start=False, stop=True)
                      nc.scalar.copy(ec2_sb[:], ec2_p[:])
                  elif layer == 2:
                      for mc in range(4):
                          ecp = psr.tile([128, E], F32, tag="tp22", name="ecp",
                                         bufs=2)
                          nc.tensor.transpose(ecp[:],
                                              ec2_sb[:, mc * 128:(mc + 1) * 128],
                                              ident[0:E, 0:E])
                          if mc % 2 == 0:
                              nc.scalar.copy(ectxT_sb[mc][:], ecp[:])
                          else:
                              nc.vector.tensor_copy(out=ectxT_sb[mc][:],
                                                    in_=ecp[:])
                  nrows = E if layer == 3 else NN
                  rw = E if layer == 3 else NN
                  hn = prg.tile([NN, EMB], BF16, tag="h_next", name="h_next")
                  for yh in range(2):
                      y_p = psr.tile([NN, 256], F32, tag=f"y_p{yh}",
                                     name=f"y_p{yh}")
                      n_mm = nrel_l * nk
                      k_mm = 0
                      for si, (s0, sl) in enumerate(kcs):
                          for r in range(nrel_l):
                              nc.tensor.matmul(
                                  y_p[0:nrows, :],
                                  u_sb[si][0:sl, r * rw:r * rw + nrows],
                                  wst_t[r * nk + si][0:sl,
                                                     yh * 256:(yh + 1) * 256],
                                  start=(k_mm == 0), stop=(k_mm == n_mm - 1))
                              k_mm += 1
                      nc.scalar.activation(hn[0:nrows, yh * 256:(yh + 1) * 256],
                                           y_p[0:nrows, :], AF.Relu)
                  h = hn

              # entity_struT + e_ctxT -> ecT
              for mc in range(4):
                  tp = psr.tile([128, E], F32, tag="tp22", name="est", bufs=2)
                  nc.tensor.matmul(tp[:], h[0:E, mc * 128:(mc + 1) * 128],
                                   identb[0:E, 0:E], start=True, stop=True)
                  nc.vector.tensor_tensor(out=ecT[mc][:], in0=tp[:],
                                          in1=ectxT_sb[mc][:], op=ALU.add)

        prgw_cm.__exit__(None, None, None)

        if stages >= 3:
          # ================= stage 3: fmap + SE =================
          fmap = [pwork.tile([128, EE], F32R, tag=f"fmap{i}", name=f"fmap{i}")
                  for i in range(4)]
          pooled = [pwork.tile([128, 1], F32R, tag=f"pool{i}", name=f"pool{i}")
                    for i in range(4)]
          for mc in range(4):
              o6v = fmap[mc][:].rearrange("p (i j) -> p i j", i=E)
              in0 = ecT[mc][:].rearrange("p (i j) -> p i j", j=1) \
                  .to_broadcast([128, E, E])
              in1 = ecT[mc][:].rearrange("p (o j) -> p o j", o=1) \
                  .to_broadcast([128, E, E])
              nc.vector.tensor_tensor(out=o6v, in0=in0, in1=in1, op=ALU.mult)
              rs = pwork.tile([128, 1], F32, tag=f"rs{mc}", name=f"rs{mc}")
              nc.vector.tensor_reduce(rs[:], ecT[mc][:], mybir.AxisListType.X,
                                      ALU.add)
              nc.scalar.activation(pooled[mc][:], rs[:], AF.Square, scale=1.0 / E)

          with tc.tile_pool(name="pse", bufs=1, space="PSUM") as pse:
              # c-path first (tiny serial chain, hides under fmap/s1p);
              # seb2 is folded into the fcb2 host constant.
              c1_sb = [pwork.tile([128, 1], F32R, tag=f"c1_{i}", name=f"c1_{i}")
                       for i in range(2)]
              for oc in range(2):
                  c1_p = pse.tile([128, 1], F32, tag="cp", name="c1p", bufs=2)
                  for mc in range(4):
                      nc.tensor.matmul(c1_p[:],
                                       sew["fcw1T"][mc][:, oc * 128:(oc + 1) * 128]
                                       .bitcast(F32),
                                       pooled[mc][:].bitcast(F32),
                                       start=(mc == 0), stop=(mc == 3))
                  nc.scalar.activation(c1_sb[oc][:], c1_p[:], AF.Relu,
                                       bias=sev["fcb1"][oc][:],
                                       scale=sev["fcs1"][oc][:])
              cbb = [pwork.tile([128, 1], F32, tag=f"cbb{i}", name=f"cbb{i}")
                     for i in range(4)]
              # s-path; s2's first half starts as soon as s1_sb[0] is ready
              s1_sb = [pwork.tile([128, EE], F32R, tag=f"s1_{i}", name=f"s1_{i}")
                       for i in range(2)]
              for oc in range(2):
                  s1_p = pse.tile([128, EE], F32, tag="s1p", name="s1p", bufs=2)
                  for mc in range(4):
                      nc.tensor.matmul(s1_p[:],
                                       sew["fsw1T"][mc][:, oc * 128:(oc + 1) * 128],
                                       fmap[mc][:], start=(mc == 0), stop=(mc == 3))
                  nc.scalar.activation(s1_sb[oc][:], s1_p[:], AF.Relu,
                                       bias=sev["seb1"][oc][:],
                                       scale=sev["ses1"][oc][:])
              for mc in range(4):
                  c2_p = pse.tile([128, 1], F32, tag="cp", name="c2p", bufs=2)
                  for kc in range(2):
                      nc.tensor.matmul(c2_p[:],
                                       sew["fcw2T"][kc][:, mc * 128:(mc + 1) * 128]
                                       .bitcast(F32),
                                       c1_sb[kc][:].bitcast(F32),
                                       start=(kc == 0), stop=(kc == 1))
                  nc.scalar.activation(cbb[mc][:], c2_p[:], AF.Identity,
                                       bias=sev["fcb2"][mc][:],
                                       scale=sev["fcs2"][mc][:])
              s2_ps = [pse.tile([128, EE], F32, tag="s2p", name=f"s2p{mc}",
                                bufs=3) for mc in range(4)]
              for mc in range(4):
                  nc.tensor.matmul(s2_ps[mc][:],
                                   sew["fsw2T"][0][:, mc * 128:(mc + 1) * 128],
                                   s1_sb[0][:], start=True, stop=False)
              for mc in range(4):
                  nc.tensor.matmul(s2_ps[mc][:],
                                   sew["fsw2T"][1][:, mc * 128:(mc + 1) * 128],
                                   s1_sb[1][:], start=False, stop=True)
                  sig = pwork.tile([128, EE], F32, tag="sig", name="sig", bufs=2)
                  nc.scalar.activation(sig[:], s2_ps[mc][:], AF.Sigmoid,
                                       bias=cbb[mc][:], scale=sev["ses2"][mc][:])
                  outv = fusedp[mc][:].rearrange("p (i j) -> p i j", j=26)[:, 2:24,
                                                                          2:24]
                  nc.vector.tensor_tensor(
                      out=outv,
                      in0=fmap[mc][:].rearrange("p (i j) -> p i j", i=E),
                      in1=sig[:].rearrange("p (i j) -> p i j", i=E),
                      op=ALU.mult)

        if stages >= 4:
          # ================= stage 4: conv stack =================
          def tap_rows(padt, tap, r0, nr):
              dy, dx = tap // 5, tap % 5
              return padt[:].rearrange("p (i j) -> p i j", j=26)[
                  :, dy + r0:dy + r0 + nr, dx:dx + 22]

          def tap_view(padt, tap):
              return tap_rows(padt, tap, 0, 22)

          with tc.tile_pool(name="pcw", bufs=1) as pcw, \
               tc.tile_pool(name="psc", bufs=1, space="PSUM") as psc:
              w1, w2, w3 = [], [], []
              for kc in range(4):
                  t = pcw.tile([128, 25 * 128], BF16, tag=f"w1_{kc}",
                               name=f"w1_{kc}")
                  nc.sync.dma_start(t[:], w1sb_d[kc])
                  w1.append(t)
              for kc in range(2):
                  t = pcw.tile([128, 25 * 128], BF16, tag=f"w2_{kc}",
                               name=f"w2_{kc}")
                  nc.sync.dma_start(t[:], w2sb_d[kc])
                  w2.append(t)
              for kc in range(2):
                  t = pcw.tile([128, 25 * 256], BF16, tag=f"w3_{kc}",
                               name=f"w3_{kc}")
                  nc.sync.dma_start(t[:], w3sb_d[kc])
                  w3.append(t)
              mtop = cf("mtop")
              mbot = cf("mbot")

              def interior_rows(t_, r0, nr):
                  return t_[:].rearrange("p (i j) -> p i j", j=26)[
                      :, 2 + r0:2 + r0 + nr, 2:24]

              def gather_combine(src_flat, dst_pad, gg, rh, nm, rows=None):
                  """relu'd flat rows rh -> allgather -> mask-combine into
                  dst rows rh, issued right after the producing relu."""
                  r0, nr = rows
                  rb = pdram.tile([128, nr * 22], BF16, tag=f"{nm}b{rh}",
                                  name=f"{nm}b{rh}")
                  nc.sync.dma_start(rb[:], src_flat[:])
                  if solo:
                      nc.sync.dma_start(gg[0:128, :], rb[:])
                      nc.gpsimd.dma_start(gg[128:256, :], rb[:])
                  else:
                      nc.gpsimd.collective_compute(
                          "AllGather", ALU.bypass, replica_groups=groups,
                          ins=[rb[:].opt()], outs=[gg[:].opt()])
                  gt = pcw.tile([128, nr * 22], BF16, tag=f"{nm}gt{rh}",
                                name=f"{nm}gt{rh}")
                  gb = pcw.tile([128, nr * 22], BF16, tag=f"{nm}gb{rh}",
                                name=f"{nm}gb{rh}")
                  nc.sync.dma_start(gt[:], gg[0:128, :])
                  nc.gpsimd.dma_start(gb[:], gg[128:256, :])
                  tmp = pcw.tile([128, nr * 22], F32, tag=f"{nm}tmp{rh}",
                                 name=f"{nm}tmp{rh}")
                  nc.vector.tensor_scalar(out=tmp[:], in0=gb[:],
                                          scalar1=mbot[:], scalar2=None,
                                          op0=ALU.mult)
                  nc.vector.scalar_tensor_tensor(
                      out=interior_rows(dst_pad, r0, nr), in0=gt[:],
                      scalar=mtop[:], in1=tmp[:], op0=ALU.mult, op1=ALU.add)

              # conv1 rows (0,18),(18,4): the tiny second half's gather
              # launches right at conv1's end, so its readback chain hides
              # under conv2's first-half taps.
              ROWS = ((0, 17), (17, 5))

              # conv1 (my half of 256 out channels), in two row halves; each
              # half's AllGather + readback starts as soon as the half is done
              r1g = [pdram.tile([256, nr * 22], BF16, tag=f"r1g{i}",
                                name=f"r1g{i}")
                     for i, (r0, nr) in enumerate(ROWS)]
              for rh, (r0, nr) in enumerate(ROWS):
                  r1_p = psc.tile([128, nr * 22], F32, tag="convp",
                                  name="convp", bufs=4)
                  first = True
                  for kc in range(4):
                      for tap in range(25):
                          nc.tensor.matmul(r1_p[:],
                                           w1[kc][:, tap * 128:(tap + 1) * 128],
                                           tap_rows(fusedp[kc], tap, r0, nr),
                                           start=first,
                                           stop=(kc == 3 and tap == 24))
                          first = False
                  o1f = pcw.tile([128, nr * 22], BF16, tag=f"o1f{rh}",
                                 name=f"o1f{rh}")
                  nc.scalar.activation(o1f[:], r1_p[:], AF.Relu, bias=b1h[:])
                  nc.vector.tensor_copy(out=interior_rows(own1, r0, nr),
                                        in_=o1f[:].rearrange(
                                            "p (i j) -> p i j", j=22))
                  gather_combine(o1f, oth1, r1g[rh], rh, "r1", (r0, nr))

              # conv2: own-input taps for both row-halves first (no gather
              # dependency), then other-input taps; output in row halves.
              # conv2 out rows (0-8, 9-21): the first half's own AND other
              # taps read only rows <=12 of r1, i.e. conv1-half0 + gather#0 —
              # no wait on conv1-half1's gather.
              ROWS2 = ((0, 15), (15, 7))
              r2g = [pdram.tile([256, nr * 22], BF16, tag=f"r2g{i}",
                                name=f"r2g{i}")
                     for i, (r0, nr) in enumerate(ROWS2)]
              r2_ps = [psc.tile([128, nr * 22], F32, tag=f"convp2_{i}",
                                name=f"convp2_{i}")
                       for i, (r0, nr) in enumerate(ROWS2)]
              # rh1's own-taps first (ready at conv1 end) to widen the
              # window that hides gather#1's readback chain; rh0 is fully
              # ready (gather#0 landed mid-conv1) and runs next.
              r10, n1 = ROWS2[1]
              for tap in range(25):
                  nc.tensor.matmul(r2_ps[1][:],
                                   w2[0][:, tap * 128:(tap + 1) * 128],
                                   tap_rows(own1, tap, r10, n1),
                                   start=(tap == 0), stop=False)
              r0, nr = ROWS2[0]
              for tap in range(25):
                  nc.tensor.matmul(r2_ps[0][:],
                                   w2[0][:, tap * 128:(tap + 1) * 128],
                                   tap_rows(own1, tap, r0, nr),
                                   start=(tap == 0), stop=False)
              for tap in range(25):
                  nc.tensor.matmul(r2_ps[0][:],
                                   w2[1][:, tap * 128:(tap + 1) * 128],
                                   tap_rows(oth1, tap, r0, nr),
                                   start=False, stop=(tap == 24))
              o2f0 = pcw.tile([128, nr * 22], BF16, tag="o2f0", name="o2f0")
              nc.scalar.activation(o2f0[:], r2_ps[0][:], AF.Relu, bias=b2h[:])
              nc.vector.tensor_copy(out=interior_rows(own2, r0, nr),
                                    in_=o2f0[:].rearrange(
                                        "p (i j) -> p i j", j=22))
              gather_combine(o2f0, oth2, r2g[0], 0, "r2", (r0, nr))
              for tap in range(25):
                  nc.tensor.matmul(r2_ps[1][:],
                                   w2[1][:, tap * 128:(tap + 1) * 128],
                                   tap_rows(oth1, tap, r10, n1),
                                   start=False, stop=(tap == 24))
              o2f1 = pcw.tile([128, n1 * 22], BF16, tag="o2f1", name="o2f1")
              nc.scalar.activation(o2f1[:], r2_ps[1][:], AF.Relu, bias=b2h[:])
              nc.vector.tensor_copy(out=interior_rows(own2, r10, n1),
                                    in_=o2f1[:].rearrange(
                                        "p (i j) -> p i j", j=22))
              gather_combine(o2f1, oth2, r2g[1], 1, "r2", (r10, n1))

              # conv3 (my 256 of 512 out channels): own-input taps for both
              # out chunks first, then other-input taps per (oc, row-half)
              # with relu+store pipelined per row half (short tail).
              r3_ps = [psc.tile([128, ROWS[rh][1] * 22], F32, tag="convp",
                                name=f"convp3_{oc}{rh}", bufs=4)
                       for oc in range(2) for rh in range(2)]
              for oc in range(2):
                  for rh, (r0, nr) in enumerate(ROWS):
                      for tap in range(25):
                          nc.tensor.matmul(
                              r3_ps[oc * 2 + rh][:],
                              w3[0][:, tap * 256 + oc * 128:
                                    tap * 256 + (oc + 1) * 128],
                              tap_rows(own2, tap, r0, nr),
                              start=(tap == 0), stop=False)
              for oc in range(2):
                  for rh, (r0, nr) in enumerate(ROWS):
                      for tap in range(25):
                          nc.tensor.matmul(
                              r3_ps[oc * 2 + rh][:],
                              w3[1][:, tap * 256 + oc * 128:
                                    tap * 256 + (oc + 1) * 128],
                              tap_rows(oth2, tap, r0, nr),
                              start=False, stop=(tap == 24))
                      o_sb = pcw.tile([128, nr * 22], F32, tag="osb",
                                      name="osb", bufs=4)
                      nc.scalar.activation(o_sb[:], r3_ps[oc * 2 + rh][:],
                                           AF.Relu, bias=b3h[oc][:])
                      nc.sync.dma_start(
                          out_d[oc * 128:(oc + 1) * 128,
                                r0 * 22:(r0 + nr) * 22], o_sb[:])

    nc.compile()
    return nc


_NC_CACHE = None


def _get_program():
    global _NC_CACHE
    if _NC_CACHE is None:
        _NC_CACHE = build_program()
    return _NC_CACHE


def _prep_shared(w):
    """Packed weights/constants identical on every core."""
    ADJ = _build_adj()
    out = {}
    constr = np.zeros((128, _CR), np.float32)

    def put(nm, arr):
        c0, cols = _LAY_R[nm]
        r, cc = arr.shape
        constr[0:r, c0:c0 + cc] = arr
    g2T = np.zeros((EM, E), np.float32)
    for e in range(E):
        g2T[e * M:(e + 1) * M, e] = 1.0
    put("g2T", g2T)
    sumT = np.kron(np.eye(L, dtype=np.float32), np.ones((SPAN, 1), np.float32))
    for kc in range(4):
        put(f"sumT{kc}", sumT[kc * 128:(kc + 1) * 128])
    for nm, arr, nch in (("fsw1T", w['fs_w1'].T, 4), ("fcw1T", w['fc_w1'].T, 4),
                         ("fsw2T", w['fs_w2'].T, 2), ("fcw2T", w['fc_w2'].T, 2)):
        for kc in range(nch):
            put(f"{nm}{kc}", np.ascontiguousarray(arr[kc * 128:(kc + 1) * 128]))
    out['constr'] = constr

    constw = np.zeros((128, _CW), np.float32)
    wt = w['W_trans']
    for kc in range(6):
        constw[:, kc * EMB:(kc + 1) * EMB] = wt[kc * 128:(kc + 1) * 128]
    constw[0, 6 * EMB:7 * EMB] = w['b_trans']
    out['constw'] = constw.astype(ml_dtypes.bfloat16)

    gTb = np.zeros((128, 9 * E), np.float32)
    gT = np.zeros((EMH, E), np.float32)
    for e in range(E):
        gT[e * M * H:(e + 1) * M * H, e] = 1.0 / (M * H)
    for kc in range(9):
        r = min(128, EMH - kc * 128)
        gTb[0:r, kc * E:(kc + 1) * E] = gT[kc * 128:kc * 128 + r]
    out['gTb'] = gTb.astype(ml_dtypes.bfloat16)
    aall = np.concatenate(
        [ADJ[r].T for r in range(NREL)] + [np.eye(NN, dtype=np.float32)],
        axis=1)
    UW = (NREL + 1) * NN
    aallp = np.zeros((128, 4 * UW + 2 * E), np.float32)
    aallp[0:NN, 0:UW] = aall
    aallp[0:EM, UW:2 * UW] = aall[E:E + EM]
    aallp[0:L, 2 * UW:3 * UW] = aall[E + EM:NN]
    aallp[0:E, 3 * UW:4 * UW] = aall[0:E]
    # layer-3 entity-only columns: [A0^T[:, :E] | I[:, :E]]
    aallp[0:NN, 4 * UW:4 * UW + E] = aall[:, 0:E]
    aallp[0:NN, 4 * UW + E:4 * UW + 2 * E] = aall[:, NREL * NN:NREL * NN + E]
    out['aallTb'] = aallp.astype(ml_dtypes.bfloat16)
    out['tfb'] = np.ascontiguousarray(
        w['type_embed'][_TYPES]).astype(ml_dtypes.bfloat16)
    out['identb'] = np.eye(128, dtype=np.float32).astype(ml_dtypes.bfloat16)

    constf = np.zeros((128, _CF), np.float32)

    def putf(nm, arr):
        c0, cols = _LAY_F[nm]
        constf[0:arr.shape[0], c0:c0 + 1] = arr.reshape(-1, 1)
    vecs = {"ses1": w['fs_g1'], "seb1": w['fs_b1'] * w['fs_g1'] + w['fs_be1'],
            "fcs1": w['fc_g1'], "fcb1": w['fc_b1'] * w['fc_g1'] + w['fc_be1'],
            "ses2": w['fs_g2'], "seb2": w['fs_b2'] * w['fs_g2'] + w['fs_be2'],
            "fcs2": w['fc_g2'],
            "fcb2": (w['fc_b2'] * w['fc_g2'] + w['fc_be2'] +
                     w['fs_b2'] * w['fs_g2'] + w['fs_be2'])}
    for nm, v in vecs.items():
        nch = 2 if v.shape[0] == INTER else 4
        for kc in range(nch):
            putf(f"{nm}{kc}", v[kc * 128:(kc + 1) * 128])
    out['constf_base'] = constf

    for layer in range(NLAYERS):
        din_l = D0 if layer == 0 else EMB
        kcs = _KC0 if layer == 0 else _KC1
        nk = len(kcs)
        Wst = w['rgcn_Wrel0'].reshape(NREL * D0, EMB) if layer == 0 else \
            w['rgcn_Wrel'][layer - 1].reshape(NREL * EMB, EMB)
        Wself = w['rgcn_Wself0'] if layer == 0 else w['rgcn_Wself'][layer - 1]
        if layer == 3:
            # entity rows only need rel-0 and the self-loop
            p = np.zeros((128, 2 * nk * EMB), np.float32)
            for si, (s0, sl) in enumerate(kcs):
                p[0:sl, si * EMB:(si + 1) * EMB] = Wst[s0:s0 + sl]
                p[0:sl, (nk + si) * EMB:(nk + si + 1) * EMB] = \
                    Wself[s0:s0 + sl]
        else:
            p = np.zeros((128, (NREL + 1) * nk * EMB), np.float32)
            for r in range(NREL):
                for si, (s0, sl) in enumerate(kcs):
                    p[0:sl, (r * nk + si) * EMB:(r * nk + si + 1) * EMB] = \
                        Wst[r * din_l + s0:r * din_l + s0 + sl]
            for si, (s0, sl) in enumerate(kcs):
                p[0:sl, (NREL * nk + si) * EMB:(NREL * nk + si + 1) * EMB] = \
                    Wself[s0:s0 + sl]
        out[f'wstp{layer}'] = p.astype(ml_dtypes.bfloat16)
    return out


def _prep_conv_half(w, half, constf_base):
    out = {}
    w1 = w['cr_w1'][half * 128:(half + 1) * 128]
    out['w1sb'] = np.ascontiguousarray(
        w1.transpose(1, 2, 3, 0).reshape(4, 128, 25 * 128)).astype(ml_dtypes.bfloat16)
    # conv2/conv3 weights in (own-input-half, other-input-half) chunk order
    w2 = w['cr_w2'][half * 128:(half + 1) * 128]
    w2p = w2.transpose(1, 2, 3, 0).reshape(2, 128, 25 * 128)
    order = [half, 1 - half]
    out['w2sb'] = np.ascontiguousarray(w2p[order]).astype(ml_dtypes.bfloat16)
    w3 = w['cr_w3'][half * 256:(half + 1) * 256]
    w3p = w3.transpose(1, 2, 3, 0).reshape(2, 128, 25 * 256)
    out['w3sb'] = np.ascontiguousarray(w3p[order]).astype(ml_dtypes.bfloat16)
    constf = constf_base.copy()

    def putf(nm, arr):
        c0, cols = _LAY_F[nm]
        constf[0:arr.shape[0], c0:c0 + 1] = arr.reshape(-1, 1)
    putf("b1h", w['cr_b1'][half * 128:(half + 1) * 128])
    putf("b2h", w['cr_b2'][half * 128:(half + 1) * 128])
    putf("b3h0", w['cr_b3'][half * 256:half * 256 + 128])
    putf("b3h1", w['cr_b3'][half * 256 + 128:half * 256 + 256])
    putf("mtop", np.full(128, float(half), np.float32))
    putf("mbot", np.full(128, float(1 - half), np.float32))
    c0, cols = _LAY_F["identf"]
    constf[:, c0:c0 + 128] = np.eye(128, dtype=np.float32)
    out['constf'] = constf
    return out


def _prep_doc(x, att, mi, ls):
    out = {}
    mif = mi.reshape(EM)
    attm = np.ascontiguousarray(
        att[:, mif, :].transpose(1, 0, 2).reshape(EMH, C))
    amp = np.zeros((128, 9 * C), np.float32)
    for kc in range(9):
        r = min(128, EMH - kc * 128)
        amp[0:r, kc * C:kc * C + C] = attm[kc * 128:kc * 128 + r]
    out['amp'] = amp.astype(ml_dtypes.bfloat16)
    idx = ls[:, None] + np.arange(SPAN)
    idxf = idx.reshape(LS)
    rows = att[:, idxf, :].reshape(H, L, SPAN, C)
    blocks = np.take_along_axis(rows, idx[None, :, None, :], axis=3)
    attl = blocks.transpose(0, 2, 1, 3).reshape(HS, LS)
    xmT = x[mif].T
    xspT = x[idxf].T
    actr = np.zeros((128, _CA), np.float32)

    def put(nm, arr):
        c0, cols = _LAY_A[nm]
        actr[0:arr.shape[0], c0:c0 + arr.shape[1]] = arr
    for kc in range(6):
        put(f"xmT{kc}", xmT[kc * 128:(kc + 1) * 128])
        put(f"xspT{kc}", xspT[kc * 128:(kc + 1) * 128])
    for kc in range(3):
        put(f"attl{kc}", attl[kc * 128:(kc + 1) * 128])
    out['actr'] = actr.astype(ml_dtypes.bfloat16)
    xpk = np.zeros((128, 8 * HID), np.float32)
    for kc in range(8):
        xpk[:, kc * HID:(kc + 1) * HID] = x[kc * 128:(kc + 1) * 128]
    out['xp'] = xpk.astype(ml_dtypes.bfloat16)
    return out


def build_in_maps(inputs):
    w = {}
    for k, v in inputs.items():
        a = np.asarray(v)
        w[k] = a if a.dtype in (np.int32, np.int64) else \
            np.asarray(a, np.float32)
    shared = _prep_shared(w)
    constf_base = shared.pop('constf_base')
    halves = [_prep_conv_half(w, h, constf_base) for h in range(2)]
    seq = np.asarray(inputs['sequence_output'], np.float32)
    att = np.asarray(inputs['attention'], np.float32)
    mi = np.asarray(inputs['mention_idx']).astype(np.int64)
    ls = np.asarray(inputs['link_start']).astype(np.int64)
    docs = [_prep_doc(seq[n], att[n], mi[n], ls[n]) for n in range(NB)]
    in_maps = []
    for core in range(N_CORES):
        n, half = core // 2, core % 2
        m = dict(shared)
        m.update(halves[half])
        m.update(docs[n])
        in_maps.append({k: (np.ascontiguousarray(v) if v.dtype == ml_dtypes.bfloat16
                            else np.ascontiguousarray(v, np.float32))
                        for k, v in m.items()})
    return in_maps


def kernel(**inputs):
    nc = _get_program()
    in_maps = build_in_maps(inputs)
    res = run_bass_kernel_spmd(nc, in_maps, list(range(N_CORES)))
    out = np.zeros((NB, EMB, E, E), np.float32)
    for core in range(N_CORES):
        n, half = core // 2, core % 2
        out[n, half * 256:(half + 1) * 256] = \
            res.results[core]["out"].reshape(256, E, E)
    return out


# revision 41
# speedup vs baseline: 1.0294x; 1.0080x over previous
"""Trainium2 Bass kernel for nn_DocREModel (DocRE: gather -> RGCN -> SE -> 5x5 convs).

Sharding: 4 documents x 2 cores each. Each pair replicates the cheap upstream
(mention/link/ea gathers -> RGCN -> fmap/SE) and splits the dominant 5x5 conv
stack by output channels, with intra-pair AllGathers; output halves are
assembled on host. All index-driven gathers happen on host (pure data
movement; one SPMD program serves all 8 cores), all dense math on device.

Perf notes (v3):
- DMA is a serialized ~360B/ns resource in the cost model; all input loads
  go on ONE queue (sync/SP) in exact consumption order, in <=~800KB pieces,
  so arrival order is deterministic and matches the compute schedule.
- The PE p-state ramp resets on >~2us idle gaps. Tiny "touch" matmuls that
  read one column of each arriving DMA piece pulse the PE at piece cadence,
  holding the fast clock through DMA-bound phases.
- bf16 for W_trans, gathered activations, RGCN + conv weights (f32 PSUM
  accumulation everywhere); f32r for the remaining f32 path.
- Convs are 25 shift-tap matmuls over zero-padded 26x26 images via strided
  APs. conv1/conv2 outputs are produced in two 11-row halves so each half's
  AllGather (+ readback + mask-combine) overlaps the next half's matmuls;
  conv3 output is relu'd + stored per (out-chunk, row-half) to shrink the
  kernel tail. RGCN folds the self-loop in as a 4th identity relation and
  pipelines each layer by y column halves.
"""

import numpy as np
import ml_dtypes

import concourse.bacc as bacc
import concourse.tile as tile
from concourse import mybir
from concourse.bass_utils import run_bass_kernel_spmd

F32 = mybir.dt.float32
F32R = mybir.dt.float32r
BF16 = mybir.dt.bfloat16
AF = mybir.ActivationFunctionType
ALU = mybir.AluOpType

NB, H, C, HID, EMB = 4, 12, 1024, 768, 512
E, M, L, SPAN = 22, 4, 16, 32
TD, INTER = 20, 256
NN = E + E * M + L
NREL, NLAYERS = 3, 4
EM, EMH, HS, LS = E * M, E * M * H, H * SPAN, L * SPAN
D0 = EMB + TD           # 532
EE = E * E              # 484
PADW = 26 * 26          # 676 padded 26x26 image
N_CORES = 8


def _build_adj():
    A = np.zeros((NREL, NN, NN), np.float32)
    for e in range(E):
        for m in range(M):
            mi = E + e * M + m
            A[0, e, mi] = A[0, mi, e] = 1.0
            for m2 in range(M):
                if m2 != m:
                    A[1, mi, E + e * M + m2] = 1.0
            li = E + E * M + ((e * M + m) % L)
            A[2, mi, li] = A[2, li, mi] = 1.0
    A = A / (A.sum(-1, keepdims=True) + 1e-5)
    return A


_TYPES = np.concatenate([np.zeros(E, np.int32), np.ones(EM, np.int32),
                         np.full(L, 2, np.int32)])

_KC0 = [(0, 128), (128, 128), (256, 128), (384, 128), (512, 20)]   # 532 rows
_KC1 = [(0, 128), (128, 128), (256, 128), (384, 128)]              # 512 rows


def _const_layout():
    """f32r constants [128, CR]: stage-1 smalls + SE weights."""
    lay = {}
    c = 0

    def add(nm, cols):
        nonlocal c
        lay[nm] = (c, cols)
        c += cols
    add("g2T", E)
    for kc in range(4):
        add(f"sumT{kc}", L)
    for kc in range(4):
        add(f"fsw1T{kc}", INTER)
    for kc in range(4):
        add(f"fcw1T{kc}", INTER)
    for kc in range(2):
        add(f"fsw2T{kc}", EMB)
    for kc in range(2):
        add(f"fcw2T{kc}", EMB)
    return lay, c


def _constf_layout():
    lay = {}
    c = 0

    def add(nm, cols):
        nonlocal c
        lay[nm] = (c, cols)
        c += cols
    for nm, nch in (("ses1", 2), ("seb1", 2), ("fcs1", 2), ("fcb1", 2),
                    ("ses2", 4), ("seb2", 4), ("fcs2", 4), ("fcb2", 4)):
        for kc in range(nch):
            add(f"{nm}{kc}", 1)
    add("b1h", 1)
    add("b2h", 1)
    add("b3h0", 1)
    add("b3h1", 1)
    add("mtop", 1)
    add("mbot", 1)
    add("identf", 128)
    return lay, c


def _actr_layout():
    """bf16 gathered activations [128, CA]."""
    lay = {}
    c = 0

    def add(nm, cols):
        nonlocal c
        lay[nm] = (c, cols)
        c += cols
    for kc in range(6):
        add(f"xmT{kc}", EM)
    for kc in range(3):
        add(f"attl{kc}", LS)
    for kc in range(6):
        add(f"xspT{kc}", LS)
    return lay, c


_LAY_R, _CR = _const_layout()
_LAY_F, _CF = _constf_layout()
_LAY_A, _CA = _actr_layout()

_CRB = _LAY_R["fsw1T0"][0]          # stage-1 smalls | SE weights split
_CAA = _LAY_A["xspT0"][0]           # xmT+attl | xspT split
_CW = 7 * EMB                       # constrw (bf16): wtr chunks + brow block


def build_program(solo=False, stages=4):
    nc = bacc.Bacc("TRN2", target_bir_lowering=False, debug=False)

    def din(name, shape, dt=F32R):
        return nc.dram_tensor(name, list(shape), dt, kind="ExternalInput").ap()

    constr_d = din("constr", [128, _CR])
    constf_d = din("constf", [128, _CF], F32)
    constw_d = din("constw", [128, _CW], BF16)
    actr_d = din("actr", [128, _CA], BF16)
    xp_d = din("xp", [128, 8 * HID], BF16)
    amp_d = din("amp", [128, 9 * C], BF16)
    gTb_d = din("gTb", [128, 9 * E], BF16)
    wstp_d = [din("wstp0", [128, 20 * EMB], BF16)] + \
             [din(f"wstp{i}", [128, 16 * EMB], BF16) for i in (1, 2)] + \
             [din("wstp3", [128, 8 * EMB], BF16)]
    w1sb_d = din("w1sb", [4, 128, 25 * 128], BF16)
    w2sb_d = din("w2sb", [2, 128, 25 * 128], BF16)
    w3sb_d = din("w3sb", [2, 128, 25 * 256], BF16)
    aallTb_d = din("aallTb", [128, 4 * (NREL + 1) * NN + 2 * E], BF16)
    tfb_d = din("tfb", [NN, TD], BF16)
    identb_d = din("identb", [128, 128], BF16)

    out_d = nc.dram_tensor("out", [256, EE], F32, kind="ExternalOutput").ap()

    groups = [[0, 1], [2, 3], [4, 5], [6, 7]]

    with tile.TileContext(nc) as tc:
      with tc.tile_pool(name="pconst", bufs=1) as pconst, \
           tc.tile_pool(name="pwork", bufs=1) as pwork, \
           tc.tile_pool(name="pwarm", bufs=1, space="PSUM") as pwarm, \
           tc.tile_pool(name="pdram", bufs=1, space="DRAM") as pdram:
        prgw_cm = tc.tile_pool(name="prgw", bufs=1)
        prgw = prgw_cm.__enter__()

        constr = pconst.tile([128, _CR], F32R)
        constf = pconst.tile([128, _CF], F32)
        constw = pconst.tile([128, _CW], BF16)
        identb = pconst.tile([128, 128], BF16)
        aallTb = pconst.tile([128, 4 * (NREL + 1) * NN + 2 * E], BF16)
        onesb = pconst.tile([128, 128], BF16)
        nc.vector.memset(onesb[:], 1.0)
        # scratch for the sigmoid-table swap dummy (see ep below)
        actwarm = pconst.tile([1, 1], F32)

        def cr(nm, rows=128):
            c0, cols = _LAY_R[nm]
            return constr[0:rows, c0:c0 + cols]

        def cf(nm, rows=128):
            c0, cols = _LAY_F[nm]
            return constf[0:rows, c0:c0 + cols]

        wtr = [constw[:, kc * EMB:(kc + 1) * EMB] for kc in range(6)]
        brow = constw[0:1, 6 * EMB:7 * EMB]
        onescol = onesb[:, 0:1]
        onesrow = onesb[0:1, :]
        g2T = cr("g2T", rows=EM)
        sumT = [cr(f"sumT{kc}") for kc in range(4)]
        sew = {nm: [cr(f"{nm}{kc}") for kc in range(n)]
               for nm, n in (("fsw1T", 4), ("fcw1T", 4), ("fsw2T", 2),
                             ("fcw2T", 2))}
        sev = {nm: [cf(f"{nm}{kc}") for kc in range(n)]
               for nm, n in (("ses1", 2), ("seb1", 2), ("fcs1", 2), ("fcb1", 2),
                             ("ses2", 4), ("seb2", 4), ("fcs2", 4),
                             ("fcb2", 4))}
        b1h = cf("b1h")
        b2h = cf("b2h")
        b3h = [cf("b3h0"), cf("b3h1")]
        ident = cf("identf")

        # conv pad images: memset early while the DVE is idle
        fusedp = [pwork.tile([128, PADW], BF16, tag=f"fusedp{i}",
                             name=f"fusedp{i}") for i in range(4)]
        for t_ in fusedp:
            nc.vector.memset(t_[:], 0.0)
        own1 = pwork.tile([128, PADW], BF16)
        oth1 = pwork.tile([128, PADW], BF16)
        own2 = pwork.tile([128, PADW], BF16)
        oth2 = pwork.tile([128, PADW], BF16)
        for t_ in (own1, oth1, own2, oth2):
            nc.vector.memset(t_[:], 0.0)

        # persistent intermediates: h0 kept as three partition-0 node-type
        # parts (entities / mentions / links); RGCN layer 0 contracts over
        # the parts with a 3-part adjacency tensor.
        eln_b = pwork.tile([E, D0], BF16)
        mrep_b = pwork.tile([EM, D0], BF16)
        link_b = pwork.tile([L, D0], BF16)
        h0p = [(mrep_b, EM), (link_b, L), (eln_b, E)]
        ectxT_sb = [pwork.tile([128, E], F32, tag=f"ectxT{i}", name=f"ectxT{i}")
                    for i in range(4)]
        z_sb = [pwork.tile([128, E], BF16, tag=f"z{i}", name=f"z{i}")
                for i in range(6)]
        easumT = pwork.tile([1, E], BF16)
        zt_sb = pwork.tile([E, HID], F32)
        ec2_sb = pwork.tile([E, EMB], F32)

        # ---------------- ordered DMA supply script (sync/SP queue) ---------
        # Exact consumption order, <=~800KB pieces. "touch" matmuls (fired at
        # chosen program points) read one column of an arriving DMA piece and
        # pulse the PE, so the p-state ramp never resets during DMA-bound
        # stretches.
        warm = pwarm.tile([1, 1], F32)
        tch = {}

        def fire(*names):
            for nm in names:
                nc.tensor.matmul(warm[:], tch[nm], tch[nm],
                                 start=True, stop=True)

        gTbt = prgw.tile([128, 9 * E], BF16)
        nc.scalar.dma_start(gTbt[:], gTb_d[:])
        amp = prgw.tile([128, 9 * C], BF16)
        nc.sync.dma_start(amp[:, 0:C], amp_d[:, 0:C])
        nc.sync.dma_start(constf[:], constf_d[:])
        for (p0, pn) in ((1, 2), (3, 2), (5, 2), (7, 2)):
            nc.sync.dma_start(amp[:, p0 * C:(p0 + pn) * C],
                              amp_d[:, p0 * C:(p0 + pn) * C])
        xp = prgw.tile([128, 8 * HID], BF16)
        for i, p0 in enumerate(range(0, 8, 2)):
            nc.sync.dma_start(xp[:, p0 * HID:(p0 + 2) * HID],
                              xp_d[:, p0 * HID:(p0 + 2) * HID])
            tch[f"xp{i}"] = xp[:, p0 * HID:p0 * HID + 1]
        for i, (p0, pn) in enumerate(((0, 3), (3, 4))):   # wtr + brow block
            nc.sync.dma_start(constw[:, p0 * EMB:(p0 + pn) * EMB],
                              constw_d[:, p0 * EMB:(p0 + pn) * EMB])
            tch[f"wtr{i}"] = constw[:, p0 * EMB:p0 * EMB + 1]
        nc.sync.dma_start(constr[:, 0:_CRB], constr_d[:, 0:_CRB])
        actr = prgw.tile([128, _CA], BF16)
        nc.sync.dma_start(actr[:, 0:_CAA], actr_d[:, 0:_CAA])
        tch["actr0"] = actr[:, 0:1]
        nc.sync.dma_start(actr[:, _CAA:_CA], actr_d[:, _CAA:_CA])
        tch["actr1"] = actr[:, _CAA:_CAA + 1]
        # RGCN weights
        wstp_t = [prgw.tile([128, 20 * EMB], BF16, tag="wstp0",
                            name="wstp0")] + \
                 [prgw.tile([128, 16 * EMB], BF16, tag=f"wstp{i}",
                            name=f"wstp{i}") for i in (1, 2)] + \
                 [prgw.tile([128, 8 * EMB], BF16, tag="wstp3", name="wstp3")]
        for layer in range(NLAYERS):
            ncols = (20 * EMB if layer == 0 else
                     16 * EMB if layer < 3 else 8 * EMB)
            hh = ncols // 2
            for i, (p0, pn) in enumerate(((0, hh), (hh, ncols - hh))):
                nc.sync.dma_start(wstp_t[layer][:, p0:p0 + pn],
                                  wstp_d[layer][:, p0:p0 + pn])
                tch[f"wstp{layer}{i}"] = wstp_t[layer][:, p0:p0 + 1]
        # SE weights
        nc.sync.dma_start(constr[:, _CRB:_CR], constr_d[:, _CRB:_CR])
        tch["sew"] = constr[:, _CRB:_CRB + 1].bitcast(F32)


        def ca(nm, rows=128):
            c0, cols = _LAY_A[nm]
            return actr[0:rows, c0:c0 + cols]

        xmT = [ca(f"xmT{kc}") for kc in range(6)]
        xspT = [ca(f"xspT{kc}") for kc in range(6)]
        attl = [ca(f"attl{kc}") for kc in range(3)]

        def fire_touches():
            for ap_sl in touches:
                nc.tensor.matmul(warm[:], ap_sl, ap_sl, start=True, stop=True)

        # ================= stage 1: gathered-row transforms =================
        with tc.tile_pool(name="pbig", bufs=1) as pbig:
            expm = pbig.tile([EM, EMB], F32R)
            sp_ps = []
            wsb = [pbig.tile([128, 1], F32, tag=f"wsb{i}", name=f"wsb{i}")
                   for i in range(4)]
            wsp = [pbig.tile([128, EMB], F32R, tag=f"wsp{i}", name=f"wsp{i}")
                   for i in range(4)]
            eaTb = pbig.tile([128, 8 * E], BF16)


            with tc.tile_pool(name="ps1b", bufs=1, space="PSUM") as ps1b:
                # ea^T computed directly: eaT[c, e] = sum_r attm[r, c] gT[r, e]
                # (amp is already [EMH, C]-major), in two 4-chunk psum tiles.
                # No [22, C] intermediate, no transposes.
                for cc in range(8):
                    eaT_p = ps1b.tile([128, E], F32, tag="eaTp", name="eaTp",
                                      bufs=2)
                    for kc in range(9):
                        rows = 128 if kc < 8 else 32
                        gt = gTbt[0:rows, kc * E:(kc + 1) * E]
                        nc.tensor.matmul(
                            eaT_p[:],
                            amp[0:rows, kc * C + cc * 128:kc * C + cc * 128 + 128],
                            gt, start=(kc == 0), stop=(kc == 8))
                    if cc % 2 == 0:
                        nc.scalar.copy(eaTb[:, cc * E:(cc + 1) * E], eaT_p[:])
                    else:
                        nc.vector.tensor_copy(out=eaTb[:, cc * E:(cc + 1) * E],
                                              in_=eaT_p[:])
                    if cc == 3:
                        fire("xp0", "xp1")
                    elif cc == 5:
                        fire("xp2")
                    elif cc == 7:
                        fire("xp3")
                # row sums of ea = column sums of ea^T (ones-vector matmuls)
                rs_p = ps1b.tile([1, E], F32, tag="rsp", name="rsp")
                for cc in range(8):
                    nc.tensor.matmul(rs_p[:], onesb[:, 0:1],
                                     eaTb[:, cc * E:(cc + 1) * E],
                                     start=(cc == 0), stop=(cc == 7))
                rsrow = pbig.tile([1, E], F32)
                nc.scalar.copy(rsrow[:], rs_p[:])
                rinvrow = pbig.tile([1, E], F32)
                nc.vector.tensor_scalar(out=rinvrow[:], in0=rsrow[:],
                                        scalar1=1e-5, scalar2=None, op0=ALU.add)
                nc.vector.reciprocal(rinvrow[:], rinvrow[:])
                easumr = pbig.tile([1, E], F32)
                nc.vector.tensor_tensor(out=easumr[:], in0=rsrow[:],
                                        in1=rinvrow[:], op=ALU.mult)
                nc.vector.tensor_copy(out=easumT[:], in_=easumr[:])
                # rinv as a [22,1] column for the zt row-scale
                rt_p = ps1b.tile([E, 1], F32, tag="rtp", name="rtp")
                nc.tensor.transpose(rt_p[:], rinvrow[:], ident[0:1, 0:1])
                rinv = pbig.tile([E, 1], F32)
                nc.scalar.copy(rinv[:], rt_p[:])
                # small late-use constants; placed here so their HWDGE slots
                # don't delay the early amp pieces
                nc.scalar.dma_start(eln_b[0:E, EMB:D0], tfb_d[0:E, :])
                nc.scalar.dma_start(mrep_b[0:EM, EMB:D0], tfb_d[E:E + EM, :])
                nc.scalar.dma_start(link_b[0:L, EMB:D0], tfb_d[E + EM:NN, :])
                nc.scalar.dma_start(aallTb[:], aallTb_d[:])
                nc.scalar.dma_start(identb[:], identb_d[:])

            with tc.tile_pool(name="ps1c", bufs=1, space="PSUM") as ps1c:
                # zT = ea @ x  [22, 768] (two 384-wide halves); rinv applied
                # on the copy out of PSUM
                zt_ps = [ps1c.tile([E, 384], F32, tag=f"zt_p{i}",
                                   name=f"zt_p{i}") for i in range(2)]
                for kc in range(8):
                    if kc == 3:
                        fire("wtr0")
                    elif kc == 6:
                        fire("wtr1")
                    xt = xp[:, kc * HID:(kc + 1) * HID]
                    for hh in range(2):
                        nc.tensor.matmul(zt_ps[hh][:],
                                         eaTb[:, kc * E:(kc + 1) * E],
                                         xt[:, hh * 384:(hh + 1) * 384],
                                         start=(kc == 0), stop=(kc == 7))
                fire("actr0")
                nc.scalar.activation(zt_sb[:, 0:384], zt_ps[0][:], AF.Copy,
                                     scale=rinv[:])
                nc.scalar.activation(zt_sb[:, 384:768], zt_ps[1][:], AF.Copy,
                                     scale=rinv[:])

            with tc.tile_pool(name="ps1a", bufs=1, space="PSUM") as ps1a:
                # mentions: mrep = x_m @ Wtr + b -> straight into h0b (bf16)
                mrep_p = ps1a.tile([EM, EMB], F32, tag="mrep", name="mrep")
                for kc in range(6):
                    nc.tensor.matmul(mrep_p[:], xmT[kc][:, 0:EM], wtr[kc][:],
                                     start=(kc == 0), stop=False)
                nc.tensor.matmul(mrep_p[:], onesrow[0:1, 0:EM], brow[:],
                                 start=False, stop=True)
                nc.scalar.activation(mrep_b[0:EM, 0:EMB], mrep_p[:], AF.Copy)
                nc.scalar.activation(expm[:], mrep_p[:], AF.Exp)
                # e_rep = ln(G2 @ exp(mrep))
                ep_p = ps1a.tile([E, EMB], F32, tag="ep", name="ep")
                nc.tensor.matmul(ep_p[:], g2T[:], expm[:], start=True, stop=True)
                nc.scalar.activation(eln_b[0:E, 0:EMB], ep_p[:], AF.Ln)
                # swap to the sigmoid table now (every later act is in it);
                # reading ep_p pins this after the Ln in the schedule
                nc.scalar.activation(actwarm[:], ep_p[0:1, 0:1], AF.Sigmoid)

                # w = colsum(attl) / 384
                for mc in range(4):
                    w_p = ps1a.tile([128, 1], F32, tag="w_p", name="w_p", bufs=1)
                    for kc in range(3):
                        nc.tensor.matmul(w_p[:],
                                         attl[kc][:, mc * 128:(mc + 1) * 128],
                                         onescol[:],
                                         start=(kc == 0), stop=(kc == 2))
                    nc.scalar.activation(wsb[mc][:], w_p[:], AF.Copy,
                                         scale=1.0 / (H * SPAN))
                fire("actr1")
                # spans: sp = x_span @ Wtr + b
                for mc in range(4):
                    if mc > 0:
                        fire(f"wstp0{mc - 1}" if mc < 3 else "wstp10")
                    sp_p = ps1a.tile([128, EMB], F32, tag="sp_p", name="sp_p",
                                     bufs=3)
                    for kc in range(6):
                        nc.tensor.matmul(sp_p[:],
                                         xspT[kc][:, mc * 128:(mc + 1) * 128],
                                         wtr[kc][:], start=(kc == 0), stop=False)
                    nc.tensor.matmul(sp_p[:], onesrow[:], brow[:],
                                     start=False, stop=True)
                    spc = pbig.tile([128, EMB], F32, tag="spc", name="spc",
                                    bufs=4)
                    nc.scalar.copy(spc[:], sp_p[:])
                    sp_ps.append(spc)
                # wsp = psum(sp) * w ; link = SUM^T @ wsp
                for mc in range(4):
                    nc.vector.tensor_scalar(out=wsp[mc][:], in0=sp_ps[mc][:],
                                            scalar1=wsb[mc][:], scalar2=None,
                                            op0=ALU.mult)
                link_p = ps1a.tile([L, EMB], F32, tag="link", name="link")
                for kc in range(4):
                    nc.tensor.matmul(link_p[:], sumT[kc][:], wsp[kc][:],
                                     start=(kc == 0), stop=(kc == 3))
                nc.scalar.activation(link_b[0:L, 0:EMB], link_p[:], AF.Copy)
                fire("wstp11")


        if stages >= 2:
          # ================= stage 2: RGCN (4 layers, y col-half pipelined) ==
          ecT = [pwork.tile([128, E], F32R, tag=f"ecT{i}", name=f"ecT{i}")
                 for i in range(4)]
          with tc.tile_pool(name="prg", bufs=2) as prg, \
               tc.tile_pool(name="psr", bufs=1, space="PSUM") as psr:
              h = None
              UW = (NREL + 1) * NN
              for layer in range(NLAYERS):
                  kcs = _KC0 if layer == 0 else _KC1
                  nk = len(kcs)
                  wstp = wstp_t[layer]
                  nrel_l = 2 if layer == 3 else NREL + 1
                  uw_l = 2 * E if layer == 3 else UW
                  wst_t = [wstp[:, (r * nk + si) * EMB:(r * nk + si + 1) * EMB]
                           for r in range(nrel_l) for si in range(nk)]
                  # u = h^T @ A_allT per d-chunk. Layer 3 only needs entity
                  # output rows, which only rel-0 and the self-loop feed.
                  u_sb = []
                  for si, (s0, sl) in enumerate(kcs):
                      u_p = psr.tile([128, (NREL + 1) * NN], F32, tag="u_p",
                                     name="u_p", bufs=2)
                      if layer == 0:
                          for pi, (pt, rows) in enumerate(h0p):
                              nc.tensor.matmul(u_p[0:sl, 0:UW],
                                               pt[0:rows, s0:s0 + sl],
                                               aallTb[0:rows,
                                                      (pi + 1) * UW:
                                                      (pi + 2) * UW],
                                               start=(pi == 0), stop=(pi == 2))
                      elif layer == 3:
                          nc.tensor.matmul(u_p[0:sl, 0:2 * E],
                                           h[0:NN, s0:s0 + sl],
                                           aallTb[0:NN, 4 * UW:4 * UW + 2 * E],
                                           start=True, stop=True)
                      else:
                          nc.tensor.matmul(u_p[0:sl, 0:UW],
                                           h[0:NN, s0:s0 + sl],
                                           aallTb[0:NN, 0:UW],
                                           start=True, stop=True)
                      u = prg.tile([128, (NREL + 1) * NN], BF16, tag=f"u{si}",
                                   name=f"u{si}")
                      if si % 2 == 0:
                          nc.scalar.copy(u[0:sl, 0:uw_l], u_p[0:sl, 0:uw_l])
                      else:
                          nc.vector.tensor_copy(out=u[0:sl, 0:uw_l],
                                                in_=u_p[0:sl, 0:uw_l])
                      u_sb.append(u)
                  # y = sum_r (u_r)^T @ Wst_r, in two column halves so the
                  # relu of half 0 overlaps the matmuls of half 1
                  if layer < 2:
                      fire(f"wstp{layer + 2}0", f"wstp{layer + 2}1")
                  elif layer == 2:
                      fire("sew")
                  # deferred e_ctx chain, one piece per inter-layer window
                  if layer == 0:
                      for kc in range(6):
                          ztp = psr.tile([128, E], F32, tag="tp22", name="ztp",
                                         bufs=2)
                          nc.tensor.transpose(ztp[:],
                                              zt_sb[:, kc * 128:(kc + 1) * 128],
                                              ident[0:E, 0:E])
                          if kc % 2 == 0:
                              nc.scalar.copy(z_sb[kc][:], ztp[:])
                          else:
                              nc.vector.tensor_copy(out=z_sb[kc][:], in_=ztp[:])
                  elif layer == 1:
                      ec2_p = psr.tile([E, EMB], F32, tag="ec2", name="ec2")
                      for kc in range(6):
                          nc.tensor.matmul(ec2_p[:], z_sb[kc][:], wtr[kc][:],
                                           start=(kc == 0), stop=False)
                      nc.tensor.matmul(ec2_p[:], easumT[:], brow[:],
                                       start=False, stop=True)
                      nc.scalar.copy(ec2_sb[:], ec2_p[:])
                  elif layer == 2:
                      for mc in range(4):
                          ecp = psr.tile([128, E], F32, tag="tp22", name="ecp",
                                         bufs=2)
                          nc.tensor.transpose(ecp[:],
                                              ec2_sb[:, mc * 128:(mc + 1) * 128],
                                              ident[0:E, 0:E])
                          if mc % 2 == 0:
                              nc.scalar.copy(ectxT_sb[mc][:], ecp[:])
                          else:
                              nc.vector.tensor_copy(out=ectxT_sb[mc][:],
                                                    in_=ecp[:])
                  nrows = E if layer == 3 else NN
                  rw = E if layer == 3 else NN
                  hn = prg.tile([NN, EMB], BF16, tag="h_next", name="h_next")
                  for yh in range(2):
                      y_p = psr.tile([NN, 256], F32, tag=f"y_p{yh}",
                                     name=f"y_p{yh}")
                      n_mm = nrel_l * nk
                      k_mm = 0
                      for si, (s0, sl) in enumerate(kcs):
                          for r in range(nrel_l):
                              nc.tensor.matmul(
                                  y_p[0:nrows, :],
                                  u_sb[si][0:sl, r * rw:r * rw + nrows],
                                  wst_t[r * nk + si][0:sl,
                                                     yh * 256:(yh + 1) * 256],
                                  start=(k_mm == 0), stop=(k_mm == n_mm - 1))
                              k_mm += 1
                      nc.scalar.activation(hn[0:nrows, yh * 256:(yh + 1) * 256],
                                           y_p[0:nrows, :], AF.Relu)
                  h = hn

              # entity_struT + e_ctxT -> ecT
              for mc in range(4):
                  tp = psr.tile([128, E], F32, tag="tp22", name="est", bufs=2)
                  nc.tensor.matmul(tp[:], h[0:E, mc * 128:(mc + 1) * 128],
                                   identb[0:E, 0:E], start=True, stop=True)
                  nc.vector.tensor_tensor(out=ecT[mc][:], in0=tp[:],
                                          in1=ectxT_sb[mc][:], op=ALU.add)

        prgw_cm.__exit__(None, None, None)

        if stages >= 3:
          # ================= stage 3: fmap + SE =================
          fmap = [pwork.tile([128, EE], F32R, tag=f"fmap{i}", name=f"fmap{i}")
                  for i in range(4)]
          pooled = [pwork.tile([128, 1], F32R, tag=f"pool{i}", name=f"pool{i}")
                    for i in range(4)]
          for mc in range(4):
              o6v = fmap[mc][:].rearrange("p (i j) -> p i j", i=E)
              in0 = ecT[mc][:].rearrange("p (i j) -> p i j", j=1) \
                  .to_broadcast([128, E, E])
              in1 = ecT[mc][:].rearrange("p (o j) -> p o j", o=1) \
                  .to_broadcast([128, E, E])
              nc.vector.tensor_tensor(out=o6v, in0=in0, in1=in1, op=ALU.mult)
              rs = pwork.tile([128, 1], F32, tag=f"rs{mc}", name=f"rs{mc}")
              nc.vector.tensor_reduce(rs[:], ecT[mc][:], mybir.AxisListType.X,
                                      ALU.add)
              nc.scalar.activation(pooled[mc][:], rs[:], AF.Square, scale=1.0 / E)

          with tc.tile_pool(name="pse", bufs=1, space="PSUM") as pse:
              # c-path first (tiny serial chain, hides under fmap/s1p);
              # seb2 is folded into the fcb2 host constant.
              c1_sb = [pwork.tile([128, 1], F32R, tag=f"c1_{i}", name=f"c1_{i}")
                       for i in range(2)]
              for oc in range(2):
                  c1_p = pse.tile([128, 1], F32, tag="cp", name="c1p", bufs=2)
                  for mc in range(4):
                      nc.tensor.matmul(c1_p[:],
                                       sew["fcw1T"][mc][:, oc * 128:(oc + 1) * 128]
                                       .bitcast(F32),
                                       pooled[mc][:].bitcast(F32),
                                       start=(mc == 0), stop=(mc == 3))
                  nc.scalar.activation(c1_sb[oc][:], c1_p[:], AF.Relu,
                                       bias=sev["fcb1"][oc][:],
                                       scale=sev["fcs1"][oc][:])
              cbb = [pwork.tile([128, 1], F32, tag=f"cbb{i}", name=f"cbb{i}")
                     for i in range(4)]
              # s-path; s2's first half starts as soon as s1_sb[0] is ready
              s1_sb = [pwork.tile([128, EE], F32R, tag=f"s1_{i}", name=f"s1_{i}")
                       for i in range(2)]
              for oc in range(2):
                  s1_p = pse.tile([128, EE], F32, tag="s1p", name="s1p", bufs=2)
                  for mc in range(4):
                      nc.tensor.matmul(s1_p[:],
                                       sew["fsw1T"][mc][:, oc * 128:(oc + 1) * 128],
                                       fmap[mc][:], start=(mc == 0), stop=(mc == 3))
                  nc.scalar.activation(s1_sb[oc][:], s1_p[:], AF.Relu,
                                       bias=sev["seb1"][oc][:],
                                       scale=sev["ses1"][oc][:])
              for mc in range(4):
                  c2_p = pse.tile([128, 1], F32, tag="cp", name="c2p", bufs=2)
                  for kc in range(2):
                      nc.tensor.matmul(c2_p[:],
                                       sew["fcw2T"][kc][:, mc * 128:(mc + 1) * 128]
                                       .bitcast(F32),
                                       c1_sb[kc][:].bitcast(F32),
                                       start=(kc == 0), stop=(kc == 1))
                  nc.scalar.activation(cbb[mc][:], c2_p[:], AF.Identity,
                                       bias=sev["fcb2"][mc][:],
                                       scale=sev["fcs2"][mc][:])
              s2_ps = [pse.tile([128, EE], F32, tag="s2p", name=f"s2p{mc}",
                                bufs=3) for mc in range(4)]
              for mc in range(4):
                  nc.tensor.matmul(s2_ps[mc][:],
                                   sew["fsw2T"][0][:, mc * 128:(mc + 1) * 128],
                                   s1_sb[0][:], start=True, stop=False)
              for mc in range(4):
                  nc.tensor.matmul(s2_ps[mc][:],
                                   sew["fsw2T"][1][:, mc * 128:(mc + 1) * 128],
                                   s1_sb[1][:], start=False, stop=True)
                  sig = pwork.tile([128, EE], F32, tag="sig", name="sig", bufs=2)
                  nc.scalar.activation(sig[:], s2_ps[mc][:], AF.Sigmoid,
                                       bias=cbb[mc][:], scale=sev["ses2"][mc][:])
                  outv = fusedp[mc][:].rearrange("p (i j) -> p i j", j=26)[:, 2:24,
                                                                          2:24]
                  nc.vector.tensor_tensor(
                      out=outv,
                      in0=fmap[mc][:].rearrange("p (i j) -> p i j", i=E),
                      in1=sig[:].rearrange("p (i j) -> p i j", i=E),
                      op=ALU.mult)

        if stages >= 4:
          # ================= stage 4: conv stack =================
          def tap_rows(padt, tap, r0, nr):
              dy, dx = tap // 5, tap % 5
              return padt[:].rearrange("p (i j) -> p i j", j=26)[
                  :, dy + r0:dy + r0 + nr, dx:dx + 22]

          def tap_view(padt, tap):
              return tap_rows(padt, tap, 0, 22)

          with tc.tile_pool(name="pcw", bufs=1) as pcw, \
               tc.tile_pool(name="psc", bufs=1, space="PSUM") as psc:
              w1, w2, w3 = [], [], []
              for kc in range(4):
                  t = pcw.tile([128, 25 * 128], BF16, tag=f"w1_{kc}",
                               name=f"w1_{kc}")
                  nc.sync.dma_start(t[:], w1sb_d[kc])
                  w1.append(t)
              for kc in range(2):
                  t = pcw.tile([128, 25 * 128], BF16, tag=f"w2_{kc}",
                               name=f"w2_{kc}")
                  nc.sync.dma_start(t[:], w2sb_d[kc])
                  w2.append(t)
              for kc in range(2):
                  t = pcw.tile([128, 25 * 256], BF16, tag=f"w3_{kc}",
                               name=f"w3_{kc}")
                  nc.sync.dma_start(t[:], w3sb_d[kc])
                  w3.append(t)
              mtop = cf("mtop")
              mbot = cf("mbot")

              def interior_rows(t_, r0, nr):
                  return t_[:].rearrange("p (i j) -> p i j", j=26)[
                      :, 2 + r0:2 + r0 + nr, 2:24]

              def gather_combine(src_flat, dst_pad, gg, rh, nm, rows=None):
                  """relu'd flat rows rh -> allgather -> mask-combine into
                  dst rows rh, issued right after the producing relu."""
                  r0, nr = rows
                  rb = pdram.tile([128, nr * 22], BF16, tag=f"{nm}b{rh}",
                                  name=f"{nm}b{rh}")
                  nc.sync.dma_start(rb[:], src_flat[:])
                  if solo:
                      nc.sync.dma_start(gg[0:128, :], rb[:])
                      nc.gpsimd.dma_start(gg[128:256, :], rb[:])
                  else:
                      nc.gpsimd.collective_compute(
                          "AllGather", ALU.bypass, replica_groups=groups,
                          ins=[rb[:].opt()], outs=[gg[:].opt()])
                  gt = pcw.tile([128, nr * 22], BF16, tag=f"{nm}gt{rh}",
                                name=f"{nm}gt{rh}")
                  gb = pcw.tile([128, nr * 22], BF16, tag=f"{nm}gb{rh}",
                                name=f"{nm}gb{rh}")
                  nc.sync.dma_start(gt[:], gg[0:128, :])
                  nc.gpsimd.dma_start(gb[:], gg[128:256, :])
                  tmp = pcw.tile([128, nr * 22], F32, tag=f"{nm}tmp{rh}",
                                 name=f"{nm}tmp{rh}")
                  nc.vector.tensor_scalar(out=tmp[:], in0=gb[:],
                                          scalar1=mbot[:], scalar2=None,
                                          op0=ALU.mult)
                  nc.vector.scalar_tensor_tensor(
                      out=interior_rows(dst_pad, r0, nr), in0=gt[:],
                      scalar=mtop[:], in1=tmp[:], op0=ALU.mult, op1=ALU.add)

              # conv1 rows (0,18),(18,4): the tiny second half's gather
              # launches right at conv1's end, so its readback chain hides
              # under conv2's first-half taps.
              ROWS = ((0, 17), (17, 5))

              # conv1 (my half of 256 out channels), in two row halves; each
              # half's AllGather + readback starts as soon as the half is done
              r1g = [pdram.tile([256, nr * 22], BF16, tag=f"r1g{i}",
                                name=f"r1g{i}")
                     for i, (r0, nr) in enumerate(ROWS)]
              for rh, (r0, nr) in enumerate(ROWS):
                  r1_p = psc.tile([128, nr * 22], F32, tag="convp",
                                  name="convp", bufs=4)
                  first = True
                  for kc in range(4):
                      for tap in range(25):
                          nc.tensor.matmul(r1_p[:],
                                           w1[kc][:, tap * 128:(tap + 1) * 128],
                                           tap_rows(fusedp[kc], tap, r0, nr),
                                           start=first,
                                           stop=(kc == 3 and tap == 24))
                          first = False
                  o1f = pcw.tile([128, nr * 22], BF16, tag=f"o1f{rh}",
                                 name=f"o1f{rh}")
                  nc.scalar.activation(o1f[:], r1_p[:], AF.Relu, bias=b1h[:])
                  nc.vector.tensor_copy(out=interior_rows(own1, r0, nr),
                                        in_=o1f[:].rearrange(
                                            "p (i j) -> p i j", j=22))
                  gather_combine(o1f, oth1, r1g[rh], rh, "r1", (r0, nr))

              # conv2: own-input taps for both row-halves first (no gather
              # dependency), then other-input taps; output in row halves.
              # conv2 out rows (0-8, 9-21): the first half's own AND other
              # taps read only rows <=12 of r1, i.e. conv1-half0 + gather#0 —
              # no wait on conv1-half1's gather.
              ROWS2 = ((0, 15), (15, 7))
              r2g = [pdram.tile([256, nr * 22], BF16, tag=f"r2g{i}",
                                name=f"r2g{i}")
                     for i, (r0, nr) in enumerate(ROWS2)]
              r2_ps = [psc.tile([128, nr * 22], F32, tag=f"convp2_{i}",
                                name=f"convp2_{i}")
                       for i, (r0, nr) in enumerate(ROWS2)]
              # rh1's own-taps first (ready at conv1 end) to widen the
              # window that hides gather#1's readback chain; rh0 is fully
              # ready (gather#0 landed mid-conv1) and runs next.
              r10, n1 = ROWS2[1]
              for tap in range(25):
                  nc.tensor.matmul(r2_ps[1][:],
                                   w2[0][:, tap * 128:(tap + 1) * 128],
                                   tap_rows(own1, tap, r10, n1),
                                   start=(tap == 0), stop=False)
              r0, nr = ROWS2[0]
              for tap in range(25):
                  nc.tensor.matmul(r2_ps[0][:],
                                   w2[0][:, tap * 128:(tap + 1) * 128],
                                   tap_rows(own1, tap, r0, nr),
                                   start=(tap == 0), stop=False)
              for tap in range(25):
                  nc.tensor.matmul(r2_ps[0][:],
                                   w2[1][:, tap * 128:(tap + 1) * 128],
                                   tap_rows(oth1, tap, r0, nr),
                                   start=False, stop=(tap == 24))
              o2f0 = pcw.tile([128, nr * 22], BF16, tag="o2f0", name="o2f0")
              nc.scalar.activation(o2f0[:], r2_ps[0][:], AF.Relu, bias=b2h[:])
              nc.vector.tensor_copy(out=interior_rows(own2, r0, nr),
                                    in_=o2f0[:].rearrange(
                                        "p (i j) -> p i j", j=22))
              gather_combine(o2f0, oth2, r2g[0], 0, "r2", (r0, nr))
              for tap in range(25):
                  nc.tensor.matmul(r2_ps[1][:],
                                   w2[1][:, tap * 128:(tap + 1) * 128],
                                   tap_rows(oth1, tap, r10, n1),
                                   start=False, stop=(tap == 24))
              o2f1 = pcw.tile([128, n1 * 22], BF16, tag="o2f1", name="o2f1")
              nc.scalar.activation(o2f1[:], r2_ps[1][:], AF.Relu, bias=b2h[:])
              nc.vector.tensor_copy(out=interior_rows(own2, r10, n1),
                                    in_=o2f1[:].rearrange(
                                        "p (i j) -> p i j", j=22))
              gather_combine(o2f1, oth2, r2g[1], 1, "r2", (r10, n1))

              # conv3 (my 256 of 512 out channels): own-input taps for both
              # out chunks first, then other-input taps per (oc, row-half)
              # with relu+store pipelined per row half (short tail).
              r3_ps = [psc.tile([128, ROWS[rh][1] * 22], F32, tag="convp",
                                name=f"convp3_{oc}{rh}", bufs=4)
                       for oc in range(2) for rh in range(2)]
              for oc in range(2):
                  for rh, (r0, nr) in enumerate(ROWS):
                      for tap in range(25):
                          nc.tensor.matmul(
                              r3_ps[oc * 2 + rh][:],
                              w3[0][:, tap * 256 + oc * 128:
                                    tap * 256 + (oc + 1) * 128],
                              tap_rows(own2, tap, r0, nr),
                              start=(tap == 0), stop=False)
              for oc in range(2):
                  for rh, (r0, nr) in enumerate(ROWS):
                      for tap in range(25):
                          nc.tensor.matmul(
                              r3_ps[oc * 2 + rh][:],
                              w3[1][:, tap * 256 + oc * 128:
                                    tap * 256 + (oc + 1) * 128],
                              tap_rows(oth2, tap, r0, nr),
                              start=False, stop=(tap == 24))
                      o_sb = pcw.tile([128, nr * 22], F32, tag="osb",
                                      name="osb", bufs=4)
                      nc.scalar.activation(o_sb[:], r3_ps[oc * 2 + rh][:],
                                           AF.Relu, bias=b3h[oc][:])
                      nc.sync.dma_start(
                          out_d[oc * 128:(oc + 1) * 128,
                                r0 * 22:(r0 + nr) * 22], o_sb[:])

    nc.compile()
    return nc


_NC_CACHE = None


def _get_program():
    global _NC_CACHE
    if _NC_CACHE is None:
        _NC_CACHE = build_program()
    return _NC_CACHE


def _prep_shared(w):
    """Packed weights/constants identical on every core."""
    ADJ = _build_adj()
    out = {}
    constr = np.zeros((128, _CR), np.float32)

    def put(nm, arr):
        c0, cols = _LAY_R[nm]
        r, cc = arr.shape
        constr[0:r, c0:c0 + cc] = arr
    g2T = np.zeros((EM, E), np.float32)
    for e in range(E):
        g2T[e * M:(e + 1) * M, e] = 1.0
    put("g2T", g2T)
    sumT = np.kron(np.eye(L, dtype=np.float32), np.ones((SPAN, 1), np.float32))
    for kc in range(4):
        put(f"sumT{kc}", sumT[kc * 128:(kc + 1) * 128])
    for nm, arr, nch in (("fsw1T", w['fs_w1'].T, 4), ("fcw1T", w['fc_w1'].T, 4),
                         ("fsw2T", w['fs_w2'].T, 2), ("fcw2T", w['fc_w2'].T, 2)):
        for kc in range(nch):
            put(f"{nm}{kc}", np.ascontiguousarray(arr[kc * 128:(kc + 1) * 128]))
    out['constr'] = constr

    constw = np.zeros((128, _CW), np.float32)
    wt = w['W_trans']
    for kc in range(6):
        constw[:, kc * EMB:(kc + 1) * EMB] = wt[kc * 128:(kc + 1) * 128]
    constw[0, 6 * EMB:7 * EMB] = w['b_trans']
    out['constw'] = constw.astype(ml_dtypes.bfloat16)

    gTb = np.zeros((128, 9 * E), np.float32)
    gT = np.zeros((EMH, E), np.float32)
    for e in range(E):
        gT[e * M * H:(e + 1) * M * H, e] = 1.0 / (M * H)
    for kc in range(9):
        r = min(128, EMH - kc * 128)
        gTb[0:r, kc * E:(kc + 1) * E] = gT[kc * 128:kc * 128 + r]
    out['gTb'] = gTb.astype(ml_dtypes.bfloat16)
    aall = np.concatenate(
        [ADJ[r].T for r in range(NREL)] + [np.eye(NN, dtype=np.float32)],
        axis=1)
    UW = (NREL + 1) * NN
    aallp = np.zeros((128, 4 * UW + 2 * E), np.float32)
    aallp[0:NN, 0:UW] = aall
    aallp[0:EM, UW:2 * UW] = aall[E:E + EM]
    aallp[0:L, 2 * UW:3 * UW] = aall[E + EM:NN]
    aallp[0:E, 3 * UW:4 * UW] = aall[0:E]
    # layer-3 entity-only columns: [A0^T[:, :E] | I[:, :E]]
    aallp[0:NN, 4 * UW:4 * UW + E] = aall[:, 0:E]
    aallp[0:NN, 4 * UW + E:4 * UW + 2 * E] = aall[:, NREL * NN:NREL * NN + E]
    out['aallTb'] = aallp.astype(ml_dtypes.bfloat16)
    out['tfb'] = np.ascontiguousarray(
        w['type_embed'][_TYPES]).astype(ml_dtypes.bfloat16)
    out['identb'] = np.eye(128, dtype=np.float32).astype(ml_dtypes.bfloat16)

    constf = np.zeros((128, _CF), np.float32)

    def putf(nm, arr):
        c0, cols = _LAY_F[nm]
        constf[0:arr.shape[0], c0:c0 + 1] = arr.reshape(-1, 1)
    vecs = {"ses1": w['fs_g1'], "seb1": w['fs_b1'] * w['fs_g1'] + w['fs_be1'],
            "fcs1": w['fc_g1'], "fcb1": w['fc_b1'] * w['fc_g1'] + w['fc_be1'],
            "ses2": w['fs_g2'], "seb2": w['fs_b2'] * w['fs_g2'] + w['fs_be2'],
            "fcs2": w['fc_g2'],
            "fcb2": (w['fc_b2'] * w['fc_g2'] + w['fc_be2'] +
                     w['fs_b2'] * w['fs_g2'] + w['fs_be2'])}
    for nm, v in vecs.items():
        nch = 2 if v.shape[0] == INTER else 4
        for kc in range(nch):
            putf(f"{nm}{kc}", v[kc * 128:(kc + 1) * 128])
    out['constf_base'] = constf

    for layer in range(NLAYERS):
        din_l = D0 if layer == 0 else EMB
        kcs = _KC0 if layer == 0 else _KC1
        nk = len(kcs)
        Wst = w['rgcn_Wrel0'].reshape(NREL * D0, EMB) if layer == 0 else \
            w['rgcn_Wrel'][layer - 1].reshape(NREL * EMB, EMB)
        Wself = w['rgcn_Wself0'] if layer == 0 else w['rgcn_Wself'][layer - 1]
        if layer == 3:
            # entity rows only need rel-0 and the self-loop
            p = np.zeros((128, 2 * nk * EMB), np.float32)
            for si, (s0, sl) in enumerate(kcs):
                p[0:sl, si * EMB:(si + 1) * EMB] = Wst[s0:s0 + sl]
                p[0:sl, (nk + si) * EMB:(nk + si + 1) * EMB] = \
                    Wself[s0:s0 + sl]
        else:
            p = np.zeros((128, (NREL + 1) * nk * EMB), np.float32)
            for r in range(NREL):
                for si, (s0, sl) in enumerate(kcs):
                    p[0:sl, (r * nk + si) * EMB:(r * nk + si + 1) * EMB] = \
                        Wst[r * din_l + s0:r * din_l + s0 + sl]
            for si, (s0, sl) in enumerate(kcs):
                p[0:sl, (NREL * nk + si) * EMB:(NREL * nk + si + 1) * EMB] = \
                    Wself[s0:s0 + sl]
        out[f'wstp{layer}'] = p.astype(ml_dtypes.bfloat16)
    return out


def _prep_conv_half(w, half, constf_base):
    out = {}
    w1 = w['cr_w1'][half * 128:(half + 1) * 128]
    out['w1sb'] = np.ascontiguousarray(
        w1.transpose(1, 2, 3, 0).reshape(4, 128, 25 * 128)).astype(ml_dtypes.bfloat16)
    # conv2/conv3 weights in (own-input-half, other-input-half) chunk order
    w2 = w['cr_w2'][half * 128:(half + 1) * 128]
    w2p = w2.transpose(1, 2, 3, 0).reshape(2, 128, 25 * 128)
    order = [half, 1 - half]
    out['w2sb'] = np.ascontiguousarray(w2p[order]).astype(ml_dtypes.bfloat16)
    w3 = w['cr_w3'][half * 256:(half + 1) * 256]
    w3p = w3.transpose(1, 2, 3, 0).reshape(2, 128, 25 * 256)
    out['w3sb'] = np.ascontiguousarray(w3p[order]).astype(ml_dtypes.bfloat16)
    constf = constf_base.copy()

    def putf(nm, arr):
        c0, cols = _LAY_F[nm]
        constf[0:arr.shape[0], c0:c0 + 1] = arr.reshape(-1, 1)
    putf("b1h", w['cr_b1'][half * 128:(half + 1) * 128])
    putf("b2h", w['cr_b2'][half * 128:(half + 1) * 128])
    putf("b3h0", w['cr_b3'][half * 256:half * 256 + 128])
    putf("b3h1", w['cr_b3'][half * 256 + 128:half * 256 + 256])
    putf("mtop", np.full(128, float(half), np.float32))
    putf("mbot", np.full(128, float(1 - half), np.float32))
    c0, cols = _LAY_F["identf"]
    constf[:, c0:c0 + 128] = np.eye(128, dtype=np.float32)
    out['constf'] = constf
    return out


def _prep_doc(x, att, mi, ls):
    out = {}
    mif = mi.reshape(EM)
    attm = np.ascontiguousarray(
        att[:, mif, :].transpose(1, 0, 2).reshape(EMH, C))
    amp = np.zeros((128, 9 * C), np.float32)
    for kc in range(9):
        r = min(128, EMH - kc * 128)
        amp[0:r, kc * C:kc * C + C] = attm[kc * 128:kc * 128 + r]
    out['amp'] = amp.astype(ml_dtypes.bfloat16)
    idx = ls[:, None] + np.arange(SPAN)
    idxf = idx.reshape(LS)
    rows = att[:, idxf, :].reshape(H, L, SPAN, C)
    blocks = np.take_along_axis(rows, idx[None, :, None, :], axis=3)
    attl = blocks.transpose(0, 2, 1, 3).reshape(HS, LS)
    xmT = x[mif].T
    xspT = x[idxf].T
    actr = np.zeros((128, _CA), np.float32)

    def put(nm, arr):
        c0, cols = _LAY_A[nm]
        actr[0:arr.shape[0], c0:c0 + arr.shape[1]] = arr
    for kc in range(6):
        put(f"xmT{kc}", xmT[kc * 128:(kc + 1) * 128])
        put(f"xspT{kc}", xspT[kc * 128:(kc + 1) * 128])
    for kc in range(3):
        put(f"attl{kc}", attl[kc * 128:(kc + 1) * 128])
    out['actr'] = actr.astype(ml_dtypes.bfloat16)
    xpk = np.zeros((128, 8 * HID), np.float32)
    for kc in range(8):
        xpk[:, kc * HID:(kc + 1) * HID] = x[kc * 128:(kc + 1) * 128]
    out['xp'] = xpk.astype(ml_dtypes.bfloat16)
    return out


def build_in_maps(inputs):
    w = {}
    for k, v in inputs.items():
        a = np.asarray(v)
        w[k] = a if a.dtype in (np.int32, np.int64) else \
            np.asarray(a, np.float32)
    shared = _prep_shared(w)
    constf_base = shared.pop('constf_base')
    halves = [_prep_conv_half(w, h, constf_base) for h in range(2)]
    seq = np.asarray(inputs['sequence_output'], np.float32)
    att = np.asarray(inputs['attention'], np.float32)
    mi = np.asarray(inputs['mention_idx']).astype(np.int64)
    ls = np.asarray(inputs['link_start']).astype(np.int64)
    docs = [_prep_doc(seq[n], att[n], mi[n], ls[n]) for n in range(NB)]
    in_maps = []
    for core in range(N_CORES):
        n, half = core // 2, core % 2
        m = dict(shared)
        m.update(halves[half])
        m.update(docs[n])
        in_maps.append({k: (np.ascontiguousarray(v) if v.dtype == ml_dtypes.bfloat16
                            else np.ascontiguousarray(v, np.float32))
                        for k, v in m.items()})
    return in_maps


def kernel(**inputs):
    nc = _get_program()
    in_maps = build_in_maps(inputs)
    res = run_bass_kernel_spmd(nc, in_maps, list(range(N_CORES)))
    out = np.zeros((NB, EMB, E, E), np.float32)
    for core in range(N_CORES):
        n, half = core // 2, core % 2
        out[n, half * 256:(half + 1) * 256] = \
            res.results[core]["out"].reshape(256, E, E)
    return out
